# revision 41
# baseline (speedup 1.0000x reference)
"""PointCloudDiscriminator Trainium2 Bass kernel (SPMD 8 cores, 2 clouds/core)."""
import numpy as np
from contextlib import ExitStack

import concourse.bass as bass
import concourse.bacc as bacc_mod
import concourse.tile as tile
import concourse.mybir as mybir
from concourse import bass_isa

F32 = mybir.dt.float32
U32 = mybir.dt.uint32
I16 = mybir.dt.int16
AF = mybir.ActivationFunctionType
OP = mybir.AluOpType
AX = mybir.AxisListType
EPS = 1e-5

B, N, S1, S2, K = 16, 8192, 512, 128, 16
NCORES = 8
BC = B // NCORES
C1A, C1B = 64, 128
C2A, C2B = 128, 256


def prep_common_weights(inp):
    f = np.float32
    w = {}
    A = lambda x: np.ascontiguousarray(np.asarray(x, f))
    pad4 = lambda a: np.concatenate([a, np.zeros((1, a.shape[1]), f)], 0)
    w1 = A(inp['sa1_c1_w'])
    w['w1s_T'] = A((w1[:, :3] + w1[:, 3:]).T)
    w['w1x_T'] = A(w1[:, :3].T)
    w['w2_T'] = A(np.asarray(inp['sa1_c2_w'], f).T)
    w['bn1_g'] = A(inp['sa1_bn_g']); w['bn1_be'] = A(inp['sa1_bn_be'])
    w['b1c2'] = A(inp['sa1_c2_b'])
    w2c1 = A(inp['sa2_c1_w'])
    w['A2x_T'] = pad4(A(w2c1[:, :3].T)); w['A2xn_T'] = A(-w['A2x_T'])
    w['A2p_T'] = A(w2c1[:, 3:].T)
    w['bn2_g'] = A(inp['sa2_bn_g']); w['bn2_be'] = A(inp['sa2_bn_be'])
    w2c2 = A(inp['sa2_c2_w'])
    w['B2a_T'] = A(w2c2[:128].T); w['B2b_T'] = A(w2c2[128:].T)
    w['b2c2'] = A(inp['sa2_c2_b'])
    w3c1 = A(inp['sa3_c1_w'])
    w['A3x_Ta'] = pad4(A(w3c1[:128, :3].T)); w['A3x_Tb'] = pad4(A(w3c1[128:, :3].T))
    w['A3pa_Ta'] = A(w3c1[:128, 3:131].T); w['A3pa_Tb'] = A(w3c1[128:, 3:131].T)
    w['A3pb_Ta'] = A(w3c1[:128, 131:259].T); w['A3pb_Tb'] = A(w3c1[128:, 131:259].T)
    w['bn3_g'] = A(inp['sa3_bn_g']); w['bn3_be'] = A(inp['sa3_bn_be'])
    w3c2 = A(inp['sa3_c2_w'])
    for r in range(2):
        for c in range(2):
            w[f'C3_{r}{c}'] = A(w3c2[128 * r:128 * (r + 1), 128 * c:128 * (c + 1)].T)
    w['b3c2'] = A(inp['sa3_c2_b'])
    f1 = A(inp['fc1_w'])
    for r in range(2):
        for c in range(2):
            w[f'FC1_{r}{c}'] = A(f1[128 * r:128 * (r + 1), 128 * c:128 * (c + 1)].T)
    w['fbn1_g'] = A(inp['fc1_bn_g']); w['fbn1_be'] = A(inp['fc1_bn_be'])
    f2 = A(inp['fc2_w'])
    w['FC2_0'] = A(f2[:, :128].T); w['FC2_1'] = A(f2[:, 128:].T)
    w['fbn2_g'] = A(inp['fc2_bn_g']); w['fbn2_be'] = A(inp['fc2_bn_be'])
    w['FC3_T'] = A(np.asarray(inp['fc3_w'], f).T)
    w['fc3_b'] = A(inp['fc3_b'])
    w['ident'] = np.eye(128, dtype=f)
    w['constrow'] = np.stack([np.full(1024, -1.0, f), np.zeros(1024, f)])
    return w


WEIGHT_SHAPES = {
    'w1s_T': (3, 64), 'w1x_T': (3, 64), 'w2_T': (64, 128),
    'bn1_g': (64,), 'bn1_be': (64,), 'b1c2': (128,),
    'A2x_T': (4, 128), 'A2xn_T': (4, 128), 'A2p_T': (128, 128),
    'bn2_g': (128,), 'bn2_be': (128,),
    'B2a_T': (128, 128), 'B2b_T': (128, 128), 'b2c2': (256,),
    'A3x_Ta': (4, 128), 'A3x_Tb': (4, 128),
    'A3pa_Ta': (128, 128), 'A3pa_Tb': (128, 128),
    'A3pb_Ta': (128, 128), 'A3pb_Tb': (128, 128),
    'bn3_g': (256,), 'bn3_be': (256,),
    'C3_00': (128, 128), 'C3_01': (128, 128), 'C3_10': (128, 128), 'C3_11': (128, 128),
    'b3c2': (256,),
    'FC1_00': (128, 128), 'FC1_01': (128, 128), 'FC1_10': (128, 128), 'FC1_11': (128, 128),
    'fbn1_g': (256,), 'fbn1_be': (256,),
    'FC2_0': (128, 128), 'FC2_1': (128, 128),
    'fbn2_g': (128,), 'fbn2_be': (128,),
    'FC3_T': (128, 1), 'fc3_b': (1,),
    'ident': (128, 128),
    'constrow': (2, 1024),
}


def build_nc(debug=False, no_cc=False, stop_after=None):
    nc = bacc_mod.Bacc()
    d = {'points': nc.dram_tensor("points", (BC, N, 3), F32, kind="ExternalInput")}
    for name, shp in WEIGHT_SHAPES.items():
        d[name] = nc.dram_tensor(name, shp, F32, kind="ExternalInput")
    out_d = nc.dram_tensor("out", (B, 1), F32, kind="ExternalOutput")
    F1d = [nc.dram_tensor(f"F1d{c}", (N, C1A), F32) for c in range(BC)]
    pre = nc.dram_tensor("pre", (3, 128, 128), F32)
    xyzTre = [nc.dram_tensor(f"xyzTre{c}", (4, N), F32) for c in range(BC)]
    nxTre = nc.dram_tensor("nxTre", (3, BC * S1), F32)
    pre2 = nc.dram_tensor("pre2", (3, 128, 8), F32)
    nxT2re = nc.dram_tensor("nxT2re", (3, BC * S2), F32)
    xyzT2re = [nc.dram_tensor(f"xyzT2re{c}", (4, S1), F32) for c in range(BC)]
    gre = nc.dram_tensor("gre", (2, 128, B), F32)
    nxd = nc.dram_tensor("nxd", (BC, S1, 3), F32)
    nxsd = nc.dram_tensor("nxsd", (128, 3, S1), F32)
    nxsd2 = nc.dram_tensor("nxsd2", (128, 3, S2), F32)
    F2d = [nc.dram_tensor(f"F2d{c}", (S1, 192), F32) for c in range(BC)]
    x1d = nc.dram_tensor("x1d", (BC, C1A, N), F32)
    x2d = nc.dram_tensor("x2d", (BC, C2A, S2 * K), F32)
    nx2d = nc.dram_tensor("nx2d", (BC, S2, 3), F32)
    cc1i = nc.dram_tensor("cc1i", (C1A, 2), F32)
    cc1o = nc.dram_tensor("cc1o", (C1A, 2), F32, addr_space="Shared")
    cc2i = nc.dram_tensor("cc2i", (C2A, 2), F32)
    cc2o = nc.dram_tensor("cc2o", (C2A, 2), F32, addr_space="Shared")
    cc3i = nc.dram_tensor("cc3i", (128, 4), F32)
    cc3o = nc.dram_tensor("cc3o", (128, 4), F32, addr_space="Shared")
    ggi = nc.dram_tensor("ggi", (2, BC, 128), F32)
    ggo = nc.dram_tensor("ggo", (NCORES, 2, BC, 128), F32, addr_space="Shared")
    dbg = {}
    if debug:
        dbg['nx'] = nc.dram_tensor("dbg_nx", (BC, S1, 3), F32, kind="ExternalOutput")
        dbg['idx1'] = nc.dram_tensor("dbg_idx1", (BC, 16, S1), U32, kind="ExternalOutput")
        dbg['l1p'] = nc.dram_tensor("dbg_l1p", (C1B, BC * S1), F32, kind="ExternalOutput")
        dbg['nx2'] = nc.dram_tensor("dbg_nx2", (BC, S2, 3), F32, kind="ExternalOutput")
        dbg['idx2'] = nc.dram_tensor("dbg_idx2", (BC, 16, S2), U32, kind="ExternalOutput")
        dbg['l2pa'] = nc.dram_tensor("dbg_l2pa", (128, BC * S2), F32, kind="ExternalOutput")
        dbg['l2pb'] = nc.dram_tensor("dbg_l2pb", (128, BC * S2), F32, kind="ExternalOutput")
        dbg['ga'] = nc.dram_tensor("dbg_ga", (128, BC), F32, kind="ExternalOutput")
        dbg['gb'] = nc.dram_tensor("dbg_gb", (128, BC), F32, kind="ExternalOutput")
    with tile.TileContext(nc) as tc:
        with nc.allow_non_contiguous_dma(reason="small strided restaging DMAs"), ExitStack() as ctx:
            emit(ctx, tc, d, out_d, F1d, nxd, F2d, nx2d, x1d, x2d,
                 (pre, xyzTre, nxTre, pre2, nxT2re, xyzT2re, gre, nxsd, nxsd2),
                 (cc1i, cc1o), (cc2i, cc2o), (cc3i, cc3o), (ggi, ggo), dbg, no_cc,
                 stop_after)
    nc.compile()
    return nc


def fps_loop(ctx, tc, pool, xs, ys, zs, xyzneg, nx, nsteps, free, name, ones1, psum,
             nxs=None):
    nc = tc.nc
    dist = pool.tile([128, free], F32, name=f"{name}_dist")
    nc.vector.memset(dist[:], 1e10)
    cneg = pool.tile([128, 3], F32, name=f"{name}_cneg")
    m8 = pool.tile([128, 8], F32, name=f"{name}_m8")
    gm = pool.tile([128, 1], F32, name=f"{name}_gm")
    r = pool.tile([128, 3], F32, name=f"{name}_r")
    junk = pool.tile([128, free], F32, name=f"{name}_junk")
    e1 = pool.tile([128, free], F32, name=f"{name}_e1")
    e2 = pool.tile([128, free], F32, name=f"{name}_e2")
    e3 = pool.tile([128, free], F32, name=f"{name}_e3")
    aa = pool.tile([128, free], F32, name=f"{name}_aa")
    ind0 = pool.tile([128, free], F32, name=f"{name}_ind0")
    nc.vector.memset(ind0[:], 0.0)
    nc.vector.memset(ind0[0:1, 0:1], 1.0)
    nc.vector.memset(ind0[64:65, 0:1], 1.0)
    # hi-half (partition 64:128) slices of partition_all_reduce return zeros on
    # HW, so route per-cloud reductions through disjoint COLUMNS of full-128
    # reduces: m2 packs per-cloud maxima, r6 per-cloud coordinate sums.
    m2 = pool.tile([128, 2], F32, name=f"{name}_m2")
    nc.vector.memset(m2[:], -1e30)
    gm2 = pool.tile([128, 2], F32, name=f"{name}_gm2")
    r6 = pool.tile([128, 6], F32, name=f"{name}_r6")
    nc.vector.memset(r6[:], 0.0)
    c6 = pool.tile([128, 6], F32, name=f"{name}_c6")

    def extract_c(mask_src, scal):
        # r[p,d] = sum_f (mask==scal ? -coord); full-128 add -> cneg everywhere
        for dd in range(3):
            nc.vector.scalar_tensor_tensor(
                out=junk[:], in0=mask_src, scalar=scal, in1=xyzneg[:, dd, :],
                op0=OP.is_equal, op1=OP.mult, accum_out=r[:, dd:dd + 1])
        nc.vector.tensor_copy(out=r6[0:64, 0:3], in_=r[0:64, :])
        nc.vector.tensor_copy(out=r6[64:128, 3:6], in_=r[64:128, :])
        nc.gpsimd.partition_all_reduce(c6[:], r6[:], 128, bass_isa.ReduceOp.add)
        nc.vector.tensor_copy(out=cneg[0:64, :], in_=c6[0:64, 0:3])
        nc.vector.tensor_copy(out=cneg[64:128, :], in_=c6[64:128, 3:6])

    def record(i):
        if nxs is not None:
            # all partitions hold their cloud-half's reduced value; negate into
            # the transposed [128, 3, nsteps] buffer
            nc.vector.tensor_scalar_mul(out=nxs[:, :, i], in0=cneg[:],
                                        scalar1=-1.0)
        else:
            nc.scalar.activation(out=nx[0:1, i, :], in_=cneg[0:1, :],
                                 func=AF.Copy, scale=-1.0)
            nc.scalar.activation(out=nx[64:65, i, :], in_=cneg[64:65, :],
                                 func=AF.Copy, scale=-1.0)

    extract_c(ind0[:], 1.0)
    record(0)
    for i in range(1, nsteps):
        nc.scalar.activation(out=e1[:], in_=xyzneg[:, 0, :], func=AF.Square,
                             scale=-1.0, bias=cneg[:, 0:1])
        nc.scalar.activation(out=e2[:], in_=xyzneg[:, 1, :], func=AF.Square,
                             scale=-1.0, bias=cneg[:, 1:2])
        nc.scalar.activation(out=e3[:], in_=xyzneg[:, 2, :], func=AF.Square,
                             scale=-1.0, bias=cneg[:, 2:3])
        nc.vector.tensor_tensor(out=aa[:], in0=e1[:], in1=e2[:], op=OP.add)
        nc.vector.tensor_tensor(out=e1[:], in0=aa[:], in1=e3[:], op=OP.add)
        nc.vector.tensor_tensor(out=dist[:], in0=dist[:], in1=e1[:], op=OP.min)
        nc.vector.max(m8[:], dist[:])
        nc.vector.tensor_copy(out=m2[0:64, 0:1], in_=m8[0:64, 0:1])
        nc.vector.tensor_copy(out=m2[64:128, 1:2], in_=m8[64:128, 0:1])
        nc.gpsimd.partition_all_reduce(gm2[:], m2[:], 128, bass_isa.ReduceOp.max)
        nc.vector.tensor_copy(out=gm[0:64, :], in_=gm2[0:64, 0:1])
        nc.vector.tensor_copy(out=gm[64:128, :], in_=gm2[64:128, 1:2])
        extract_c(dist[:], gm[:, 0:1])
        record(i)


def fps_loop_split(ctx, tc, pool, xyzsrc, nsteps, vw, name, nxs):
    """Per-cloud FPS chains: cloud c uses its own [128, vw] tiles spanning all
    128 partitions (point idx = p*vw + f), so reductions are full-128 (the only
    partition_all_reduce form that works on HW). The two chains interleave on
    the engines. xyzsrc(c, dd) -> DRAM AP of cloud c's coord row, (128, vw).
    Records into nxs[128, 3, nsteps] partition-halves (downstream layout
    unchanged: cloud c at partition 64*c)."""
    nc = tc.nc
    mf = max(vw, 8)
    T = {}
    for c in range(2):
        xyzneg = pool.tile([128, 3, vw], F32, name=f"{name}_xyzn{c}")
        for dd in range(3):
            xt = pool.tile([128, vw], F32, name=f"{name}_x{c}{dd}")
            nc.gpsimd.dma_start(xt[:], xyzsrc(c, dd))
            nc.vector.tensor_scalar_mul(out=xyzneg[:, dd, :], in0=xt[:],
                                        scalar1=-1.0)
        dist = pool.tile([128, mf], F32, name=f"{name}_dist{c}")
        nc.vector.memset(dist[:], 1e10)
        if mf > vw:
            nc.vector.memset(dist[:, vw:mf], -1e30)
        # per-step extract history: the add-reduce writes straight into
        # hist[:, :, i]; step i+1's bias reads hist[:, d, i]; one bulk negate
        # after the loop replaces per-step record ops
        hist = pool.tile([128, 3, nsteps], F32, name=f"{name}_hist{c}")
        m8 = pool.tile([128, 8], F32, name=f"{name}_m8{c}")
        gm = pool.tile([128, 1], F32, name=f"{name}_gm{c}")
        r = pool.tile([128, 3], F32, name=f"{name}_r{c}")
        junk = pool.tile([128, vw], F32, name=f"{name}_junk{c}")
        e1 = pool.tile([128, vw], F32, name=f"{name}_e1{c}")
        e2 = pool.tile([128, vw], F32, name=f"{name}_e2{c}")
        e3 = pool.tile([128, vw], F32, name=f"{name}_e3{c}")
        aa = pool.tile([128, vw], F32, name=f"{name}_aa{c}")
        ind0 = pool.tile([128, vw], F32, name=f"{name}_ind0{c}")
        nc.vector.memset(ind0[:], 0.0)
        nc.vector.memset(ind0[0:1, 0:1], 1.0)
        T[c] = (xyzneg, dist, hist, m8, gm, r, junk, e1, e2, e3, aa, ind0)

    def extract_c(c, mask_src, scal, i):
        xyzneg, dist, hist, m8, gm, r, junk = T[c][:7]
        for dd in range(3):
            nc.vector.scalar_tensor_tensor(
                out=junk[:], in0=mask_src, scalar=scal, in1=xyzneg[:, dd, :],
                op0=OP.is_equal, op1=OP.mult, accum_out=r[:, dd:dd + 1])
        nc.gpsimd.partition_all_reduce(hist[:, :, i], r[:], 128,
                                       bass_isa.ReduceOp.add)

    for c in range(2):
        extract_c(c, T[c][11][:], 1.0, 0)
    # phase-interleaved emission: both clouds' reduces are in flight before
    # either cloud's dependent phase queues, so the in-order engine queues
    # overlap the two serial chains.
    for i in range(1, nsteps):
        for c in range(2):
            xyzneg, dist, hist, m8, gm, r, junk, e1, e2, e3, aa, ind0 = T[c]
            nc.scalar.activation(out=e1[:], in_=xyzneg[:, 0, :], func=AF.Square,
                                 scale=-1.0, bias=hist[:, 0:1, i - 1])
            nc.scalar.activation(out=e2[:], in_=xyzneg[:, 1, :], func=AF.Square,
                                 scale=-1.0, bias=hist[:, 1:2, i - 1])
            nc.scalar.activation(out=e3[:], in_=xyzneg[:, 2, :], func=AF.Square,
                                 scale=-1.0, bias=hist[:, 2:3, i - 1])
            nc.vector.tensor_tensor(out=aa[:], in0=e1[:], in1=e2[:], op=OP.add)
            nc.vector.tensor_tensor(out=e1[:], in0=aa[:], in1=e3[:], op=OP.add)
            nc.vector.tensor_tensor(out=dist[:, 0:vw], in0=dist[:, 0:vw],
                                    in1=e1[:], op=OP.min)
            nc.vector.max(m8[:], dist[:])
            nc.gpsimd.partition_all_reduce(gm[:], m8[:, 0:1], 128,
                                           bass_isa.ReduceOp.max)
        for c in range(2):
            extract_c(c, T[c][1][:, 0:vw], T[c][4][:, 0:1], i)
    # bulk negate the per-step history into the shared nxs record buffer
    for c in range(2):
        nc.vector.tensor_scalar_mul(
            out=nxs[64 * c:64 * (c + 1), :, :],
            in0=T[c][2][64 * c:64 * (c + 1), :, :], scalar1=-1.0)


def bn_affine(tc, pool, sums, sqs, g_sb, be_sb, count, cpart, name):
    nc = tc.nc
    mean = pool.tile([cpart, 1], F32, name=f"{name}_mean")
    var = pool.tile([cpart, 1], F32, name=f"{name}_var")
    scale = pool.tile([cpart, 1], F32, name=f"{name}_scale")
    bias = pool.tile([cpart, 1], F32, name=f"{name}_bias")
    tmp = pool.tile([cpart, 1], F32, name=f"{name}_tmp")
    inv_n = 1.0 / float(count)
    nc.scalar.mul(mean[:], sums, inv_n)
    nc.scalar.mul(var[:], sqs, inv_n)
    nc.vector.tensor_tensor(out=tmp[:], in0=mean[:], in1=mean[:], op=OP.mult)
    nc.vector.tensor_tensor(out=var[:], in0=var[:], in1=tmp[:], op=OP.subtract)
    nc.vector.tensor_scalar_add(out=var[:], in0=var[:], scalar1=EPS)
    nc.vector.reciprocal(tmp[:], var[:])
    nc.scalar.activation(out=tmp[:], in_=tmp[:], func=AF.Sqrt)
    nc.vector.tensor_tensor(out=scale[:], in0=tmp[:], in1=g_sb, op=OP.mult)
    nc.vector.tensor_tensor(out=tmp[:], in0=mean[:], in1=scale[:], op=OP.mult)
    nc.vector.tensor_tensor(out=bias[:], in0=be_sb, in1=tmp[:], op=OP.subtract)
    scale_a = pool.tile([cpart, 1], F32, name=f"{name}_scale_a")
    bias_a = pool.tile([cpart, 1], F32, name=f"{name}_bias_a")
    nc.scalar.activation(out=scale_a[:], in_=scale[:], func=AF.Copy)
    nc.scalar.activation(out=bias_a[:], in_=bias[:], func=AF.Copy)
    return scale_a, bias_a


def emit(ctx, tc, d, out_d, F1d, nxd, F2d, nx2d, x1d, x2d, stg, cc1, cc2, cc3, gg, dbg,
         no_cc=False, stop_after=None):
    pre, xyzTre, nxTre, pre2, nxT2re, xyzT2re, gre, nxsd, nxsd2 = stg
    nc = tc.nc

    def bail():
        zout = sing.tile([16, 1], F32, name="zout")
        nc.vector.memset(zout[:], 0.0)
        nc.gpsimd.dma_start(out_d[:], zout[:])
    P = 128
    RG = [list(range(NCORES))]
    sing = ctx.enter_context(tc.tile_pool(name="sing", bufs=1))
    big = ctx.enter_context(tc.tile_pool(name="big", bufs=1))
    work = ctx.enter_context(tc.tile_pool(name="work", bufs=1))
    psum = ctx.enter_context(tc.tile_pool(name="psum", bufs=3, space="PSUM"))
    psumT = ctx.enter_context(tc.tile_pool(name="psumT", bufs=3, space="PSUM"))
    bpool = ctx.enter_context(tc.tile_pool(name="bnp", bufs=1))


    ones1 = sing.tile([1, 128], F32, name="ones1")
    nc.vector.memset(ones1[:], 1.0)
    wsb = {}
    for name, shp in WEIGHT_SHAPES.items():
        if len(shp) == 1:
            if shp[0] > 128:
                for hh in range(shp[0] // 128):
                    t = sing.tile([128, 1], F32, name=f"w_{name}_{hh}")
                    nc.gpsimd.dma_start(t[:], d[name][128 * hh:128 * (hh + 1), None])
                    wsb[f"{name}_{hh}"] = t
                continue
            t = sing.tile([shp[0], 1], F32, name=f"w_{name}")
            nc.gpsimd.dma_start(t[:], d[name][:, None])
        else:
            t = sing.tile(list(shp), F32, name=f"w_{name}")
            nc.gpsimd.dma_start(t[:], d[name][:])
        wsb[name] = t

    # ---- points load (restage so each SBUF tile = ONE DMA) ----
    for dd in range(3):
        for c in range(BC):
            nc.gpsimd.dma_start(
                pre[dd, 64 * c:64 * (c + 1), :],
                d['points'][c, :, dd].rearrange("(p f) -> p f", p=64))
    xs = sing.tile([P, 128], F32, name="xs")
    ys = sing.tile([P, 128], F32, name="ys")
    zs = sing.tile([P, 128], F32, name="zs")
    for dd, t in enumerate((xs, ys, zs)):
        nc.gpsimd.dma_start(t[:], pre[dd])
    xyzneg = sing.tile([P, 3, 128], F32, name="xyzneg")
    for dd, t in enumerate((xs, ys, zs)):
        nc.vector.tensor_scalar_mul(out=xyzneg[:, dd, :], in0=t[:], scalar1=-1.0)
    sqt0 = work.tile([P, 128], F32, name="sqt0", tag="sqt0")
    rnf = sing.tile([P, 128], F32, name="rnf")
    nc.scalar.activation(out=rnf[:], in_=xyzneg[:, 0, :], func=AF.Square)
    nc.scalar.activation(out=sqt0[:], in_=xyzneg[:, 1, :], func=AF.Square)
    nc.vector.tensor_tensor(out=rnf[:], in0=rnf[:], in1=sqt0[:], op=OP.add)
    nc.scalar.activation(out=sqt0[:], in_=xyzneg[:, 2, :], func=AF.Square)
    nc.vector.tensor_tensor(out=rnf[:], in0=rnf[:], in1=sqt0[:], op=OP.add)
    # xyzT staging: rows xyz from points, row3 = rn (per cloud), all in DRAM
    for c in range(BC):
        for dd, t in enumerate((xs, ys, zs)):
            nc.gpsimd.dma_start(xyzTre[c][dd:dd + 1, :], t[64 * c:64 * (c + 1), :])
        nc.gpsimd.dma_start(xyzTre[c][3:4, :], rnf[64 * c:64 * (c + 1), :])
    xyzTt = sing.tile([4, N], F32, name="xyzTt")

    def fill_xyzT(c):
        nc.gpsimd.dma_start(xyzTt[:], xyzTre[c][:])

    # ---- F1 rows-major -> F1d ----
    for c in range(BC):
        fill_xyzT(c)
        for j in range(8):
            ps = psum.tile([P, 512], F32, name="f1ps", tag="mm")
            st = work.tile([P, 512], F32, name="f1st", tag="f1st")
            for jj in range(8):
                ch = 8 * j + jj
                nc.tensor.matmul(ps[:, 64 * jj:64 * (jj + 1)],
                                 xyzTt[0:3, 128 * ch:128 * (ch + 1)],
                                 wsb['w1s_T'][:])
            nc.scalar.activation(out=st[:], in_=ps[:], func=AF.Copy)
            nc.gpsimd.dma_start(
                F1d[c][:].rearrange("(j p) q -> p j q", p=128)[:, 8 * j:8 * (j + 1), :],
                st[:].rearrange("p (j q) -> p j q", j=8))

    if stop_after == 1:
        bail()
        return

    # ---- FPS1 ----
    nxs = sing.tile([P, 3, S1], F32, name="nxs")
    fpool = ctx.enter_context(tc.tile_pool(name="fps1", bufs=1))
    if stop_after == 22:
        nc.vector.memset(nxs[:], 0.25)
    else:
        fps_loop_split(ctx, tc, fpool,
                       lambda c, dd: xyzTre[c][dd, :].rearrange(
                           "(p f) -> p f", p=128),
                       S1, 64, "f1", nxs)
    if stop_after == 21:
        bail()
        return
    # dump the transposed coord record to DRAM (contiguous per-partition rows),
    # then restage: cloud c's coords live on partition 64*c
    nc.gpsimd.dma_start(nxsd[:], nxs[:])
    nxT = sing.tile([3, BC * S1], F32, name="nxT")
    for c in range(BC):
        nc.gpsimd.dma_start(nxT[:, S1 * c:S1 * (c + 1)], nxsd[64 * c])
    q4T = sing.tile([4, BC * S1], F32, name="q4T")
    nc.vector.tensor_scalar_mul(out=q4T[0:3, :], in0=nxT[:], scalar1=2.0)
    nc.gpsimd.dma_start(q4T[3:4, :], d['constrow'][0:1, :])
    if dbg:
        for c in range(BC):
            nc.gpsimd.dma_start(dbg['nx'][c],
                                nxsd[64 * c].rearrange("dd q -> q dd"))

    if stop_after in (2, 22):
        bail()
        return

    Gc = sing.tile([C1A, BC * S1], F32, name="Gc")
    for h in range(2):
        psg = psum.tile([C1A, 512], F32, name="gcps", tag="mm")
        nc.tensor.matmul(psg[:], wsb['w1x_T'][:], nxT[:, 512 * h:512 * (h + 1)])
        nc.vector.tensor_copy(out=Gc[:, 512 * h:512 * (h + 1)], in_=psg[:])

    # ---- KNN1 + gather + conv1-space + stats ----
    scores = big.tile([P, N], F32, name="scores")
    sum1 = sing.tile([C1A, 128], F32, name="sum1")
    sq1 = sing.tile([C1A, 128], F32, name="sq1")
    l1pT = big.tile([C1B, BC * S1], F32, name="l1pT")

    for t in range(8):
        c = t // 4
        if t % 4 == 0:
            fill_xyzT(c)
        for jj in range(16):
            ps = psum.tile([P, 512], F32, name="knnps", tag="mm")
            nc.tensor.matmul(ps[:], q4T[:, 128 * t:128 * (t + 1)],
                             xyzTt[:, 512 * jj:512 * (jj + 1)])
            nc.scalar.activation(out=scores[:, 512 * jj:512 * (jj + 1)], in_=ps[:],
                                  func=AF.Copy)
        m8a = work.tile([P, 8], F32, name="m8a", tag="m8a")
        m8b = work.tile([P, 8], F32, name="m8b", tag="m8b")
        ia = work.tile([P, 16], U32, name="iab", tag="iab")
        nc.vector.max(m8a[:], scores[:])
        nc.vector.max_index(ia[:, 0:8], m8a[:], scores[:])
        nc.vector.match_replace(scores[:], m8a[:], scores[:], -1e30)
        nc.vector.max(m8b[:], scores[:])
        nc.vector.max_index(ia[:, 8:16], m8b[:], scores[:])
        if dbg:
            iaf = work.tile([P, 16], F32, name="iaf", tag="iaf")
            nc.vector.tensor_copy(out=iaf[:], in_=ia[:])
            pst = psumT.tile([16, P], F32, name="idxps", tag="T")
            nc.tensor.transpose(pst[:], iaf[:], wsb['ident'][:])
            dcp = work.tile([16, P], U32, name="dcp", tag="dcp")
            nc.vector.tensor_copy(out=dcp[:], in_=pst[:])
            nc.gpsimd.dma_start(dbg['idx1'][c, :, 128 * (t % 4):128 * (t % 4 + 1)],
                              dcp[:])
        # gather + conv1-space blocks, k-major columns: col = 512*k + 128*(t%4) + q
        for k in range(K):
            gblk = work.tile([P, C1A], F32, name="gblk", tag="gblk")
            nc.gpsimd.indirect_dma_start(
                out=gblk[:], out_offset=None, in_=F1d[c][:],
                in_offset=bass.IndirectOffsetOnAxis(ap=ia[:, k:k + 1], axis=0))
            psx1 = psumT.tile([C1A, P], F32, name="psx1", tag="T")
            nc.tensor.transpose(psx1[:], gblk[:], wsb['ident'][:])
            q0 = S1 * c + 128 * (t % 4)
            xblk = work.tile([C1A, P], F32, name="xblk", tag="xblk")
            nc.vector.scalar_tensor_tensor(
                out=xblk[:], in0=psx1[:], scalar=0.0,
                in1=Gc[:, q0:q0 + 128],
                op0=OP.bypass, op1=OP.subtract,
                accum_out=sum1[:, 64 * c + 16 * (t % 4) + k:64 * c + 16 * (t % 4) + k + 1])
            sqt = work.tile([C1A, P], F32, name="sqt", tag="sqt")
            nc.scalar.activation(
                out=sqt[:], in_=xblk[:], func=AF.Square,
                accum_out=sq1[:, 64 * c + 16 * (t % 4) + k:64 * c + 16 * (t % 4) + k + 1])
            nc.gpsimd.dma_start(
                x1d[c, :, 512 * k + 128 * (t % 4):512 * k + 128 * (t % 4) + 128],
                xblk[:])

    red1 = sing.tile([C1A, 2], F32, name="red1")
    nc.vector.tensor_reduce(out=red1[:, 0:1], in_=sum1[:, None, :], axis=AX.X, op=OP.add)
    nc.vector.tensor_reduce(out=red1[:, 1:2], in_=sq1[:, None, :], axis=AX.X, op=OP.add)
    nc.gpsimd.dma_start(cc1[0][:], red1[:])
    if stop_after == 3:
        bail()
        return
    if not no_cc:
        nc.gpsimd.collective_compute("AllReduce", OP.add, replica_groups=RG,
                                     ins=[cc1[0][:]], outs=[cc1[1][:]])
    stat1 = sing.tile([C1A, 2], F32, name="stat1")
    nc.gpsimd.dma_start(stat1[:], cc1[0 if no_cc else 1][:])
    sc1, bi1 = bn_affine(tc, bpool, stat1[:, 0:1], stat1[:, 1:2],
                         wsb['bn1_g'][:], wsb['bn1_be'][:], B * S1 * K, C1A, "bn1")

    for c in range(BC):
        for k in range(K):
            col = 512 * k
            x1c = work.tile([C1A, 512], F32, name="x1c", tag="x1c")
            nc.gpsimd.dma_start(x1c[:], x1d[c, :, col:col + 512])
            x1v = work.tile([C1A, 512], F32, name="x1v", tag="x1v")
            nc.scalar.activation(out=x1v[:], in_=x1c[:], func=AF.Copy)
            h1 = work.tile([C1A, 512], F32, name="h1", tag="h1")
            nc.scalar.activation(out=h1[:], in_=x1v[:], func=AF.Relu,
                                 scale=sc1[:], bias=bi1[:])
            ps = psum.tile([C1B, 512], F32, name="c2ps", tag="mm")
            nc.tensor.matmul(ps[:], wsb['w2_T'][:], h1[:])
            sl = l1pT[:, S1 * c:S1 * (c + 1)]
            if k == 0:
                nc.vector.tensor_copy(out=sl, in_=ps[:])
            else:
                nc.vector.tensor_tensor(out=sl, in0=sl, in1=ps[:], op=OP.max)
    nc.vector.tensor_scalar(out=l1pT[:], in0=l1pT[:], scalar1=wsb['b1c2'][:],
                            scalar2=None, op0=OP.add)
    if dbg:
        nc.gpsimd.dma_start(dbg['l1p'][:], l1pT[:])
    if stop_after == 4:
        bail()
        return

    # ---- SA2 prep ----
    zpad = sing.tile([128, 60], F32, name="zpad")
    nc.vector.memset(zpad[:], 0.0)
    for c in range(BC):
        nc.gpsimd.dma_start(F2d[c][:, 0:3],
                            nxsd[64 * c].rearrange("dd q -> q dd"))
        for j in range(4):
            nc.gpsimd.dma_start(F2d[c][128 * j:128 * (j + 1), 3], zpad[:, 0:1])
            nc.gpsimd.dma_start(F2d[c][128 * j:128 * (j + 1), 132:192], zpad[:])
        for j in range(4):
            pst = psumT.tile([P, P], F32, name="ftps", tag="T")
            nc.tensor.transpose(pst[:], l1pT[:, S1 * c + 128 * j:S1 * c + 128 * (j + 1)],
                                wsb['ident'][:])
            stg = work.tile([P, P], F32, name="fstg", tag="fstg")
            nc.vector.tensor_copy(out=stg[:], in_=pst[:])
            nc.gpsimd.dma_start(F2d[c][128 * j:128 * (j + 1), 4:132], stg[:])

    nxs2 = sing.tile([P, 3, S2], F32, name="nxs2")
    fpool2 = ctx.enter_context(tc.tile_pool(name="fps2", bufs=1))
    fps_loop_split(ctx, tc, fpool2,
                   lambda c, dd: nxsd[64 * c, dd, :].rearrange(
                       "(p f) -> p f", p=128),
                   S2, 4, "f2", nxs2)
    nc.gpsimd.dma_start(nxsd2[:], nxs2[:])
    if dbg:
        for c in range(BC):
            nc.gpsimd.dma_start(dbg['nx2'][c],
                                nxsd2[64 * c].rearrange("dd q -> q dd"))

    nxT2f = sing.tile([4, BC * S2], F32, name="nxT2f")
    for c in range(BC):
        nc.gpsimd.dma_start(nxT2f[0:3, S2 * c:S2 * (c + 1)], nxsd2[64 * c])
    nc.gpsimd.dma_start(nxT2f[3:4, :], d['constrow'][1:2, 0:BC * S2])
    nxT2 = nxT2f
    q4T2 = sing.tile([3, BC * S2], F32, name="q4T2")
    nc.vector.tensor_scalar_mul(out=q4T2[:], in0=nxT2[0:3, :], scalar1=2.0)
    monerow = sing.tile([1, 128], F32, name="monerow")
    nc.gpsimd.dma_start(monerow[:], d['constrow'][0:1, 0:128])
    xyzT2 = [sing.tile([3, S1], F32, name=f"xyzT2_{c}") for c in range(BC)]
    rn2ts = [sing.tile([1, S1], F32, name=f"rn2t_{c}") for c in range(BC)]
    ones3 = sing.tile([3, 1], F32, name="ones3")
    nc.vector.memset(ones3[:], 1.0)
    for c in range(BC):
        nc.gpsimd.dma_start(xyzT2[c][:], nxsd[64 * c])
        sq2t = work.tile([3, S1], F32, name="sq2t", tag="sq2t")
        nc.scalar.activation(out=sq2t[:], in_=xyzT2[c][:], func=AF.Square)
        psr = psum.tile([1, S1], F32, name="rnps", tag="mm")
        nc.tensor.matmul(psr[:], ones3[:], sq2t[:])
        nc.vector.tensor_copy(out=rn2ts[c][:], in_=psr[:])

    if stop_after == 5:
        bail()
        return

    # ---- KNN2 + gather + MLP2 ----
    sum2 = sing.tile([C2A, 8], F32, name="sum2")
    sq2 = sing.tile([C2A, 8], F32, name="sq2")
    l2paT = big.tile([128, BC * S2], F32, name="l2paT")
    l2pbT = big.tile([128, BC * S2], F32, name="l2pbT")

    for c in range(BC):
        ps = psum.tile([P, S1], F32, name="kn2ps", tag="mm")
        nc.tensor.matmul(ps[:], q4T2[:, S2 * c:S2 * (c + 1)], xyzT2[c][:],
                         start=True, stop=False)
        nc.tensor.matmul(ps[:], monerow[:], rn2ts[c][:], start=False, stop=True)
        sc2t = work.tile([P, S1], F32, name="sc2t", tag="sc2t")
        nc.scalar.activation(out=sc2t[:], in_=ps[:], func=AF.Copy)
        m8a = work.tile([P, 8], F32, name="m8a2", tag="m8a2")
        m8b = work.tile([P, 8], F32, name="m8b2", tag="m8b2")
        ia = work.tile([P, 16], U32, name="iab2", tag="iab2")
        nc.vector.max(m8a[:], sc2t[:])
        nc.vector.max_index(ia[:, 0:8], m8a[:], sc2t[:])
        nc.vector.match_replace(sc2t[:], m8a[:], sc2t[:], -1e30)
        nc.vector.max(m8b[:], sc2t[:])
        nc.vector.max_index(ia[:, 8:16], m8b[:], sc2t[:])
        if dbg:
            iaf2 = work.tile([P, 16], F32, name="iaf2", tag="iaf2")
            nc.vector.tensor_copy(out=iaf2[:], in_=ia[:])
            pst2 = psumT.tile([16, P], F32, name="idx2ps", tag="T")
            nc.tensor.transpose(pst2[:], iaf2[:], wsb['ident'][:])
            dcp2 = work.tile([16, P], U32, name="dcp2", tag="dcp2")
            nc.vector.tensor_copy(out=dcp2[:], in_=pst2[:])
            nc.gpsimd.dma_start(dbg['idx2'][c], dcp2[:])
        rhx = big.tile([4, S2 * K], F32, name="rhx", tag="rhx")
        rhp = big.tile([C2A, S2 * K], F32, name="rhp", tag="rhp")
        for k in range(K):
            gblk2 = work.tile([P, 192], F32, name="gblk2", tag="gblk2")
            nc.gpsimd.indirect_dma_start(
                out=gblk2[:], out_offset=None, in_=F2d[c][:],
                in_offset=bass.IndirectOffsetOnAxis(ap=ia[:, k:k + 1], axis=0))
            psx = psumT.tile([4, P], F32, name="psx", tag="T")
            nc.tensor.transpose(psx[:], gblk2[:, 0:4], wsb['ident'][:])
            nc.vector.tensor_copy(out=rhx[:, 128 * k:128 * (k + 1)], in_=psx[:])
            psp = psumT.tile([C2A, P], F32, name="psp", tag="T")
            nc.tensor.transpose(psp[:], gblk2[:, 4:132], wsb['ident'][:])
            nc.vector.tensor_copy(out=rhp[:, 128 * k:128 * (k + 1)], in_=psp[:])
        for chk in range(4):
            col = 512 * chk
            ps2 = psum.tile([C2A, 512], F32, name="c1ps2", tag="mm")
            nc.tensor.matmul(ps2[:], wsb['A2x_T'][:], rhx[:, col:col + 512],
                             start=True, stop=False)
            nc.tensor.matmul(ps2[:], wsb['A2p_T'][:], rhp[:, col:col + 512],
                             start=False, stop=False)
            nc.tensor.matmul(
                ps2[:], wsb['A2xn_T'][:],
                nxT2[:, S2 * c:S2 * (c + 1)][:, None, :].broadcast_to((4, 4, S2)),
                start=False, stop=True)
            x2blk = work.tile([C2A, 512], F32, name="x2blk", tag="x2blk")
            nc.scalar.activation(out=x2blk[:], in_=ps2[:], func=AF.Copy,
                                 accum_out=sum2[:, 4 * c + chk:4 * c + chk + 1])
            sqt2 = work.tile([C2A, 512], F32, name="sqt2", tag="sqt2")
            nc.scalar.activation(out=sqt2[:], in_=x2blk[:], func=AF.Square,
                                 accum_out=sq2[:, 4 * c + chk:4 * c + chk + 1])
            nc.gpsimd.dma_start(x2d[c, :, col:col + 512], x2blk[:])

    red2 = sing.tile([C2A, 2], F32, name="red2")
    nc.vector.tensor_reduce(out=red2[:, 0:1], in_=sum2[:, None, :], axis=AX.X, op=OP.add)
    nc.vector.tensor_reduce(out=red2[:, 1:2], in_=sq2[:, None, :], axis=AX.X, op=OP.add)
    nc.gpsimd.dma_start(cc2[0][:], red2[:])
    if not no_cc:
        nc.gpsimd.collective_compute("AllReduce", OP.add, replica_groups=RG,
                                     ins=[cc2[0][:]], outs=[cc2[1][:]])
    stat2 = sing.tile([C2A, 2], F32, name="stat2")
    nc.gpsimd.dma_start(stat2[:], cc2[0 if no_cc else 1][:])
    sc2, bi2 = bn_affine(tc, bpool, stat2[:, 0:1], stat2[:, 1:2],
                         wsb['bn2_g'][:], wsb['bn2_be'][:], B * S2 * K, C2A, "bn2")

    for c in range(BC):
        for chk in range(4):
            col = 512 * chk
            x2c = work.tile([C2A, 512], F32, name="x2c", tag="x2c")
            nc.gpsimd.dma_start(x2c[:], x2d[c, :, col:col + 512])
            x2v = work.tile([C2A, 512], F32, name="x2v", tag="x2v")
            nc.scalar.activation(out=x2v[:], in_=x2c[:], func=AF.Copy)
            h2 = work.tile([C2A, 512], F32, name="h2", tag="h2")
            nc.scalar.activation(out=h2[:], in_=x2v[:], func=AF.Relu,
                                 scale=sc2[:], bias=bi2[:])
            psa = psum.tile([128, 512], F32, name="c2psa", tag="mm")
            nc.tensor.matmul(psa[:], wsb['B2a_T'][:], h2[:])
            psb = psum.tile([128, 512], F32, name="c2psb", tag="mm")
            nc.tensor.matmul(psb[:], wsb['B2b_T'][:], h2[:])
            for half, (pp, ll) in enumerate(((psa, l2paT), (psb, l2pbT))):
                sl = ll[:, S2 * c:S2 * (c + 1)]
                for kk in range(4):
                    yk = pp[:, 128 * kk:128 * (kk + 1)]
                    if chk == 0 and kk == 0:
                        nc.vector.tensor_copy(out=sl, in_=yk)
                    else:
                        nc.vector.tensor_tensor(out=sl, in0=sl, in1=yk, op=OP.max)
    nc.vector.tensor_scalar(out=l2paT[:], in0=l2paT[:], scalar1=wsb['b2c2_0'][:],
                            scalar2=None, op0=OP.add)
    nc.vector.tensor_scalar(out=l2pbT[:], in0=l2pbT[:], scalar1=wsb['b2c2_1'][:],
                            scalar2=None, op0=OP.add)
    if dbg:
        nc.gpsimd.dma_start(dbg['l2pa'][:], l2paT[:])
        nc.gpsimd.dma_start(dbg['l2pb'][:], l2pbT[:])
    if stop_after == 6:
        bail()
        return

    # ---- SA3 ----
    NR3 = BC * S2
    x3a = big.tile([128, NR3], F32, name="x3a")
    x3b = big.tile([128, NR3], F32, name="x3b")
    s3 = sing.tile([128, 4], F32, name="s3")
    for half, (x3, xw, paw, pbw) in enumerate(
            ((x3a, 'A3x_Ta', 'A3pa_Ta', 'A3pb_Ta'),
             (x3b, 'A3x_Tb', 'A3pa_Tb', 'A3pb_Tb'))):
        ps3 = psum.tile([128, NR3], F32, name="ps3", tag="mm")
        nc.tensor.matmul(ps3[:], wsb[xw][:], nxT2[:], start=True, stop=False)
        nc.tensor.matmul(ps3[:], wsb[paw][:], l2paT[:], start=False, stop=False)
        nc.tensor.matmul(ps3[:], wsb[pbw][:], l2pbT[:], start=False, stop=True)
        nc.scalar.activation(out=x3[:], in_=ps3[:], func=AF.Copy,
                             accum_out=s3[:, 2 * half:2 * half + 1])
        sqt3 = work.tile([128, NR3], F32, name="sqt3", tag="sqt3")
        nc.scalar.activation(out=sqt3[:], in_=x3[:], func=AF.Square,
                             accum_out=s3[:, 2 * half + 1:2 * half + 2])
    nc.gpsimd.dma_start(cc3[0][:], s3[:])
    if not no_cc:
        nc.gpsimd.collective_compute("AllReduce", OP.add, replica_groups=RG,
                                     ins=[cc3[0][:]], outs=[cc3[1][:]])
    stat3 = sing.tile([128, 4], F32, name="stat3")
    nc.gpsimd.dma_start(stat3[:], cc3[0 if no_cc else 1][:])
    n3 = B * S2
    sc3a, bi3a = bn_affine(tc, bpool, stat3[:, 0:1], stat3[:, 1:2],
                           wsb['bn3_g_0'][:], wsb['bn3_be_0'][:], n3, 128, "bn3a")
    sc3b, bi3b = bn_affine(tc, bpool, stat3[:, 2:3], stat3[:, 3:4],
                           wsb['bn3_g_1'][:], wsb['bn3_be_1'][:], n3, 128, "bn3b")
    h3a = work.tile([128, NR3], F32, name="h3a")
    h3b = work.tile([128, NR3], F32, name="h3b")
    nc.scalar.activation(out=h3a[:], in_=x3a[:], func=AF.Relu, scale=sc3a[:], bias=bi3a[:])
    nc.scalar.activation(out=h3b[:], in_=x3b[:], func=AF.Relu, scale=sc3b[:], bias=bi3b[:])
    ga = sing.tile([128, BC], F32, name="ga")
    gb = sing.tile([128, BC], F32, name="gb")
    for half, g in ((0, ga), (1, gb)):
        psg3 = psum.tile([128, NR3], F32, name="psg3", tag="mm")
        nc.tensor.matmul(psg3[:], wsb[f'C3_{half}0'][:], h3a[:], start=True, stop=False)
        nc.tensor.matmul(psg3[:], wsb[f'C3_{half}1'][:], h3b[:], start=False, stop=True)
        nc.vector.tensor_reduce(out=g[:], in_=psg3[:].rearrange("p (c q) -> p c q", c=BC),
                                axis=AX.X, op=OP.max)
        nc.vector.tensor_scalar(out=g[:], in0=g[:],
                                scalar1=wsb[f'b3c2_{half}'][:],
                                scalar2=None, op0=OP.add)
    if dbg:
        nc.gpsimd.dma_start(dbg['ga'][:], ga[:])
        nc.gpsimd.dma_start(dbg['gb'][:], gb[:])
    if stop_after == 7:
        bail()
        return

    # ---- AllGather + FC head ----
    nc.gpsimd.dma_start(gg[0][0].rearrange("c p -> p c"), ga[:])
    nc.gpsimd.dma_start(gg[0][1].rearrange("c p -> p c"), gb[:])
    if not no_cc:
        nc.gpsimd.collective_compute("AllGather", OP.bypass, replica_groups=RG,
                                     ins=[gg[0][:]], outs=[gg[1][:]])
    for n in range(NCORES):
        ggsrc = gg[0] if no_cc else gg[1][n]
        nc.gpsimd.dma_start(gre[0, :, BC * n:BC * (n + 1)],
                            ggsrc[0].rearrange("c p -> p c"))
        nc.gpsimd.dma_start(gre[1, :, BC * n:BC * (n + 1)],
                            ggsrc[1].rearrange("c p -> p c"))
    gaal = sing.tile([128, B], F32, name="gaal")
    gbal = sing.tile([128, B], F32, name="gbal")
    nc.gpsimd.dma_start(gaal[:], gre[0])
    nc.gpsimd.dma_start(gbal[:], gre[1])

    def fc_layer(xins, wnames, gslice, beslice, name, alpha=0.2):
        ps = psum.tile([128, B], F32, name=f"{name}ps", tag="mm")
        for i, (xt, wn) in enumerate(zip(xins, wnames)):
            nc.tensor.matmul(ps[:], wsb[wn][:], xt[:], start=(i == 0),
                             stop=(i == len(xins) - 1))
        xsb = work.tile([128, B], F32, name=f"{name}x", tag=f"{name}x")
        ssq = sing.tile([128, 2], F32, name=f"{name}ssq")
        nc.scalar.activation(out=xsb[:], in_=ps[:], func=AF.Copy,
                             accum_out=ssq[:, 0:1])
        sqf = work.tile([128, B], F32, name=f"{name}sq", tag=f"{name}sq")
        nc.scalar.activation(out=sqf[:], in_=xsb[:], func=AF.Square,
                             accum_out=ssq[:, 1:2])
        sc, bi = bn_affine(tc, bpool, ssq[:, 0:1], ssq[:, 1:2], gslice, beslice,
                           B, 128, name)
        act = work.tile([128, B], F32, name=f"{name}act", tag=f"{name}act")
        vv = work.tile([128, B], F32, name=f"{name}vv", tag=f"{name}vv")
        nc.scalar.activation(out=vv[:], in_=xsb[:], func=AF.Identity,
                             scale=sc[:], bias=bi[:])
        av = work.tile([128, B], F32, name=f"{name}av", tag=f"{name}av")
        nc.vector.tensor_scalar_mul(out=av[:], in0=vv[:], scalar1=alpha)
        nc.vector.tensor_tensor(out=act[:], in0=vv[:], in1=av[:], op=OP.max)
        return act

    h1a = fc_layer([gaal, gbal], ['FC1_00', 'FC1_01'],
                   wsb['fbn1_g_0'][:], wsb['fbn1_be_0'][:], "fc1a")
    h1b = fc_layer([gaal, gbal], ['FC1_10', 'FC1_11'],
                   wsb['fbn1_g_1'][:], wsb['fbn1_be_1'][:], "fc1b")
    h2f = fc_layer([h1a, h1b], ['FC2_0', 'FC2_1'],
                   wsb['fbn2_g'][:], wsb['fbn2_be'][:], "fc2")
    ps_o = psum.tile([1, B], F32, name="ps_o", tag="mm")
    nc.tensor.matmul(ps_o[:], wsb['FC3_T'][:], h2f[:])
    o_sb = sing.tile([1, B], F32, name="o_sb")
    nc.vector.tensor_scalar(out=o_sb[:], in0=ps_o[:], scalar1=wsb['fc3_b'][:],
                            scalar2=None, op0=OP.add)
    nc.gpsimd.dma_start(out_d[:, 0][None, :], o_sb[:])


# ===================== host-side entry point =====================
_NC_CACHE = {}


def _get_nc():
    if 'nc' not in _NC_CACHE:
        _NC_CACHE['nc'] = build_nc(debug=False)
    return _NC_CACHE['nc']


def _kernel_numpy(inputs):
    """Exact numpy fallback of the reference model (host-side)."""
    f = np.float32
    pts = np.asarray(inputs['points'], f)
    Bn, Nn = pts.shape[0], pts.shape[1]

    def fps(x, npoint):
        n = x.shape[0]
        xs_, ys_, zs_ = x[:, 0], x[:, 1], x[:, 2]
        dist = np.full(n, 1e10, f)
        idxs = np.zeros(npoint, np.int64)
        far = 0
        for i in range(npoint):
            idxs[i] = far
            c = x[far]
            e = ((xs_ - c[0]) ** 2).astype(f) + ((ys_ - c[1]) ** 2).astype(f)
            dist = np.minimum(dist, (e + ((zs_ - c[2]) ** 2).astype(f)).astype(f))
            far = int(np.argmax(dist))
        return idxs

    def knn(q, r, k):
        d = (np.sum(q ** 2, -1)[:, None] - 2.0 * (q @ r.T) + np.sum(r ** 2, -1)[None, :])
        return np.argsort(d, axis=1, kind='stable')[:, :k]

    def bn(x, g, b, axes):
        m = x.mean(axes, keepdims=True, dtype=np.float64).astype(f)
        v = x.var(axes, keepdims=True).astype(f)
        return (x - m) / np.sqrt(v + 1e-5) * g + b

    def mlp2(x, w1, b1, g1, be1, w2, b2, axes):
        h = x @ np.asarray(w1, f).T + b1
        h = np.maximum(bn(h, g1, be1, axes), 0)
        return h @ np.asarray(w2, f).T + b2

    def sa_knn(xyz, ptsf, npoint, k, w1, b1, g1, be1, w2, b2):
        nx_l, np_l, gx_l, gp_l = [], [], [], []
        for b_ in range(xyz.shape[0]):
            fi = fps(xyz[b_], npoint)
            nxb = xyz[b_][fi]
            idx = knn(nxb, xyz[b_], k)
            gx_l.append(xyz[b_][idx] - nxb[:, None, :])
            gp_l.append(ptsf[b_][idx])
            nx_l.append(nxb)
        nxa = np.stack(nx_l); gx = np.stack(gx_l); gp = np.stack(gp_l)
        grouped = np.concatenate([gx, gp], -1)
        out = mlp2(grouped, w1, b1, g1, be1, w2, b2, (0, 1, 2))
        return nxa, out.max(2)

    i = {k: np.asarray(v, f) for k, v in inputs.items()}
    l1x, l1p = sa_knn(pts, pts, 512, 16, i['sa1_c1_w'], i['sa1_c1_b'],
                      i['sa1_bn_g'], i['sa1_bn_be'], i['sa1_c2_w'], i['sa1_c2_b'])
    l2x, l2p = sa_knn(l1x, l1p, 128, 16, i['sa2_c1_w'], i['sa2_c1_b'],
                      i['sa2_bn_g'], i['sa2_bn_be'], i['sa2_c2_w'], i['sa2_c2_b'])
    grouped = np.concatenate([l2x, l2p], -1)[:, None]
    g = mlp2(grouped, i['sa3_c1_w'], i['sa3_c1_b'], i['sa3_bn_g'], i['sa3_bn_be'],
             i['sa3_c2_w'], i['sa3_c2_b'], (0, 1, 2)).max(2)[:, 0]

    def lrelu(x):
        return np.where(x > 0, x, 0.2 * x)
    h = g @ i['fc1_w'].T + i['fc1_b']
    h = lrelu(bn(h, i['fc1_bn_g'], i['fc1_bn_be'], (0,)))
    h = h @ i['fc2_w'].T + i['fc2_b']
    h = lrelu(bn(h, i['fc2_bn_g'], i['fc2_bn_be'], (0,)))
    return (h @ i['fc3_w'].T + i['fc3_b']).astype(f)


def kernel(**inputs):
    """Full-input entry: shard over 8 NeuronCores, run, return (16,1) logits."""
    try:
        from concourse.bass_utils import run_bass_kernel_spmd
        w = prep_common_weights(inputs)
        pts = np.asarray(inputs['points'], np.float32)
        in_maps = []
        for t in range(NCORES):
            m = {'points': np.ascontiguousarray(pts[BC * t:BC * (t + 1)])}
            for name, shp in WEIGHT_SHAPES.items():
                m[name] = np.ascontiguousarray(w[name].reshape(shp))
            in_maps.append(m)
        nc = _get_nc()
        res = run_bass_kernel_spmd(nc, in_maps, list(range(NCORES)))
        out = np.asarray(res.results[0]['out'], np.float32)
        return out
    except Exception:
        import traceback
        traceback.print_exc()
        return _kernel_numpy(inputs)



# revision 43
# speedup vs baseline: 1.0028x; 1.0028x over previous
"""PointCloudDiscriminator Trainium2 Bass kernel (SPMD 8 cores, 2 clouds/core)."""
import numpy as np
from contextlib import ExitStack

import concourse.bass as bass
import concourse.bacc as bacc_mod
import concourse.tile as tile
import concourse.mybir as mybir
from concourse import bass_isa

F32 = mybir.dt.float32
U32 = mybir.dt.uint32
I16 = mybir.dt.int16
AF = mybir.ActivationFunctionType
OP = mybir.AluOpType
AX = mybir.AxisListType
EPS = 1e-5

B, N, S1, S2, K = 16, 8192, 512, 128, 16
NCORES = 8
BC = B // NCORES
C1A, C1B = 64, 128
C2A, C2B = 128, 256


def prep_common_weights(inp):
    f = np.float32
    w = {}
    A = lambda x: np.ascontiguousarray(np.asarray(x, f))
    pad4 = lambda a: np.concatenate([a, np.zeros((1, a.shape[1]), f)], 0)
    w1 = A(inp['sa1_c1_w'])
    w['w1s_T'] = A((w1[:, :3] + w1[:, 3:]).T)
    w['w1x_T'] = A(w1[:, :3].T)
    w['w2_T'] = A(np.asarray(inp['sa1_c2_w'], f).T)
    w['bn1_g'] = A(inp['sa1_bn_g']); w['bn1_be'] = A(inp['sa1_bn_be'])
    w['b1c2'] = A(inp['sa1_c2_b'])
    w2c1 = A(inp['sa2_c1_w'])
    w['A2x_T'] = pad4(A(w2c1[:, :3].T)); w['A2xn_T'] = A(-w['A2x_T'])
    w['A2p_T'] = A(w2c1[:, 3:].T)
    w['bn2_g'] = A(inp['sa2_bn_g']); w['bn2_be'] = A(inp['sa2_bn_be'])
    w2c2 = A(inp['sa2_c2_w'])
    w['B2a_T'] = A(w2c2[:128].T); w['B2b_T'] = A(w2c2[128:].T)
    w['b2c2'] = A(inp['sa2_c2_b'])
    w3c1 = A(inp['sa3_c1_w'])
    w['A3x_Ta'] = pad4(A(w3c1[:128, :3].T)); w['A3x_Tb'] = pad4(A(w3c1[128:, :3].T))
    w['A3pa_Ta'] = A(w3c1[:128, 3:131].T); w['A3pa_Tb'] = A(w3c1[128:, 3:131].T)
    w['A3pb_Ta'] = A(w3c1[:128, 131:259].T); w['A3pb_Tb'] = A(w3c1[128:, 131:259].T)
    w['bn3_g'] = A(inp['sa3_bn_g']); w['bn3_be'] = A(inp['sa3_bn_be'])
    w3c2 = A(inp['sa3_c2_w'])
    for r in range(2):
        for c in range(2):
            w[f'C3_{r}{c}'] = A(w3c2[128 * r:128 * (r + 1), 128 * c:128 * (c + 1)].T)
    w['b3c2'] = A(inp['sa3_c2_b'])
    f1 = A(inp['fc1_w'])
    for r in range(2):
        for c in range(2):
            w[f'FC1_{r}{c}'] = A(f1[128 * r:128 * (r + 1), 128 * c:128 * (c + 1)].T)
    w['fbn1_g'] = A(inp['fc1_bn_g']); w['fbn1_be'] = A(inp['fc1_bn_be'])
    f2 = A(inp['fc2_w'])
    w['FC2_0'] = A(f2[:, :128].T); w['FC2_1'] = A(f2[:, 128:].T)
    w['fbn2_g'] = A(inp['fc2_bn_g']); w['fbn2_be'] = A(inp['fc2_bn_be'])
    w['FC3_T'] = A(np.asarray(inp['fc3_w'], f).T)
    w['fc3_b'] = A(inp['fc3_b'])
    w['ident'] = np.eye(128, dtype=f)
    w['constrow'] = np.stack([np.full(1024, -1.0, f), np.zeros(1024, f)])
    return w


WEIGHT_SHAPES = {
    'w1s_T': (3, 64), 'w1x_T': (3, 64), 'w2_T': (64, 128),
    'bn1_g': (64,), 'bn1_be': (64,), 'b1c2': (128,),
    'A2x_T': (4, 128), 'A2xn_T': (4, 128), 'A2p_T': (128, 128),
    'bn2_g': (128,), 'bn2_be': (128,),
    'B2a_T': (128, 128), 'B2b_T': (128, 128), 'b2c2': (256,),
    'A3x_Ta': (4, 128), 'A3x_Tb': (4, 128),
    'A3pa_Ta': (128, 128), 'A3pa_Tb': (128, 128),
    'A3pb_Ta': (128, 128), 'A3pb_Tb': (128, 128),
    'bn3_g': (256,), 'bn3_be': (256,),
    'C3_00': (128, 128), 'C3_01': (128, 128), 'C3_10': (128, 128), 'C3_11': (128, 128),
    'b3c2': (256,),
    'FC1_00': (128, 128), 'FC1_01': (128, 128), 'FC1_10': (128, 128), 'FC1_11': (128, 128),
    'fbn1_g': (256,), 'fbn1_be': (256,),
    'FC2_0': (128, 128), 'FC2_1': (128, 128),
    'fbn2_g': (128,), 'fbn2_be': (128,),
    'FC3_T': (128, 1), 'fc3_b': (1,),
    'ident': (128, 128),
    'constrow': (2, 1024),
}


def build_nc(debug=False, no_cc=False, stop_after=None):
    nc = bacc_mod.Bacc()
    d = {'points': nc.dram_tensor("points", (BC, N, 3), F32, kind="ExternalInput")}
    for name, shp in WEIGHT_SHAPES.items():
        d[name] = nc.dram_tensor(name, shp, F32, kind="ExternalInput")
    out_d = nc.dram_tensor("out", (B, 1), F32, kind="ExternalOutput")
    F1d = [nc.dram_tensor(f"F1d{c}", (N, C1A), F32) for c in range(BC)]
    pre = nc.dram_tensor("pre", (3, 128, 128), F32)
    xyzTre = [nc.dram_tensor(f"xyzTre{c}", (4, N), F32) for c in range(BC)]
    nxTre = nc.dram_tensor("nxTre", (3, BC * S1), F32)
    pre2 = nc.dram_tensor("pre2", (3, 128, 8), F32)
    nxT2re = nc.dram_tensor("nxT2re", (3, BC * S2), F32)
    xyzT2re = [nc.dram_tensor(f"xyzT2re{c}", (4, S1), F32) for c in range(BC)]
    gre = nc.dram_tensor("gre", (2, 128, B), F32)
    nxd = nc.dram_tensor("nxd", (BC, S1, 3), F32)
    nxsd = nc.dram_tensor("nxsd", (128, 3, S1), F32)
    nxsd2 = nc.dram_tensor("nxsd2", (128, 3, S2), F32)
    F2d = [nc.dram_tensor(f"F2d{c}", (S1, 192), F32) for c in range(BC)]
    x1d = nc.dram_tensor("x1d", (BC, C1A, N), F32)
    x2d = nc.dram_tensor("x2d", (BC, C2A, S2 * K), F32)
    nx2d = nc.dram_tensor("nx2d", (BC, S2, 3), F32)
    cc1i = nc.dram_tensor("cc1i", (C1A, 2), F32)
    cc1o = nc.dram_tensor("cc1o", (C1A, 2), F32, addr_space="Shared")
    cc2i = nc.dram_tensor("cc2i", (C2A, 2), F32)
    cc2o = nc.dram_tensor("cc2o", (C2A, 2), F32, addr_space="Shared")
    cc3i = nc.dram_tensor("cc3i", (128, 4), F32)
    cc3o = nc.dram_tensor("cc3o", (128, 4), F32, addr_space="Shared")
    ggi = nc.dram_tensor("ggi", (2, BC, 128), F32)
    ggo = nc.dram_tensor("ggo", (NCORES, 2, BC, 128), F32, addr_space="Shared")
    dbg = {}
    if debug:
        dbg['nx'] = nc.dram_tensor("dbg_nx", (BC, S1, 3), F32, kind="ExternalOutput")
        dbg['idx1'] = nc.dram_tensor("dbg_idx1", (BC, 16, S1), U32, kind="ExternalOutput")
        dbg['l1p'] = nc.dram_tensor("dbg_l1p", (C1B, BC * S1), F32, kind="ExternalOutput")
        dbg['nx2'] = nc.dram_tensor("dbg_nx2", (BC, S2, 3), F32, kind="ExternalOutput")
        dbg['idx2'] = nc.dram_tensor("dbg_idx2", (BC, 16, S2), U32, kind="ExternalOutput")
        dbg['l2pa'] = nc.dram_tensor("dbg_l2pa", (128, BC * S2), F32, kind="ExternalOutput")
        dbg['l2pb'] = nc.dram_tensor("dbg_l2pb", (128, BC * S2), F32, kind="ExternalOutput")
        dbg['ga'] = nc.dram_tensor("dbg_ga", (128, BC), F32, kind="ExternalOutput")
        dbg['gb'] = nc.dram_tensor("dbg_gb", (128, BC), F32, kind="ExternalOutput")
    with tile.TileContext(nc) as tc:
        with nc.allow_non_contiguous_dma(reason="small strided restaging DMAs"), ExitStack() as ctx:
            emit(ctx, tc, d, out_d, F1d, nxd, F2d, nx2d, x1d, x2d,
                 (pre, xyzTre, nxTre, pre2, nxT2re, xyzT2re, gre, nxsd, nxsd2),
                 (cc1i, cc1o), (cc2i, cc2o), (cc3i, cc3o), (ggi, ggo), dbg, no_cc,
                 stop_after)
    nc.compile()
    return nc


def fps_loop(ctx, tc, pool, xs, ys, zs, xyzneg, nx, nsteps, free, name, ones1, psum,
             nxs=None):
    nc = tc.nc
    dist = pool.tile([128, free], F32, name=f"{name}_dist")
    nc.vector.memset(dist[:], 1e10)
    cneg = pool.tile([128, 3], F32, name=f"{name}_cneg")
    m8 = pool.tile([128, 8], F32, name=f"{name}_m8")
    gm = pool.tile([128, 1], F32, name=f"{name}_gm")
    r = pool.tile([128, 3], F32, name=f"{name}_r")
    junk = pool.tile([128, free], F32, name=f"{name}_junk")
    e1 = pool.tile([128, free], F32, name=f"{name}_e1")
    e2 = pool.tile([128, free], F32, name=f"{name}_e2")
    e3 = pool.tile([128, free], F32, name=f"{name}_e3")
    aa = pool.tile([128, free], F32, name=f"{name}_aa")
    ind0 = pool.tile([128, free], F32, name=f"{name}_ind0")
    nc.vector.memset(ind0[:], 0.0)
    nc.vector.memset(ind0[0:1, 0:1], 1.0)
    nc.vector.memset(ind0[64:65, 0:1], 1.0)
    # hi-half (partition 64:128) slices of partition_all_reduce return zeros on
    # HW, so route per-cloud reductions through disjoint COLUMNS of full-128
    # reduces: m2 packs per-cloud maxima, r6 per-cloud coordinate sums.
    m2 = pool.tile([128, 2], F32, name=f"{name}_m2")
    nc.vector.memset(m2[:], -1e30)
    gm2 = pool.tile([128, 2], F32, name=f"{name}_gm2")
    r6 = pool.tile([128, 6], F32, name=f"{name}_r6")
    nc.vector.memset(r6[:], 0.0)
    c6 = pool.tile([128, 6], F32, name=f"{name}_c6")

    def extract_c(mask_src, scal):
        # r[p,d] = sum_f (mask==scal ? -coord); full-128 add -> cneg everywhere
        for dd in range(3):
            nc.vector.scalar_tensor_tensor(
                out=junk[:], in0=mask_src, scalar=scal, in1=xyzneg[:, dd, :],
                op0=OP.is_equal, op1=OP.mult, accum_out=r[:, dd:dd + 1])
        nc.vector.tensor_copy(out=r6[0:64, 0:3], in_=r[0:64, :])
        nc.vector.tensor_copy(out=r6[64:128, 3:6], in_=r[64:128, :])
        nc.gpsimd.partition_all_reduce(c6[:], r6[:], 128, bass_isa.ReduceOp.add)
        nc.vector.tensor_copy(out=cneg[0:64, :], in_=c6[0:64, 0:3])
        nc.vector.tensor_copy(out=cneg[64:128, :], in_=c6[64:128, 3:6])

    def record(i):
        if nxs is not None:
            # all partitions hold their cloud-half's reduced value; negate into
            # the transposed [128, 3, nsteps] buffer
            nc.vector.tensor_scalar_mul(out=nxs[:, :, i], in0=cneg[:],
                                        scalar1=-1.0)
        else:
            nc.scalar.activation(out=nx[0:1, i, :], in_=cneg[0:1, :],
                                 func=AF.Copy, scale=-1.0)
            nc.scalar.activation(out=nx[64:65, i, :], in_=cneg[64:65, :],
                                 func=AF.Copy, scale=-1.0)

    extract_c(ind0[:], 1.0)
    record(0)
    for i in range(1, nsteps):
        nc.scalar.activation(out=e1[:], in_=xyzneg[:, 0, :], func=AF.Square,
                             scale=-1.0, bias=cneg[:, 0:1])
        nc.scalar.activation(out=e2[:], in_=xyzneg[:, 1, :], func=AF.Square,
                             scale=-1.0, bias=cneg[:, 1:2])
        nc.scalar.activation(out=e3[:], in_=xyzneg[:, 2, :], func=AF.Square,
                             scale=-1.0, bias=cneg[:, 2:3])
        nc.vector.tensor_tensor(out=aa[:], in0=e1[:], in1=e2[:], op=OP.add)
        nc.vector.tensor_tensor(out=e1[:], in0=aa[:], in1=e3[:], op=OP.add)
        nc.vector.tensor_tensor(out=dist[:], in0=dist[:], in1=e1[:], op=OP.min)
        nc.vector.max(m8[:], dist[:])
        nc.vector.tensor_copy(out=m2[0:64, 0:1], in_=m8[0:64, 0:1])
        nc.vector.tensor_copy(out=m2[64:128, 1:2], in_=m8[64:128, 0:1])
        nc.gpsimd.partition_all_reduce(gm2[:], m2[:], 128, bass_isa.ReduceOp.max)
        nc.vector.tensor_copy(out=gm[0:64, :], in_=gm2[0:64, 0:1])
        nc.vector.tensor_copy(out=gm[64:128, :], in_=gm2[64:128, 1:2])
        extract_c(dist[:], gm[:, 0:1])
        record(i)


def fps_loop_split(ctx, tc, pool, xyzsrc, nsteps, vw, name, nxs):
    """Per-cloud FPS chains: cloud c uses its own [128, vw] tiles spanning all
    128 partitions (point idx = p*vw + f), so reductions are full-128 (the only
    partition_all_reduce form that works on HW). The two chains interleave on
    the engines. xyzsrc(c, dd) -> DRAM AP of cloud c's coord row, (128, vw).
    Records into nxs[128, 3, nsteps] partition-halves (downstream layout
    unchanged: cloud c at partition 64*c)."""
    nc = tc.nc
    mf = max(vw, 8)
    T = {}
    for c in range(2):
        xyzneg = pool.tile([128, 3, vw], F32, name=f"{name}_xyzn{c}")
        for dd in range(3):
            xt = pool.tile([128, vw], F32, name=f"{name}_x{c}{dd}")
            nc.gpsimd.dma_start(xt[:], xyzsrc(c, dd))
            nc.vector.tensor_scalar_mul(out=xyzneg[:, dd, :], in0=xt[:],
                                        scalar1=-1.0)
        dist = pool.tile([128, mf], F32, name=f"{name}_dist{c}")
        nc.vector.memset(dist[:], 1e10)
        if mf > vw:
            nc.vector.memset(dist[:, vw:mf], -1e30)
        # per-step extract history: the add-reduce writes straight into
        # hist[:, :, i]; step i+1's bias reads hist[:, d, i]; one bulk negate
        # after the loop replaces per-step record ops
        hist = pool.tile([128, 3, nsteps], F32, name=f"{name}_hist{c}")
        m8 = pool.tile([128, 8], F32, name=f"{name}_m8{c}")
        gm = pool.tile([128, 1], F32, name=f"{name}_gm{c}")
        r = pool.tile([128, 3], F32, name=f"{name}_r{c}")
        junk = pool.tile([128, vw], F32, name=f"{name}_junk{c}")
        e1 = pool.tile([128, vw], F32, name=f"{name}_e1{c}")
        e2 = pool.tile([128, vw], F32, name=f"{name}_e2{c}")
        e3 = pool.tile([128, vw], F32, name=f"{name}_e3{c}")
        aa = pool.tile([128, vw], F32, name=f"{name}_aa{c}")
        ind0 = pool.tile([128, vw], F32, name=f"{name}_ind0{c}")
        nc.vector.memset(ind0[:], 0.0)
        nc.vector.memset(ind0[0:1, 0:1], 1.0)
        T[c] = (xyzneg, dist, hist, m8, gm, r, junk, e1, e2, e3, aa, ind0)

    def extract_c(c, mask_src, scal, i):
        xyzneg, dist, hist, m8, gm, r, junk = T[c][:7]
        for dd in range(3):
            nc.vector.scalar_tensor_tensor(
                out=junk[:], in0=mask_src, scalar=scal, in1=xyzneg[:, dd, :],
                op0=OP.is_equal, op1=OP.mult, accum_out=r[:, dd:dd + 1])
        nc.gpsimd.partition_all_reduce(hist[:, :, i], r[:], 128,
                                       bass_isa.ReduceOp.add)

    for c in range(2):
        extract_c(c, T[c][11][:], 1.0, 0)
    # phase-interleaved emission: both clouds' reduces are in flight before
    # either cloud's dependent phase queues, so the in-order engine queues
    # overlap the two serial chains.
    for i in range(1, nsteps):
        for c in range(2):
            xyzneg, dist, hist, m8, gm, r, junk, e1, e2, e3, aa, ind0 = T[c]
            nc.scalar.activation(out=e1[:], in_=xyzneg[:, 0, :], func=AF.Square,
                                 scale=-1.0, bias=hist[:, 0:1, i - 1])
            nc.scalar.activation(out=e2[:], in_=xyzneg[:, 1, :], func=AF.Square,
                                 scale=-1.0, bias=hist[:, 1:2, i - 1])
            nc.scalar.activation(out=e3[:], in_=xyzneg[:, 2, :], func=AF.Square,
                                 scale=-1.0, bias=hist[:, 2:3, i - 1])
            nc.vector.tensor_tensor(out=aa[:], in0=e1[:], in1=e2[:], op=OP.add)
            nc.vector.tensor_tensor(out=e1[:], in0=aa[:], in1=e3[:], op=OP.add)
            nc.vector.tensor_tensor(out=dist[:, 0:vw], in0=dist[:, 0:vw],
                                    in1=e1[:], op=OP.min)
            nc.vector.max(m8[:], dist[:])
            nc.gpsimd.partition_all_reduce(gm[:], m8[:, 0:1], 128,
                                           bass_isa.ReduceOp.max)
        for c in range(2):
            extract_c(c, T[c][1][:, 0:vw], T[c][4][:, 0:1], i)
    # bulk negate the per-step history into the shared nxs record buffer
    for c in range(2):
        nc.vector.tensor_scalar_mul(
            out=nxs[64 * c:64 * (c + 1), :, :],
            in0=T[c][2][64 * c:64 * (c + 1), :, :], scalar1=-1.0)


def bn_affine(tc, pool, sums, sqs, g_sb, be_sb, count, cpart, name):
    nc = tc.nc
    mean = pool.tile([cpart, 1], F32, name=f"{name}_mean")
    var = pool.tile([cpart, 1], F32, name=f"{name}_var")
    scale = pool.tile([cpart, 1], F32, name=f"{name}_scale")
    bias = pool.tile([cpart, 1], F32, name=f"{name}_bias")
    tmp = pool.tile([cpart, 1], F32, name=f"{name}_tmp")
    inv_n = 1.0 / float(count)
    nc.scalar.mul(mean[:], sums, inv_n)
    nc.scalar.mul(var[:], sqs, inv_n)
    nc.vector.tensor_tensor(out=tmp[:], in0=mean[:], in1=mean[:], op=OP.mult)
    nc.vector.tensor_tensor(out=var[:], in0=var[:], in1=tmp[:], op=OP.subtract)
    nc.vector.tensor_scalar_add(out=var[:], in0=var[:], scalar1=EPS)
    nc.vector.reciprocal(tmp[:], var[:])
    nc.scalar.activation(out=tmp[:], in_=tmp[:], func=AF.Sqrt)
    nc.vector.tensor_tensor(out=scale[:], in0=tmp[:], in1=g_sb, op=OP.mult)
    nc.vector.tensor_tensor(out=tmp[:], in0=mean[:], in1=scale[:], op=OP.mult)
    nc.vector.tensor_tensor(out=bias[:], in0=be_sb, in1=tmp[:], op=OP.subtract)
    scale_a = pool.tile([cpart, 1], F32, name=f"{name}_scale_a")
    bias_a = pool.tile([cpart, 1], F32, name=f"{name}_bias_a")
    nc.scalar.activation(out=scale_a[:], in_=scale[:], func=AF.Copy)
    nc.scalar.activation(out=bias_a[:], in_=bias[:], func=AF.Copy)
    return scale_a, bias_a


def emit(ctx, tc, d, out_d, F1d, nxd, F2d, nx2d, x1d, x2d, stg, cc1, cc2, cc3, gg, dbg,
         no_cc=False, stop_after=None):
    pre, xyzTre, nxTre, pre2, nxT2re, xyzT2re, gre, nxsd, nxsd2 = stg
    nc = tc.nc

    def bail():
        zout = sing.tile([16, 1], F32, name="zout")
        nc.vector.memset(zout[:], 0.0)
        nc.gpsimd.dma_start(out_d[:], zout[:])
    P = 128
    RG = [list(range(NCORES))]
    sing = ctx.enter_context(tc.tile_pool(name="sing", bufs=1))
    big = ctx.enter_context(tc.tile_pool(name="big", bufs=1))
    work = ctx.enter_context(tc.tile_pool(name="work", bufs=1))
    psum = ctx.enter_context(tc.tile_pool(name="psum", bufs=3, space="PSUM"))
    psumT = ctx.enter_context(tc.tile_pool(name="psumT", bufs=3, space="PSUM"))
    bpool = ctx.enter_context(tc.tile_pool(name="bnp", bufs=1))


    ones1 = sing.tile([1, 128], F32, name="ones1")
    nc.vector.memset(ones1[:], 1.0)
    wsb = {}
    for name, shp in WEIGHT_SHAPES.items():
        if len(shp) == 1:
            if shp[0] > 128:
                for hh in range(shp[0] // 128):
                    t = sing.tile([128, 1], F32, name=f"w_{name}_{hh}")
                    nc.gpsimd.dma_start(t[:], d[name][128 * hh:128 * (hh + 1), None])
                    wsb[f"{name}_{hh}"] = t
                continue
            t = sing.tile([shp[0], 1], F32, name=f"w_{name}")
            nc.gpsimd.dma_start(t[:], d[name][:, None])
        else:
            t = sing.tile(list(shp), F32, name=f"w_{name}")
            nc.gpsimd.dma_start(t[:], d[name][:])
        wsb[name] = t

    # ---- points load (restage so each SBUF tile = ONE DMA) ----
    for dd in range(3):
        for c in range(BC):
            nc.gpsimd.dma_start(
                pre[dd, 64 * c:64 * (c + 1), :],
                d['points'][c, :, dd].rearrange("(p f) -> p f", p=64))
    xs = sing.tile([P, 128], F32, name="xs")
    ys = sing.tile([P, 128], F32, name="ys")
    zs = sing.tile([P, 128], F32, name="zs")
    for dd, t in enumerate((xs, ys, zs)):
        nc.gpsimd.dma_start(t[:], pre[dd])
    xyzneg = sing.tile([P, 3, 128], F32, name="xyzneg")
    for dd, t in enumerate((xs, ys, zs)):
        nc.vector.tensor_scalar_mul(out=xyzneg[:, dd, :], in0=t[:], scalar1=-1.0)
    sqt0 = work.tile([P, 128], F32, name="sqt0", tag="sqt0")
    rnf = sing.tile([P, 128], F32, name="rnf")
    nc.scalar.activation(out=rnf[:], in_=xyzneg[:, 0, :], func=AF.Square)
    nc.scalar.activation(out=sqt0[:], in_=xyzneg[:, 1, :], func=AF.Square)
    nc.vector.tensor_tensor(out=rnf[:], in0=rnf[:], in1=sqt0[:], op=OP.add)
    nc.scalar.activation(out=sqt0[:], in_=xyzneg[:, 2, :], func=AF.Square)
    nc.vector.tensor_tensor(out=rnf[:], in0=rnf[:], in1=sqt0[:], op=OP.add)
    # xyzT staging: rows xyz from points, row3 = rn (per cloud), all in DRAM
    for c in range(BC):
        for dd, t in enumerate((xs, ys, zs)):
            nc.gpsimd.dma_start(xyzTre[c][dd:dd + 1, :], t[64 * c:64 * (c + 1), :])
        nc.gpsimd.dma_start(xyzTre[c][3:4, :], rnf[64 * c:64 * (c + 1), :])
    xyzTt = sing.tile([4, N], F32, name="xyzTt")

    def fill_xyzT(c):
        nc.gpsimd.dma_start(xyzTt[:], xyzTre[c][:])

    # ---- F1 rows-major -> F1d ----
    for c in range(BC):
        fill_xyzT(c)
        for j in range(8):
            ps = psum.tile([P, 512], F32, name="f1ps", tag="mm")
            st = work.tile([P, 512], F32, name="f1st", tag="f1st")
            for jj in range(8):
                ch = 8 * j + jj
                nc.tensor.matmul(ps[:, 64 * jj:64 * (jj + 1)],
                                 xyzTt[0:3, 128 * ch:128 * (ch + 1)],
                                 wsb['w1s_T'][:])
            nc.scalar.activation(out=st[:], in_=ps[:], func=AF.Copy)
            nc.gpsimd.dma_start(
                F1d[c][:].rearrange("(j p) q -> p j q", p=128)[:, 8 * j:8 * (j + 1), :],
                st[:].rearrange("p (j q) -> p j q", j=8))

    if stop_after == 1:
        bail()
        return

    # ---- FPS1 ----
    nxs = sing.tile([P, 3, S1], F32, name="nxs")
    fpool = ctx.enter_context(tc.tile_pool(name="fps1", bufs=1))
    if stop_after == 22:
        nc.vector.memset(nxs[:], 0.25)
    else:
        fps_loop_split(ctx, tc, fpool,
                       lambda c, dd: xyzTre[c][dd, :].rearrange(
                           "(p f) -> p f", p=128),
                       S1, 64, "f1", nxs)
    if stop_after == 21:
        bail()
        return
    # dump the transposed coord record to DRAM (contiguous per-partition rows),
    # then restage: cloud c's coords live on partition 64*c
    nc.gpsimd.dma_start(nxsd[:], nxs[:])
    nxT = sing.tile([3, BC * S1], F32, name="nxT")
    for c in range(BC):
        nc.gpsimd.dma_start(nxT[:, S1 * c:S1 * (c + 1)], nxsd[64 * c])
    q4T = sing.tile([4, BC * S1], F32, name="q4T")
    nc.vector.tensor_scalar_mul(out=q4T[0:3, :], in0=nxT[:], scalar1=2.0)
    nc.gpsimd.dma_start(q4T[3:4, :], d['constrow'][0:1, :])
    if dbg:
        for c in range(BC):
            nc.gpsimd.dma_start(dbg['nx'][c],
                                nxsd[64 * c].rearrange("dd q -> q dd"))

    if stop_after in (2, 22):
        bail()
        return

    Gc = sing.tile([C1A, BC * S1], F32, name="Gc")
    for h in range(2):
        psg = psum.tile([C1A, 512], F32, name="gcps", tag="mm")
        nc.tensor.matmul(psg[:], wsb['w1x_T'][:], nxT[:, 512 * h:512 * (h + 1)])
        nc.vector.tensor_copy(out=Gc[:, 512 * h:512 * (h + 1)], in_=psg[:])

    # ---- KNN1 + gather + conv1-space + stats ----
    scores = big.tile([P, N], F32, name="scores")
    sum1 = sing.tile([C1A, 128], F32, name="sum1")
    sq1 = sing.tile([C1A, 128], F32, name="sq1")
    l1pT = big.tile([C1B, BC * S1], F32, name="l1pT")

    for t in range(8):
        c = t // 4
        if t % 4 == 0:
            fill_xyzT(c)
        for jj in range(16):
            ps = psum.tile([P, 512], F32, name="knnps", tag="mm")
            nc.tensor.matmul(ps[:], q4T[:, 128 * t:128 * (t + 1)],
                             xyzTt[:, 512 * jj:512 * (jj + 1)])
            nc.scalar.activation(out=scores[:, 512 * jj:512 * (jj + 1)], in_=ps[:],
                                  func=AF.Copy)
        m8a = work.tile([P, 8], F32, name="m8a", tag="m8a")
        m8b = work.tile([P, 8], F32, name="m8b", tag="m8b")
        ia = work.tile([P, 16], U32, name="iab", tag="iab")
        nc.vector.max(m8a[:], scores[:])
        nc.vector.max_index(ia[:, 0:8], m8a[:], scores[:])
        nc.vector.match_replace(scores[:], m8a[:], scores[:], -1e30)
        nc.vector.max(m8b[:], scores[:])
        nc.vector.max_index(ia[:, 8:16], m8b[:], scores[:])
        if dbg:
            iaf = work.tile([P, 16], F32, name="iaf", tag="iaf")
            nc.vector.tensor_copy(out=iaf[:], in_=ia[:])
            pst = psumT.tile([16, P], F32, name="idxps", tag="T")
            nc.tensor.transpose(pst[:], iaf[:], wsb['ident'][:])
            dcp = work.tile([16, P], U32, name="dcp", tag="dcp")
            nc.vector.tensor_copy(out=dcp[:], in_=pst[:])
            nc.gpsimd.dma_start(dbg['idx1'][c, :, 128 * (t % 4):128 * (t % 4 + 1)],
                              dcp[:])
        # gather + conv1-space blocks, k-major columns: col = 512*k + 128*(t%4) + q
        for k in range(K):
            gblk = work.tile([P, C1A], F32, name="gblk", tag="gblk")
            nc.gpsimd.indirect_dma_start(
                out=gblk[:], out_offset=None, in_=F1d[c][:],
                in_offset=bass.IndirectOffsetOnAxis(ap=ia[:, k:k + 1], axis=0))
            psx1 = psumT.tile([C1A, P], F32, name="psx1", tag="T")
            nc.tensor.transpose(psx1[:], gblk[:], wsb['ident'][:])
            q0 = S1 * c + 128 * (t % 4)
            xblk = work.tile([C1A, P], F32, name="xblk", tag="xblk")
            nc.vector.scalar_tensor_tensor(
                out=xblk[:], in0=psx1[:], scalar=0.0,
                in1=Gc[:, q0:q0 + 128],
                op0=OP.bypass, op1=OP.subtract,
                accum_out=sum1[:, 64 * c + 16 * (t % 4) + k:64 * c + 16 * (t % 4) + k + 1])
            sqt = work.tile([C1A, P], F32, name="sqt", tag="sqt")
            nc.scalar.activation(
                out=sqt[:], in_=xblk[:], func=AF.Square,
                accum_out=sq1[:, 64 * c + 16 * (t % 4) + k:64 * c + 16 * (t % 4) + k + 1])
            nc.gpsimd.dma_start(
                x1d[c, :, 512 * k + 128 * (t % 4):512 * k + 128 * (t % 4) + 128],
                xblk[:])

    red1 = sing.tile([C1A, 2], F32, name="red1")
    nc.vector.tensor_reduce(out=red1[:, 0:1], in_=sum1[:, None, :], axis=AX.X, op=OP.add)
    nc.vector.tensor_reduce(out=red1[:, 1:2], in_=sq1[:, None, :], axis=AX.X, op=OP.add)
    nc.gpsimd.dma_start(cc1[0][:], red1[:])
    if stop_after == 3:
        bail()
        return
    if not no_cc:
        nc.gpsimd.collective_compute("AllReduce", OP.add, replica_groups=RG,
                                     ins=[cc1[0][:]], outs=[cc1[1][:]])
    stat1 = sing.tile([C1A, 2], F32, name="stat1")
    nc.gpsimd.dma_start(stat1[:], cc1[0 if no_cc else 1][:])
    sc1, bi1 = bn_affine(tc, bpool, stat1[:, 0:1], stat1[:, 1:2],
                         wsb['bn1_g'][:], wsb['bn1_be'][:], B * S1 * K, C1A, "bn1")

    for c in range(BC):
        for k in range(K):
            col = 512 * k
            x1c = work.tile([C1A, 512], F32, name="x1c", tag="x1c")
            nc.gpsimd.dma_start(x1c[:], x1d[c, :, col:col + 512])
            x1v = work.tile([C1A, 512], F32, name="x1v", tag="x1v")
            nc.scalar.activation(out=x1v[:], in_=x1c[:], func=AF.Copy)
            h1 = work.tile([C1A, 512], F32, name="h1", tag="h1")
            nc.scalar.activation(out=h1[:], in_=x1v[:], func=AF.Relu,
                                 scale=sc1[:], bias=bi1[:])
            ps = psum.tile([C1B, 512], F32, name="c2ps", tag="mm")
            nc.tensor.matmul(ps[:], wsb['w2_T'][:], h1[:])
            sl = l1pT[:, S1 * c:S1 * (c + 1)]
            if k == 0:
                nc.vector.tensor_copy(out=sl, in_=ps[:])
            else:
                nc.vector.tensor_tensor(out=sl, in0=sl, in1=ps[:], op=OP.max)
    nc.vector.tensor_scalar(out=l1pT[:], in0=l1pT[:], scalar1=wsb['b1c2'][:],
                            scalar2=None, op0=OP.add)
    if dbg:
        nc.gpsimd.dma_start(dbg['l1p'][:], l1pT[:])
    if stop_after == 4:
        bail()
        return

    # ---- SA2 prep ----
    zpad = sing.tile([128, 60], F32, name="zpad")
    nc.vector.memset(zpad[:], 0.0)
    for c in range(BC):
        nc.gpsimd.dma_start(F2d[c][:, 0:3],
                            nxsd[64 * c].rearrange("dd q -> q dd"))
        for j in range(4):
            nc.gpsimd.dma_start(F2d[c][128 * j:128 * (j + 1), 3], zpad[:, 0:1])
            nc.gpsimd.dma_start(F2d[c][128 * j:128 * (j + 1), 132:192], zpad[:])
        for j in range(4):
            pst = psumT.tile([P, P], F32, name="ftps", tag="T")
            nc.tensor.transpose(pst[:], l1pT[:, S1 * c + 128 * j:S1 * c + 128 * (j + 1)],
                                wsb['ident'][:])
            stg = work.tile([P, P], F32, name="fstg", tag="fstg")
            nc.vector.tensor_copy(out=stg[:], in_=pst[:])
            nc.gpsimd.dma_start(F2d[c][128 * j:128 * (j + 1), 4:132], stg[:])

    nxs2 = sing.tile([P, 3, S2], F32, name="nxs2")
    fpool2 = ctx.enter_context(tc.tile_pool(name="fps2", bufs=1))
    fps_loop_split(ctx, tc, fpool2,
                   lambda c, dd: nxsd[64 * c, dd, :].rearrange(
                       "(p f) -> p f", p=128),
                   S2, 4, "f2", nxs2)
    nc.gpsimd.dma_start(nxsd2[:], nxs2[:])
    if dbg:
        for c in range(BC):
            nc.gpsimd.dma_start(dbg['nx2'][c],
                                nxsd2[64 * c].rearrange("dd q -> q dd"))

    nxT2f = sing.tile([4, BC * S2], F32, name="nxT2f")
    for c in range(BC):
        nc.gpsimd.dma_start(nxT2f[0:3, S2 * c:S2 * (c + 1)], nxsd2[64 * c])
    nc.gpsimd.dma_start(nxT2f[3:4, :], d['constrow'][1:2, 0:BC * S2])
    nxT2 = nxT2f
    q4T2 = sing.tile([3, BC * S2], F32, name="q4T2")
    nc.vector.tensor_scalar_mul(out=q4T2[:], in0=nxT2[0:3, :], scalar1=2.0)
    monerow = sing.tile([1, 128], F32, name="monerow")
    nc.gpsimd.dma_start(monerow[:], d['constrow'][0:1, 0:128])
    xyzT2 = [sing.tile([3, S1], F32, name=f"xyzT2_{c}") for c in range(BC)]
    rn2ts = [sing.tile([1, S1], F32, name=f"rn2t_{c}") for c in range(BC)]
    ones3 = sing.tile([3, 1], F32, name="ones3")
    nc.vector.memset(ones3[:], 1.0)
    for c in range(BC):
        nc.gpsimd.dma_start(xyzT2[c][:], nxsd[64 * c])
        sq2t = work.tile([3, S1], F32, name="sq2t", tag="sq2t")
        nc.scalar.activation(out=sq2t[:], in_=xyzT2[c][:], func=AF.Square)
        psr = psum.tile([1, S1], F32, name="rnps", tag="mm")
        nc.tensor.matmul(psr[:], ones3[:], sq2t[:])
        nc.vector.tensor_copy(out=rn2ts[c][:], in_=psr[:])

    if stop_after == 5:
        bail()
        return

    # ---- KNN2 + gather + MLP2 ----
    sum2 = sing.tile([C2A, 8], F32, name="sum2")
    sq2 = sing.tile([C2A, 8], F32, name="sq2")
    l2paT = big.tile([128, BC * S2], F32, name="l2paT")
    x2sb = big.tile([C2A, BC * S2 * K], F32, name="x2sb")
    l2pbT = big.tile([128, BC * S2], F32, name="l2pbT")

    for c in range(BC):
        ps = psum.tile([P, S1], F32, name="kn2ps", tag="mm")
        nc.tensor.matmul(ps[:], q4T2[:, S2 * c:S2 * (c + 1)], xyzT2[c][:],
                         start=True, stop=False)
        nc.tensor.matmul(ps[:], monerow[:], rn2ts[c][:], start=False, stop=True)
        sc2t = work.tile([P, S1], F32, name="sc2t", tag="sc2t")
        nc.scalar.activation(out=sc2t[:], in_=ps[:], func=AF.Copy)
        m8a = work.tile([P, 8], F32, name="m8a2", tag="m8a2")
        m8b = work.tile([P, 8], F32, name="m8b2", tag="m8b2")
        ia = work.tile([P, 16], U32, name="iab2", tag="iab2")
        nc.vector.max(m8a[:], sc2t[:])
        nc.vector.max_index(ia[:, 0:8], m8a[:], sc2t[:])
        nc.vector.match_replace(sc2t[:], m8a[:], sc2t[:], -1e30)
        nc.vector.max(m8b[:], sc2t[:])
        nc.vector.max_index(ia[:, 8:16], m8b[:], sc2t[:])
        if dbg:
            iaf2 = work.tile([P, 16], F32, name="iaf2", tag="iaf2")
            nc.vector.tensor_copy(out=iaf2[:], in_=ia[:])
            pst2 = psumT.tile([16, P], F32, name="idx2ps", tag="T")
            nc.tensor.transpose(pst2[:], iaf2[:], wsb['ident'][:])
            dcp2 = work.tile([16, P], U32, name="dcp2", tag="dcp2")
            nc.vector.tensor_copy(out=dcp2[:], in_=pst2[:])
            nc.gpsimd.dma_start(dbg['idx2'][c], dcp2[:])
        rhx = big.tile([4, S2 * K], F32, name="rhx", tag="rhx")
        rhp = big.tile([C2A, S2 * K], F32, name="rhp", tag="rhp")
        for k in range(K):
            gblk2 = work.tile([P, 192], F32, name="gblk2", tag="gblk2")
            nc.gpsimd.indirect_dma_start(
                out=gblk2[:], out_offset=None, in_=F2d[c][:],
                in_offset=bass.IndirectOffsetOnAxis(ap=ia[:, k:k + 1], axis=0))
            psx = psumT.tile([4, P], F32, name="psx", tag="T")
            nc.tensor.transpose(psx[:], gblk2[:, 0:4], wsb['ident'][:])
            nc.vector.tensor_copy(out=rhx[:, 128 * k:128 * (k + 1)], in_=psx[:])
            psp = psumT.tile([C2A, P], F32, name="psp", tag="T")
            nc.tensor.transpose(psp[:], gblk2[:, 4:132], wsb['ident'][:])
            nc.vector.tensor_copy(out=rhp[:, 128 * k:128 * (k + 1)], in_=psp[:])
        for chk in range(4):
            col = 512 * chk
            ps2 = psum.tile([C2A, 512], F32, name="c1ps2", tag="mm")
            nc.tensor.matmul(ps2[:], wsb['A2x_T'][:], rhx[:, col:col + 512],
                             start=True, stop=False)
            nc.tensor.matmul(ps2[:], wsb['A2p_T'][:], rhp[:, col:col + 512],
                             start=False, stop=False)
            nc.tensor.matmul(
                ps2[:], wsb['A2xn_T'][:],
                nxT2[:, S2 * c:S2 * (c + 1)][:, None, :].broadcast_to((4, 4, S2)),
                start=False, stop=True)
            x2col = S2 * K * c + col
            nc.scalar.activation(out=x2sb[:, x2col:x2col + 512], in_=ps2[:],
                                 func=AF.Copy,
                                 accum_out=sum2[:, 4 * c + chk:4 * c + chk + 1])
            sqt2 = work.tile([C2A, 512], F32, name="sqt2", tag="sqt2")
            nc.scalar.activation(out=sqt2[:], in_=x2sb[:, x2col:x2col + 512],
                                 func=AF.Square,
                                 accum_out=sq2[:, 4 * c + chk:4 * c + chk + 1])

    red2 = sing.tile([C2A, 2], F32, name="red2")
    nc.vector.tensor_reduce(out=red2[:, 0:1], in_=sum2[:, None, :], axis=AX.X, op=OP.add)
    nc.vector.tensor_reduce(out=red2[:, 1:2], in_=sq2[:, None, :], axis=AX.X, op=OP.add)
    nc.gpsimd.dma_start(cc2[0][:], red2[:])
    if not no_cc:
        nc.gpsimd.collective_compute("AllReduce", OP.add, replica_groups=RG,
                                     ins=[cc2[0][:]], outs=[cc2[1][:]])
    stat2 = sing.tile([C2A, 2], F32, name="stat2")
    nc.gpsimd.dma_start(stat2[:], cc2[0 if no_cc else 1][:])
    sc2, bi2 = bn_affine(tc, bpool, stat2[:, 0:1], stat2[:, 1:2],
                         wsb['bn2_g'][:], wsb['bn2_be'][:], B * S2 * K, C2A, "bn2")

    for c in range(BC):
        for chk in range(4):
            col = S2 * K * c + 512 * chk
            h2 = work.tile([C2A, 512], F32, name="h2", tag="h2")
            nc.scalar.activation(out=h2[:], in_=x2sb[:, col:col + 512],
                                 func=AF.Relu, scale=sc2[:], bias=bi2[:])
            psa = psum.tile([128, 512], F32, name="c2psa", tag="mm")
            nc.tensor.matmul(psa[:], wsb['B2a_T'][:], h2[:])
            psb = psum.tile([128, 512], F32, name="c2psb", tag="mm")
            nc.tensor.matmul(psb[:], wsb['B2b_T'][:], h2[:])
            for half, (pp, ll) in enumerate(((psa, l2paT), (psb, l2pbT))):
                sl = ll[:, S2 * c:S2 * (c + 1)]
                for kk in range(4):
                    yk = pp[:, 128 * kk:128 * (kk + 1)]
                    if chk == 0 and kk == 0:
                        nc.vector.tensor_copy(out=sl, in_=yk)
                    else:
                        nc.vector.tensor_tensor(out=sl, in0=sl, in1=yk, op=OP.max)
    nc.vector.tensor_scalar(out=l2paT[:], in0=l2paT[:], scalar1=wsb['b2c2_0'][:],
                            scalar2=None, op0=OP.add)
    nc.vector.tensor_scalar(out=l2pbT[:], in0=l2pbT[:], scalar1=wsb['b2c2_1'][:],
                            scalar2=None, op0=OP.add)
    if dbg:
        nc.gpsimd.dma_start(dbg['l2pa'][:], l2paT[:])
        nc.gpsimd.dma_start(dbg['l2pb'][:], l2pbT[:])
    if stop_after == 6:
        bail()
        return

    # ---- SA3 ----
    NR3 = BC * S2
    x3a = big.tile([128, NR3], F32, name="x3a")
    x3b = big.tile([128, NR3], F32, name="x3b")
    s3 = sing.tile([128, 4], F32, name="s3")
    for half, (x3, xw, paw, pbw) in enumerate(
            ((x3a, 'A3x_Ta', 'A3pa_Ta', 'A3pb_Ta'),
             (x3b, 'A3x_Tb', 'A3pa_Tb', 'A3pb_Tb'))):
        ps3 = psum.tile([128, NR3], F32, name="ps3", tag="mm")
        nc.tensor.matmul(ps3[:], wsb[xw][:], nxT2[:], start=True, stop=False)
        nc.tensor.matmul(ps3[:], wsb[paw][:], l2paT[:], start=False, stop=False)
        nc.tensor.matmul(ps3[:], wsb[pbw][:], l2pbT[:], start=False, stop=True)
        nc.scalar.activation(out=x3[:], in_=ps3[:], func=AF.Copy,
                             accum_out=s3[:, 2 * half:2 * half + 1])
        sqt3 = work.tile([128, NR3], F32, name="sqt3", tag="sqt3")
        nc.scalar.activation(out=sqt3[:], in_=x3[:], func=AF.Square,
                             accum_out=s3[:, 2 * half + 1:2 * half + 2])
    nc.gpsimd.dma_start(cc3[0][:], s3[:])
    if not no_cc:
        nc.gpsimd.collective_compute("AllReduce", OP.add, replica_groups=RG,
                                     ins=[cc3[0][:]], outs=[cc3[1][:]])
    stat3 = sing.tile([128, 4], F32, name="stat3")
    nc.gpsimd.dma_start(stat3[:], cc3[0 if no_cc else 1][:])
    n3 = B * S2
    sc3a, bi3a = bn_affine(tc, bpool, stat3[:, 0:1], stat3[:, 1:2],
                           wsb['bn3_g_0'][:], wsb['bn3_be_0'][:], n3, 128, "bn3a")
    sc3b, bi3b = bn_affine(tc, bpool, stat3[:, 2:3], stat3[:, 3:4],
                           wsb['bn3_g_1'][:], wsb['bn3_be_1'][:], n3, 128, "bn3b")
    h3a = work.tile([128, NR3], F32, name="h3a")
    h3b = work.tile([128, NR3], F32, name="h3b")
    nc.scalar.activation(out=h3a[:], in_=x3a[:], func=AF.Relu, scale=sc3a[:], bias=bi3a[:])
    nc.scalar.activation(out=h3b[:], in_=x3b[:], func=AF.Relu, scale=sc3b[:], bias=bi3b[:])
    ga = sing.tile([128, BC], F32, name="ga")
    gb = sing.tile([128, BC], F32, name="gb")
    for half, g in ((0, ga), (1, gb)):
        psg3 = psum.tile([128, NR3], F32, name="psg3", tag="mm")
        nc.tensor.matmul(psg3[:], wsb[f'C3_{half}0'][:], h3a[:], start=True, stop=False)
        nc.tensor.matmul(psg3[:], wsb[f'C3_{half}1'][:], h3b[:], start=False, stop=True)
        nc.vector.tensor_reduce(out=g[:], in_=psg3[:].rearrange("p (c q) -> p c q", c=BC),
                                axis=AX.X, op=OP.max)
        nc.vector.tensor_scalar(out=g[:], in0=g[:],
                                scalar1=wsb[f'b3c2_{half}'][:],
                                scalar2=None, op0=OP.add)
    if dbg:
        nc.gpsimd.dma_start(dbg['ga'][:], ga[:])
        nc.gpsimd.dma_start(dbg['gb'][:], gb[:])
    if stop_after == 7:
        bail()
        return

    # ---- AllGather + FC head ----
    nc.gpsimd.dma_start(gg[0][0].rearrange("c p -> p c"), ga[:])
    nc.gpsimd.dma_start(gg[0][1].rearrange("c p -> p c"), gb[:])
    if not no_cc:
        nc.gpsimd.collective_compute("AllGather", OP.bypass, replica_groups=RG,
                                     ins=[gg[0][:]], outs=[gg[1][:]])
    for n in range(NCORES):
        ggsrc = gg[0] if no_cc else gg[1][n]
        nc.gpsimd.dma_start(gre[0, :, BC * n:BC * (n + 1)],
                            ggsrc[0].rearrange("c p -> p c"))
        nc.gpsimd.dma_start(gre[1, :, BC * n:BC * (n + 1)],
                            ggsrc[1].rearrange("c p -> p c"))
    gaal = sing.tile([128, B], F32, name="gaal")
    gbal = sing.tile([128, B], F32, name="gbal")
    nc.gpsimd.dma_start(gaal[:], gre[0])
    nc.gpsimd.dma_start(gbal[:], gre[1])

    def fc_layer(xins, wnames, gslice, beslice, name, alpha=0.2):
        ps = psum.tile([128, B], F32, name=f"{name}ps", tag="mm")
        for i, (xt, wn) in enumerate(zip(xins, wnames)):
            nc.tensor.matmul(ps[:], wsb[wn][:], xt[:], start=(i == 0),
                             stop=(i == len(xins) - 1))
        xsb = work.tile([128, B], F32, name=f"{name}x", tag=f"{name}x")
        ssq = sing.tile([128, 2], F32, name=f"{name}ssq")
        nc.scalar.activation(out=xsb[:], in_=ps[:], func=AF.Copy,
                             accum_out=ssq[:, 0:1])
        sqf = work.tile([128, B], F32, name=f"{name}sq", tag=f"{name}sq")
        nc.scalar.activation(out=sqf[:], in_=xsb[:], func=AF.Square,
                             accum_out=ssq[:, 1:2])
        sc, bi = bn_affine(tc, bpool, ssq[:, 0:1], ssq[:, 1:2], gslice, beslice,
                           B, 128, name)
        act = work.tile([128, B], F32, name=f"{name}act", tag=f"{name}act")
        vv = work.tile([128, B], F32, name=f"{name}vv", tag=f"{name}vv")
        nc.scalar.activation(out=vv[:], in_=xsb[:], func=AF.Identity,
                             scale=sc[:], bias=bi[:])
        av = work.tile([128, B], F32, name=f"{name}av", tag=f"{name}av")
        nc.vector.tensor_scalar_mul(out=av[:], in0=vv[:], scalar1=alpha)
        nc.vector.tensor_tensor(out=act[:], in0=vv[:], in1=av[:], op=OP.max)
        return act

    h1a = fc_layer([gaal, gbal], ['FC1_00', 'FC1_01'],
                   wsb['fbn1_g_0'][:], wsb['fbn1_be_0'][:], "fc1a")
    h1b = fc_layer([gaal, gbal], ['FC1_10', 'FC1_11'],
                   wsb['fbn1_g_1'][:], wsb['fbn1_be_1'][:], "fc1b")
    h2f = fc_layer([h1a, h1b], ['FC2_0', 'FC2_1'],
                   wsb['fbn2_g'][:], wsb['fbn2_be'][:], "fc2")
    ps_o = psum.tile([1, B], F32, name="ps_o", tag="mm")
    nc.tensor.matmul(ps_o[:], wsb['FC3_T'][:], h2f[:])
    o_sb = sing.tile([1, B], F32, name="o_sb")
    nc.vector.tensor_scalar(out=o_sb[:], in0=ps_o[:], scalar1=wsb['fc3_b'][:],
                            scalar2=None, op0=OP.add)
    nc.gpsimd.dma_start(out_d[:, 0][None, :], o_sb[:])


# ===================== host-side entry point =====================
_NC_CACHE = {}


def _get_nc():
    if 'nc' not in _NC_CACHE:
        _NC_CACHE['nc'] = build_nc(debug=False)
    return _NC_CACHE['nc']


def _kernel_numpy(inputs):
    """Exact numpy fallback of the reference model (host-side)."""
    f = np.float32
    pts = np.asarray(inputs['points'], f)
    Bn, Nn = pts.shape[0], pts.shape[1]

    def fps(x, npoint):
        n = x.shape[0]
        xs_, ys_, zs_ = x[:, 0], x[:, 1], x[:, 2]
        dist = np.full(n, 1e10, f)
        idxs = np.zeros(npoint, np.int64)
        far = 0
        for i in range(npoint):
            idxs[i] = far
            c = x[far]
            e = ((xs_ - c[0]) ** 2).astype(f) + ((ys_ - c[1]) ** 2).astype(f)
            dist = np.minimum(dist, (e + ((zs_ - c[2]) ** 2).astype(f)).astype(f))
            far = int(np.argmax(dist))
        return idxs

    def knn(q, r, k):
        d = (np.sum(q ** 2, -1)[:, None] - 2.0 * (q @ r.T) + np.sum(r ** 2, -1)[None, :])
        return np.argsort(d, axis=1, kind='stable')[:, :k]

    def bn(x, g, b, axes):
        m = x.mean(axes, keepdims=True, dtype=np.float64).astype(f)
        v = x.var(axes, keepdims=True).astype(f)
        return (x - m) / np.sqrt(v + 1e-5) * g + b

    def mlp2(x, w1, b1, g1, be1, w2, b2, axes):
        h = x @ np.asarray(w1, f).T + b1
        h = np.maximum(bn(h, g1, be1, axes), 0)
        return h @ np.asarray(w2, f).T + b2

    def sa_knn(xyz, ptsf, npoint, k, w1, b1, g1, be1, w2, b2):
        nx_l, np_l, gx_l, gp_l = [], [], [], []
        for b_ in range(xyz.shape[0]):
            fi = fps(xyz[b_], npoint)
            nxb = xyz[b_][fi]
            idx = knn(nxb, xyz[b_], k)
            gx_l.append(xyz[b_][idx] - nxb[:, None, :])
            gp_l.append(ptsf[b_][idx])
            nx_l.append(nxb)
        nxa = np.stack(nx_l); gx = np.stack(gx_l); gp = np.stack(gp_l)
        grouped = np.concatenate([gx, gp], -1)
        out = mlp2(grouped, w1, b1, g1, be1, w2, b2, (0, 1, 2))
        return nxa, out.max(2)

    i = {k: np.asarray(v, f) for k, v in inputs.items()}
    l1x, l1p = sa_knn(pts, pts, 512, 16, i['sa1_c1_w'], i['sa1_c1_b'],
                      i['sa1_bn_g'], i['sa1_bn_be'], i['sa1_c2_w'], i['sa1_c2_b'])
    l2x, l2p = sa_knn(l1x, l1p, 128, 16, i['sa2_c1_w'], i['sa2_c1_b'],
                      i['sa2_bn_g'], i['sa2_bn_be'], i['sa2_c2_w'], i['sa2_c2_b'])
    grouped = np.concatenate([l2x, l2p], -1)[:, None]
    g = mlp2(grouped, i['sa3_c1_w'], i['sa3_c1_b'], i['sa3_bn_g'], i['sa3_bn_be'],
             i['sa3_c2_w'], i['sa3_c2_b'], (0, 1, 2)).max(2)[:, 0]

    def lrelu(x):
        return np.where(x > 0, x, 0.2 * x)
    h = g @ i['fc1_w'].T + i['fc1_b']
    h = lrelu(bn(h, i['fc1_bn_g'], i['fc1_bn_be'], (0,)))
    h = h @ i['fc2_w'].T + i['fc2_b']
    h = lrelu(bn(h, i['fc2_bn_g'], i['fc2_bn_be'], (0,)))
    return (h @ i['fc3_w'].T + i['fc3_b']).astype(f)


def kernel(**inputs):
    """Full-input entry: shard over 8 NeuronCores, run, return (16,1) logits."""
    try:
        from concourse.bass_utils import run_bass_kernel_spmd
        w = prep_common_weights(inputs)
        pts = np.asarray(inputs['points'], np.float32)
        in_maps = []
        for t in range(NCORES):
            m = {'points': np.ascontiguousarray(pts[BC * t:BC * (t + 1)])}
            for name, shp in WEIGHT_SHAPES.items():
                m[name] = np.ascontiguousarray(w[name].reshape(shp))
            in_maps.append(m)
        nc = _get_nc()
        res = run_bass_kernel_spmd(nc, in_maps, list(range(NCORES)))
        out = np.asarray(res.results[0]['out'], np.float32)
        return out
    except Exception:
        import traceback
        traceback.print_exc()
        return _kernel_numpy(inputs)



# revision 52
# speedup vs baseline: 1.0176x; 1.0147x over previous
"""PointCloudDiscriminator Trainium2 Bass kernel (SPMD 8 cores, 2 clouds/core)."""
import numpy as np
from contextlib import ExitStack

import concourse.bass as bass
import concourse.bacc as bacc_mod
import concourse.tile as tile
import concourse.mybir as mybir
from concourse import bass_isa

F32 = mybir.dt.float32
U32 = mybir.dt.uint32
I16 = mybir.dt.int16
AF = mybir.ActivationFunctionType
OP = mybir.AluOpType
AX = mybir.AxisListType
EPS = 1e-5

B, N, S1, S2, K = 16, 8192, 512, 128, 16
NCORES = 8
BC = B // NCORES
C1A, C1B = 64, 128
C2A, C2B = 128, 256


def prep_common_weights(inp):
    f = np.float32
    w = {}
    A = lambda x: np.ascontiguousarray(np.asarray(x, f))
    pad4 = lambda a: np.concatenate([a, np.zeros((1, a.shape[1]), f)], 0)
    w1 = A(inp['sa1_c1_w'])
    w['w1s_T'] = A((w1[:, :3] + w1[:, 3:]).T)
    w['w1x_T'] = A(w1[:, :3].T)
    w['w2_T'] = A(np.asarray(inp['sa1_c2_w'], f).T)
    w['bn1_g'] = A(inp['sa1_bn_g']); w['bn1_be'] = A(inp['sa1_bn_be'])
    w['b1c2'] = A(inp['sa1_c2_b'])
    w2c1 = A(inp['sa2_c1_w'])
    w['A2x_T'] = pad4(A(w2c1[:, :3].T)); w['A2xn_T'] = A(-w['A2x_T'])
    w['A2p_T'] = A(w2c1[:, 3:].T)
    w['bn2_g'] = A(inp['sa2_bn_g']); w['bn2_be'] = A(inp['sa2_bn_be'])
    w2c2 = A(inp['sa2_c2_w'])
    w['B2a_T'] = A(w2c2[:128].T); w['B2b_T'] = A(w2c2[128:].T)
    w['b2c2'] = A(inp['sa2_c2_b'])
    w3c1 = A(inp['sa3_c1_w'])
    w['A3x_Ta'] = pad4(A(w3c1[:128, :3].T)); w['A3x_Tb'] = pad4(A(w3c1[128:, :3].T))
    w['A3pa_Ta'] = A(w3c1[:128, 3:131].T); w['A3pa_Tb'] = A(w3c1[128:, 3:131].T)
    w['A3pb_Ta'] = A(w3c1[:128, 131:259].T); w['A3pb_Tb'] = A(w3c1[128:, 131:259].T)
    w['bn3_g'] = A(inp['sa3_bn_g']); w['bn3_be'] = A(inp['sa3_bn_be'])
    w3c2 = A(inp['sa3_c2_w'])
    for r in range(2):
        for c in range(2):
            w[f'C3_{r}{c}'] = A(w3c2[128 * r:128 * (r + 1), 128 * c:128 * (c + 1)].T)
    w['b3c2'] = A(inp['sa3_c2_b'])
    f1 = A(inp['fc1_w'])
    for r in range(2):
        for c in range(2):
            w[f'FC1_{r}{c}'] = A(f1[128 * r:128 * (r + 1), 128 * c:128 * (c + 1)].T)
    w['fbn1_g'] = A(inp['fc1_bn_g']); w['fbn1_be'] = A(inp['fc1_bn_be'])
    f2 = A(inp['fc2_w'])
    w['FC2_0'] = A(f2[:, :128].T); w['FC2_1'] = A(f2[:, 128:].T)
    w['fbn2_g'] = A(inp['fc2_bn_g']); w['fbn2_be'] = A(inp['fc2_bn_be'])
    w['FC3_T'] = A(np.asarray(inp['fc3_w'], f).T)
    w['fc3_b'] = A(inp['fc3_b'])
    w['ident'] = np.eye(128, dtype=f)
    w['constrow'] = np.stack([np.full(1024, -1.0, f), np.zeros(1024, f)])
    return w


WEIGHT_SHAPES = {
    'w1s_T': (3, 64), 'w1x_T': (3, 64), 'w2_T': (64, 128),
    'bn1_g': (64,), 'bn1_be': (64,), 'b1c2': (128,),
    'A2x_T': (4, 128), 'A2xn_T': (4, 128), 'A2p_T': (128, 128),
    'bn2_g': (128,), 'bn2_be': (128,),
    'B2a_T': (128, 128), 'B2b_T': (128, 128), 'b2c2': (256,),
    'A3x_Ta': (4, 128), 'A3x_Tb': (4, 128),
    'A3pa_Ta': (128, 128), 'A3pa_Tb': (128, 128),
    'A3pb_Ta': (128, 128), 'A3pb_Tb': (128, 128),
    'bn3_g': (256,), 'bn3_be': (256,),
    'C3_00': (128, 128), 'C3_01': (128, 128), 'C3_10': (128, 128), 'C3_11': (128, 128),
    'b3c2': (256,),
    'FC1_00': (128, 128), 'FC1_01': (128, 128), 'FC1_10': (128, 128), 'FC1_11': (128, 128),
    'fbn1_g': (256,), 'fbn1_be': (256,),
    'FC2_0': (128, 128), 'FC2_1': (128, 128),
    'fbn2_g': (128,), 'fbn2_be': (128,),
    'FC3_T': (128, 1), 'fc3_b': (1,),
    'ident': (128, 128),
    'constrow': (2, 1024),
}


def build_nc(debug=False, no_cc=False, stop_after=None):
    nc = bacc_mod.Bacc()
    d = {'points': nc.dram_tensor("points", (BC, N, 3), F32, kind="ExternalInput")}
    for name, shp in WEIGHT_SHAPES.items():
        d[name] = nc.dram_tensor(name, shp, F32, kind="ExternalInput")
    out_d = nc.dram_tensor("out", (B, 1), F32, kind="ExternalOutput")
    F1d = [nc.dram_tensor(f"F1d{c}", (N, C1A), F32) for c in range(BC)]
    pre = nc.dram_tensor("pre", (3, 128, 128), F32)
    xyzTre = [nc.dram_tensor(f"xyzTre{c}", (4, N), F32) for c in range(BC)]
    nxTre = nc.dram_tensor("nxTre", (3, BC * S1), F32)
    pre2 = nc.dram_tensor("pre2", (3, 128, 8), F32)
    nxT2re = nc.dram_tensor("nxT2re", (3, BC * S2), F32)
    xyzT2re = [nc.dram_tensor(f"xyzT2re{c}", (4, S1), F32) for c in range(BC)]
    gre = nc.dram_tensor("gre", (2, 128, B), F32)
    nxd = nc.dram_tensor("nxd", (BC, S1, 3), F32)
    nxsd = nc.dram_tensor("nxsd", (128, 3, S1), F32)
    nxsd2 = nc.dram_tensor("nxsd2", (128, 3, S2), F32)
    F2d = [nc.dram_tensor(f"F2d{c}", (S1, 192), F32) for c in range(BC)]
    x1d = nc.dram_tensor("x1d", (BC, C1A, N), F32)
    x2d = nc.dram_tensor("x2d", (BC, C2A, S2 * K), F32)
    nx2d = nc.dram_tensor("nx2d", (BC, S2, 3), F32)
    cc1i = nc.dram_tensor("cc1i", (C1A, 2), F32)
    cc1o = nc.dram_tensor("cc1o", (C1A, 2), F32, addr_space="Shared")
    cc2i = nc.dram_tensor("cc2i", (C2A, 2), F32)
    cc2o = nc.dram_tensor("cc2o", (C2A, 2), F32, addr_space="Shared")
    cc3i = nc.dram_tensor("cc3i", (128, 4), F32)
    cc3o = nc.dram_tensor("cc3o", (128, 4), F32, addr_space="Shared")
    ggi = nc.dram_tensor("ggi", (2, BC, 128), F32)
    ggo = nc.dram_tensor("ggo", (NCORES, 2, BC, 128), F32, addr_space="Shared")
    dbg = {}
    if debug:
        dbg['nx'] = nc.dram_tensor("dbg_nx", (BC, S1, 3), F32, kind="ExternalOutput")
        dbg['idx1'] = nc.dram_tensor("dbg_idx1", (BC, 16, S1), U32, kind="ExternalOutput")
        dbg['l1p'] = nc.dram_tensor("dbg_l1p", (C1B, BC * S1), F32, kind="ExternalOutput")
        dbg['nx2'] = nc.dram_tensor("dbg_nx2", (BC, S2, 3), F32, kind="ExternalOutput")
        dbg['idx2'] = nc.dram_tensor("dbg_idx2", (BC, 16, S2), U32, kind="ExternalOutput")
        dbg['l2pa'] = nc.dram_tensor("dbg_l2pa", (128, BC * S2), F32, kind="ExternalOutput")
        dbg['l2pb'] = nc.dram_tensor("dbg_l2pb", (128, BC * S2), F32, kind="ExternalOutput")
        dbg['ga'] = nc.dram_tensor("dbg_ga", (128, BC), F32, kind="ExternalOutput")
        dbg['gb'] = nc.dram_tensor("dbg_gb", (128, BC), F32, kind="ExternalOutput")
    with tile.TileContext(nc) as tc:
        with nc.allow_non_contiguous_dma(reason="small strided restaging DMAs"), ExitStack() as ctx:
            emit(ctx, tc, d, out_d, F1d, nxd, F2d, nx2d, x1d, x2d,
                 (pre, xyzTre, nxTre, pre2, nxT2re, xyzT2re, gre, nxsd, nxsd2),
                 (cc1i, cc1o), (cc2i, cc2o), (cc3i, cc3o), (ggi, ggo), dbg, no_cc,
                 stop_after)
    nc.compile()
    return nc


def fps_loop(ctx, tc, pool, xs, ys, zs, xyzneg, nx, nsteps, free, name, ones1, psum,
             nxs=None):
    nc = tc.nc
    dist = pool.tile([128, free], F32, name=f"{name}_dist")
    nc.vector.memset(dist[:], 1e10)
    cneg = pool.tile([128, 3], F32, name=f"{name}_cneg")
    m8 = pool.tile([128, 8], F32, name=f"{name}_m8")
    gm = pool.tile([128, 1], F32, name=f"{name}_gm")
    r = pool.tile([128, 3], F32, name=f"{name}_r")
    junk = pool.tile([128, free], F32, name=f"{name}_junk")
    e1 = pool.tile([128, free], F32, name=f"{name}_e1")
    e2 = pool.tile([128, free], F32, name=f"{name}_e2")
    e3 = pool.tile([128, free], F32, name=f"{name}_e3")
    aa = pool.tile([128, free], F32, name=f"{name}_aa")
    ind0 = pool.tile([128, free], F32, name=f"{name}_ind0")
    nc.vector.memset(ind0[:], 0.0)
    nc.vector.memset(ind0[0:1, 0:1], 1.0)
    nc.vector.memset(ind0[64:65, 0:1], 1.0)
    # hi-half (partition 64:128) slices of partition_all_reduce return zeros on
    # HW, so route per-cloud reductions through disjoint COLUMNS of full-128
    # reduces: m2 packs per-cloud maxima, r6 per-cloud coordinate sums.
    m2 = pool.tile([128, 2], F32, name=f"{name}_m2")
    nc.vector.memset(m2[:], -1e30)
    gm2 = pool.tile([128, 2], F32, name=f"{name}_gm2")
    r6 = pool.tile([128, 6], F32, name=f"{name}_r6")
    nc.vector.memset(r6[:], 0.0)
    c6 = pool.tile([128, 6], F32, name=f"{name}_c6")

    def extract_c(mask_src, scal):
        # r[p,d] = sum_f (mask==scal ? -coord); full-128 add -> cneg everywhere
        for dd in range(3):
            nc.vector.scalar_tensor_tensor(
                out=junk[:], in0=mask_src, scalar=scal, in1=xyzneg[:, dd, :],
                op0=OP.is_equal, op1=OP.mult, accum_out=r[:, dd:dd + 1])
        nc.vector.tensor_copy(out=r6[0:64, 0:3], in_=r[0:64, :])
        nc.vector.tensor_copy(out=r6[64:128, 3:6], in_=r[64:128, :])
        nc.gpsimd.partition_all_reduce(c6[:], r6[:], 128, bass_isa.ReduceOp.add)
        nc.vector.tensor_copy(out=cneg[0:64, :], in_=c6[0:64, 0:3])
        nc.vector.tensor_copy(out=cneg[64:128, :], in_=c6[64:128, 3:6])

    def record(i):
        if nxs is not None:
            # all partitions hold their cloud-half's reduced value; negate into
            # the transposed [128, 3, nsteps] buffer
            nc.vector.tensor_scalar_mul(out=nxs[:, :, i], in0=cneg[:],
                                        scalar1=-1.0)
        else:
            nc.scalar.activation(out=nx[0:1, i, :], in_=cneg[0:1, :],
                                 func=AF.Copy, scale=-1.0)
            nc.scalar.activation(out=nx[64:65, i, :], in_=cneg[64:65, :],
                                 func=AF.Copy, scale=-1.0)

    extract_c(ind0[:], 1.0)
    record(0)
    for i in range(1, nsteps):
        nc.scalar.activation(out=e1[:], in_=xyzneg[:, 0, :], func=AF.Square,
                             scale=-1.0, bias=cneg[:, 0:1])
        nc.scalar.activation(out=e2[:], in_=xyzneg[:, 1, :], func=AF.Square,
                             scale=-1.0, bias=cneg[:, 1:2])
        nc.scalar.activation(out=e3[:], in_=xyzneg[:, 2, :], func=AF.Square,
                             scale=-1.0, bias=cneg[:, 2:3])
        nc.vector.tensor_tensor(out=aa[:], in0=e1[:], in1=e2[:], op=OP.add)
        nc.vector.tensor_tensor(out=e1[:], in0=aa[:], in1=e3[:], op=OP.add)
        nc.vector.tensor_tensor(out=dist[:], in0=dist[:], in1=e1[:], op=OP.min)
        nc.vector.max(m8[:], dist[:])
        nc.vector.tensor_copy(out=m2[0:64, 0:1], in_=m8[0:64, 0:1])
        nc.vector.tensor_copy(out=m2[64:128, 1:2], in_=m8[64:128, 0:1])
        nc.gpsimd.partition_all_reduce(gm2[:], m2[:], 128, bass_isa.ReduceOp.max)
        nc.vector.tensor_copy(out=gm[0:64, :], in_=gm2[0:64, 0:1])
        nc.vector.tensor_copy(out=gm[64:128, :], in_=gm2[64:128, 1:2])
        extract_c(dist[:], gm[:, 0:1])
        record(i)


def fps_loop_split(ctx, tc, pool, xyzsrc, nsteps, vw, name, nxs, on_chunk=None,
                   chunk=None, on_step=None):
    """Per-cloud FPS chains: cloud c uses its own [128, vw] tiles spanning all
    128 partitions (point idx = p*vw + f), so reductions are full-128 (the only
    partition_all_reduce form that works on HW). The two chains interleave on
    the engines. xyzsrc(c, dd) -> DRAM AP of cloud c's coord row, (128, vw).
    Records into nxs[128, 3, nsteps] partition-halves (downstream layout
    unchanged: cloud c at partition 64*c)."""
    nc = tc.nc
    mf = max(vw, 8)
    T = {}
    for c in range(2):
        xyzneg = pool.tile([128, 3, vw], F32, name=f"{name}_xyzn{c}")
        for dd in range(3):
            xt = pool.tile([128, vw], F32, name=f"{name}_x{c}{dd}")
            nc.gpsimd.dma_start(xt[:], xyzsrc(c, dd))
            nc.vector.tensor_scalar_mul(out=xyzneg[:, dd, :], in0=xt[:],
                                        scalar1=-1.0)
        dist = pool.tile([128, mf], F32, name=f"{name}_dist{c}")
        nc.vector.memset(dist[:], 1e10)
        if mf > vw:
            nc.vector.memset(dist[:, vw:mf], -1e30)
        # per-step extract history: the add-reduce writes straight into
        # hist[:, :, i]; step i+1's bias reads hist[:, d, i]; one bulk negate
        # after the loop replaces per-step record ops
        hist = pool.tile([128, 3, nsteps], F32, name=f"{name}_hist{c}")
        m8 = pool.tile([128, 8], F32, name=f"{name}_m8{c}")
        gm = pool.tile([128, 1], F32, name=f"{name}_gm{c}")
        r = pool.tile([128, 3], F32, name=f"{name}_r{c}")
        junk = pool.tile([128, vw], F32, name=f"{name}_junk{c}")
        e1 = pool.tile([128, vw], F32, name=f"{name}_e1{c}")
        e2 = pool.tile([128, vw], F32, name=f"{name}_e2{c}")
        e3 = pool.tile([128, vw], F32, name=f"{name}_e3{c}")
        aa = pool.tile([128, vw], F32, name=f"{name}_aa{c}")
        ind0 = pool.tile([128, vw], F32, name=f"{name}_ind0{c}")
        nc.vector.memset(ind0[:], 0.0)
        nc.vector.memset(ind0[0:1, 0:1], 1.0)
        T[c] = (xyzneg, dist, hist, m8, gm, r, junk, e1, e2, e3, aa, ind0)

    def extract_c(c, mask_src, scal, i):
        xyzneg, dist, hist, m8, gm, r, junk = T[c][:7]
        for dd in range(3):
            nc.vector.scalar_tensor_tensor(
                out=junk[:], in0=mask_src, scalar=scal, in1=xyzneg[:, dd, :],
                op0=OP.is_equal, op1=OP.mult, accum_out=r[:, dd:dd + 1])
        nc.gpsimd.partition_all_reduce(hist[:, :, i], r[:], 128,
                                       bass_isa.ReduceOp.add)

    for c in range(2):
        extract_c(c, T[c][11][:], 1.0, 0)
    # phase-interleaved emission: both clouds' reduces are in flight before
    # either cloud's dependent phase queues, so the in-order engine queues
    # overlap the two serial chains.
    for i in range(1, nsteps):
        for c in range(2):
            xyzneg, dist, hist, m8, gm, r, junk, e1, e2, e3, aa, ind0 = T[c]
            nc.scalar.activation(out=e1[:], in_=xyzneg[:, 0, :], func=AF.Square,
                                 scale=-1.0, bias=hist[:, 0:1, i - 1])
            nc.scalar.activation(out=e2[:], in_=xyzneg[:, 1, :], func=AF.Square,
                                 scale=-1.0, bias=hist[:, 1:2, i - 1])
            nc.scalar.activation(out=e3[:], in_=xyzneg[:, 2, :], func=AF.Square,
                                 scale=-1.0, bias=hist[:, 2:3, i - 1])
            nc.vector.tensor_tensor(out=aa[:], in0=e1[:], in1=e2[:], op=OP.add)
            nc.vector.tensor_tensor(out=e1[:], in0=aa[:], in1=e3[:], op=OP.add)
            nc.vector.tensor_tensor(out=dist[:, 0:vw], in0=dist[:, 0:vw],
                                    in1=e1[:], op=OP.min)
            nc.vector.max(m8[:], dist[:])
            nc.gpsimd.partition_all_reduce(gm[:], m8[:, 0:1], 128,
                                           bass_isa.ReduceOp.max)
        if on_step is not None:
            # emitted between the max-reduce issue and the dependent extract,
            # so the drained KNN piece runs during the gpsimd round-trip
            on_step()
        for c in range(2):
            extract_c(c, T[c][1][:, 0:vw], T[c][4][:, 0:1], i)
        if chunk is not None and (i + 1) % chunk == 0:
            # chunk of samples complete: negate its history slice into nxs and
            # hand off (e.g. to emit the KNN tiles that only need these queries)
            j = (i + 1) // chunk - 1
            for c in range(2):
                nc.vector.tensor_scalar_mul(
                    out=nxs[64 * c:64 * (c + 1), :, chunk * j:chunk * (j + 1)],
                    in0=T[c][2][64 * c:64 * (c + 1), :, chunk * j:chunk * (j + 1)],
                    scalar1=-1.0)
            if on_chunk is not None:
                on_chunk(j)
    if chunk is None:
        # bulk negate the per-step history into the shared nxs record buffer
        for c in range(2):
            nc.vector.tensor_scalar_mul(
                out=nxs[64 * c:64 * (c + 1), :, :],
                in0=T[c][2][64 * c:64 * (c + 1), :, :], scalar1=-1.0)


def bn_affine(tc, pool, sums, sqs, g_sb, be_sb, count, cpart, name):
    nc = tc.nc
    mean = pool.tile([cpart, 1], F32, name=f"{name}_mean")
    var = pool.tile([cpart, 1], F32, name=f"{name}_var")
    scale = pool.tile([cpart, 1], F32, name=f"{name}_scale")
    bias = pool.tile([cpart, 1], F32, name=f"{name}_bias")
    tmp = pool.tile([cpart, 1], F32, name=f"{name}_tmp")
    inv_n = 1.0 / float(count)
    nc.scalar.mul(mean[:], sums, inv_n)
    nc.scalar.mul(var[:], sqs, inv_n)
    nc.vector.tensor_tensor(out=tmp[:], in0=mean[:], in1=mean[:], op=OP.mult)
    nc.vector.tensor_tensor(out=var[:], in0=var[:], in1=tmp[:], op=OP.subtract)
    nc.vector.tensor_scalar_add(out=var[:], in0=var[:], scalar1=EPS)
    nc.vector.reciprocal(tmp[:], var[:])
    nc.scalar.activation(out=tmp[:], in_=tmp[:], func=AF.Sqrt)
    nc.vector.tensor_tensor(out=scale[:], in0=tmp[:], in1=g_sb, op=OP.mult)
    nc.vector.tensor_tensor(out=tmp[:], in0=mean[:], in1=scale[:], op=OP.mult)
    nc.vector.tensor_tensor(out=bias[:], in0=be_sb, in1=tmp[:], op=OP.subtract)
    scale_a = pool.tile([cpart, 1], F32, name=f"{name}_scale_a")
    bias_a = pool.tile([cpart, 1], F32, name=f"{name}_bias_a")
    nc.scalar.activation(out=scale_a[:], in_=scale[:], func=AF.Copy)
    nc.scalar.activation(out=bias_a[:], in_=bias[:], func=AF.Copy)
    return scale_a, bias_a


def emit(ctx, tc, d, out_d, F1d, nxd, F2d, nx2d, x1d, x2d, stg, cc1, cc2, cc3, gg, dbg,
         no_cc=False, stop_after=None):
    pre, xyzTre, nxTre, pre2, nxT2re, xyzT2re, gre, nxsd, nxsd2 = stg
    nc = tc.nc

    def bail():
        zout = sing.tile([16, 1], F32, name="zout")
        nc.vector.memset(zout[:], 0.0)
        nc.gpsimd.dma_start(out_d[:], zout[:])
    P = 128
    RG = [list(range(NCORES))]
    sing = ctx.enter_context(tc.tile_pool(name="sing", bufs=1))
    big = ctx.enter_context(tc.tile_pool(name="big", bufs=1))
    work = ctx.enter_context(tc.tile_pool(name="work", bufs=1))
    psum = ctx.enter_context(tc.tile_pool(name="psum", bufs=3, space="PSUM"))
    psumT = ctx.enter_context(tc.tile_pool(name="psumT", bufs=3, space="PSUM"))
    bpool = ctx.enter_context(tc.tile_pool(name="bnp", bufs=1))


    ones1 = sing.tile([1, 128], F32, name="ones1")
    nc.vector.memset(ones1[:], 1.0)
    wsb = {}
    for name, shp in WEIGHT_SHAPES.items():
        if len(shp) == 1:
            if shp[0] > 128:
                for hh in range(shp[0] // 128):
                    t = sing.tile([128, 1], F32, name=f"w_{name}_{hh}")
                    nc.gpsimd.dma_start(t[:], d[name][128 * hh:128 * (hh + 1), None])
                    wsb[f"{name}_{hh}"] = t
                continue
            t = sing.tile([shp[0], 1], F32, name=f"w_{name}")
            nc.gpsimd.dma_start(t[:], d[name][:, None])
        else:
            t = sing.tile(list(shp), F32, name=f"w_{name}")
            nc.gpsimd.dma_start(t[:], d[name][:])
        wsb[name] = t

    # ---- points load (restage so each SBUF tile = ONE DMA) ----
    for dd in range(3):
        for c in range(BC):
            nc.gpsimd.dma_start(
                pre[dd, 64 * c:64 * (c + 1), :],
                d['points'][c, :, dd].rearrange("(p f) -> p f", p=64))
    xs = sing.tile([P, 128], F32, name="xs")
    ys = sing.tile([P, 128], F32, name="ys")
    zs = sing.tile([P, 128], F32, name="zs")
    for dd, t in enumerate((xs, ys, zs)):
        nc.gpsimd.dma_start(t[:], pre[dd])
    xyzneg = sing.tile([P, 3, 128], F32, name="xyzneg")
    for dd, t in enumerate((xs, ys, zs)):
        nc.vector.tensor_scalar_mul(out=xyzneg[:, dd, :], in0=t[:], scalar1=-1.0)
    sqt0 = work.tile([P, 128], F32, name="sqt0", tag="sqt0")
    rnf = sing.tile([P, 128], F32, name="rnf")
    nc.scalar.activation(out=rnf[:], in_=xyzneg[:, 0, :], func=AF.Square)
    nc.scalar.activation(out=sqt0[:], in_=xyzneg[:, 1, :], func=AF.Square)
    nc.vector.tensor_tensor(out=rnf[:], in0=rnf[:], in1=sqt0[:], op=OP.add)
    nc.scalar.activation(out=sqt0[:], in_=xyzneg[:, 2, :], func=AF.Square)
    nc.vector.tensor_tensor(out=rnf[:], in0=rnf[:], in1=sqt0[:], op=OP.add)
    # xyzT staging: rows xyz from points, row3 = rn (per cloud), all in DRAM
    for c in range(BC):
        for dd, t in enumerate((xs, ys, zs)):
            nc.gpsimd.dma_start(xyzTre[c][dd:dd + 1, :], t[64 * c:64 * (c + 1), :])
        nc.gpsimd.dma_start(xyzTre[c][3:4, :], rnf[64 * c:64 * (c + 1), :])
    xyzTt = sing.tile([4, N], F32, name="xyzTt")

    def fill_xyzT(c):
        nc.gpsimd.dma_start(xyzTt[:], xyzTre[c][:])

    # ---- F1 rows-major -> F1d ----
    for c in range(BC):
        fill_xyzT(c)
        for j in range(8):
            ps = psum.tile([P, 512], F32, name="f1ps", tag="mm")
            st = work.tile([P, 512], F32, name="f1st", tag="f1st")
            for jj in range(8):
                ch = 8 * j + jj
                nc.tensor.matmul(ps[:, 64 * jj:64 * (jj + 1)],
                                 xyzTt[0:3, 128 * ch:128 * (ch + 1)],
                                 wsb['w1s_T'][:])
            nc.scalar.activation(out=st[:], in_=ps[:], func=AF.Copy)
            nc.gpsimd.dma_start(
                F1d[c][:].rearrange("(j p) q -> p j q", p=128)[:, 8 * j:8 * (j + 1), :],
                st[:].rearrange("p (j q) -> p j q", j=8))

    if stop_after == 1:
        bail()
        return

    # ---- FPS1 with KNN1 tiles emitted per 128-sample chunk so the KNN
    # matmul/gather/scan work fills FPS1's idle engine time ----
    nxs = sing.tile([P, 3, S1], F32, name="nxs")
    nxT = sing.tile([3, BC * S1], F32, name="nxT")
    q4T = sing.tile([4, BC * S1], F32, name="q4T")
    nc.gpsimd.dma_start(q4T[3:4, :], d['constrow'][0:1, :])
    Gc = sing.tile([C1A, BC * S1], F32, name="Gc")
    scores = big.tile([P, N], F32, name="scores")
    sum1 = sing.tile([C1A, 128], F32, name="sum1")
    sq1 = sing.tile([C1A, 128], F32, name="sq1")
    l1pT = big.tile([C1B, BC * S1], F32, name="l1pT")
    fpool = ctx.enter_context(tc.tile_pool(name="fps1", bufs=1))

    def stage_chunk1(j):
        # queries 128j..128(j+1) of each cloud are final: stage nxsd/nxT/q4T/Gc
        # for them, then emit their two KNN tiles (t=j cloud 0, t=4+j cloud 1)
        nc.gpsimd.dma_start(nxsd[:, :, 128 * j:128 * (j + 1)],
                            nxs[:, :, 128 * j:128 * (j + 1)])
        for c in range(BC):
            q0 = S1 * c + 128 * j
            nc.gpsimd.dma_start(nxT[:, q0:q0 + 128],
                                nxsd[64 * c][:, 128 * j:128 * (j + 1)])
            nc.vector.tensor_scalar_mul(out=q4T[0:3, q0:q0 + 128],
                                        in0=nxT[:, q0:q0 + 128], scalar1=2.0)
            psg = psum.tile([C1A, 128], F32, name="gcps", tag="mm")
            nc.tensor.matmul(psg[:], wsb['w1x_T'][:], nxT[:, q0:q0 + 128])
            nc.scalar.activation(out=Gc[:, q0:q0 + 128], in_=psg[:], func=AF.Copy)
        knn_q.append(emit_knn1_tile(j))
        knn_q.append(emit_knn1_tile(4 + j))

    knn_q = []

    def drain_knn(n=1):
        # advance the pending KNN generators by n pieces (called once per FPS
        # step so KNN work lands in FPS1's per-step latency bubbles)
        for _ in range(n):
            while knn_q:
                try:
                    next(knn_q[0])
                    return
                except StopIteration:
                    knn_q.pop(0)
            return

    def emit_knn1_tile(t):
        c = t // 4
        fill_xyzT(c)
        for jj in range(16):
            ps = psum.tile([P, 512], F32, name="knnps", tag="mm")
            nc.tensor.matmul(ps[:], q4T[:, 128 * t:128 * (t + 1)],
                             xyzTt[:, 512 * jj:512 * (jj + 1)])
            nc.scalar.activation(out=scores[:, 512 * jj:512 * (jj + 1)], in_=ps[:],
                                  func=AF.Copy)
            yield
        m8a = work.tile([P, 8], F32, name="m8a", tag="m8a")
        m8b = work.tile([P, 8], F32, name="m8b", tag="m8b")
        ia = work.tile([P, 16], U32, name="iab", tag="iab")
        nc.vector.max(m8a[:], scores[:])
        yield
        nc.vector.max_index(ia[:, 0:8], m8a[:], scores[:])
        yield
        nc.vector.match_replace(scores[:], m8a[:], scores[:], -1e30)
        yield
        nc.vector.max(m8b[:], scores[:])
        yield
        nc.vector.max_index(ia[:, 8:16], m8b[:], scores[:])
        yield
        if dbg:
            iaf = work.tile([P, 16], F32, name="iaf", tag="iaf")
            nc.vector.tensor_copy(out=iaf[:], in_=ia[:])
            pst = psumT.tile([16, P], F32, name="idxps", tag="T")
            nc.tensor.transpose(pst[:], iaf[:], wsb['ident'][:])
            dcp = work.tile([16, P], U32, name="dcp", tag="dcp")
            nc.vector.tensor_copy(out=dcp[:], in_=pst[:])
            nc.gpsimd.dma_start(dbg['idx1'][c, :, 128 * (t % 4):128 * (t % 4 + 1)],
                              dcp[:])
        # gather + conv1-space blocks, k-major columns: col = 512*k + 128*(t%4) + q
        for k in range(K):
            gblk = work.tile([P, C1A], F32, name="gblk", tag="gblk")
            nc.gpsimd.indirect_dma_start(
                out=gblk[:], out_offset=None, in_=F1d[c][:],
                in_offset=bass.IndirectOffsetOnAxis(ap=ia[:, k:k + 1], axis=0))
            psx1 = psumT.tile([C1A, P], F32, name="psx1", tag="T")
            nc.tensor.transpose(psx1[:], gblk[:], wsb['ident'][:])
            q0 = S1 * c + 128 * (t % 4)
            xblk = work.tile([C1A, P], F32, name="xblk", tag="xblk")
            nc.vector.scalar_tensor_tensor(
                out=xblk[:], in0=psx1[:], scalar=0.0,
                in1=Gc[:, q0:q0 + 128],
                op0=OP.bypass, op1=OP.subtract,
                accum_out=sum1[:, 64 * c + 16 * (t % 4) + k:64 * c + 16 * (t % 4) + k + 1])
            sqt = work.tile([C1A, P], F32, name="sqt", tag="sqt")
            nc.scalar.activation(
                out=sqt[:], in_=xblk[:], func=AF.Square,
                accum_out=sq1[:, 64 * c + 16 * (t % 4) + k:64 * c + 16 * (t % 4) + k + 1])
            nc.gpsimd.dma_start(
                x1d[c, :, 512 * k + 128 * (t % 4):512 * k + 128 * (t % 4) + 128],
                xblk[:])
            yield

    if stop_after == 22:
        nc.vector.memset(nxs[:], 0.25)
        bail()
        return
    fps_loop_split(ctx, tc, fpool,
                   lambda c, dd: xyzTre[c][dd, :].rearrange("(p f) -> p f", p=128),
                   S1, 64, "f1", nxs, on_chunk=stage_chunk1, chunk=128,
                   on_step=drain_knn)
    while knn_q:
        drain_knn()
    if dbg:
        for c in range(BC):
            nc.gpsimd.dma_start(dbg['nx'][c],
                                nxsd[64 * c].rearrange("dd q -> q dd"))
    if stop_after in (2, 21):
        bail()
        return

    red1 = sing.tile([C1A, 2], F32, name="red1")
    nc.vector.tensor_reduce(out=red1[:, 0:1], in_=sum1[:, None, :], axis=AX.X, op=OP.add)
    nc.vector.tensor_reduce(out=red1[:, 1:2], in_=sq1[:, None, :], axis=AX.X, op=OP.add)
    nc.gpsimd.dma_start(cc1[0][:], red1[:])
    if stop_after == 3:
        bail()
        return
    if not no_cc:
        nc.gpsimd.collective_compute("AllReduce", OP.add, replica_groups=RG,
                                     ins=[cc1[0][:]], outs=[cc1[1][:]])
    stat1 = sing.tile([C1A, 2], F32, name="stat1")
    nc.gpsimd.dma_start(stat1[:], cc1[0 if no_cc else 1][:])
    sc1, bi1 = bn_affine(tc, bpool, stat1[:, 0:1], stat1[:, 1:2],
                         wsb['bn1_g'][:], wsb['bn1_be'][:], B * S1 * K, C1A, "bn1")

    for c in range(BC):
        for k in range(K):
            col = 512 * k
            x1c = work.tile([C1A, 512], F32, name="x1c", tag="x1c")
            nc.gpsimd.dma_start(x1c[:], x1d[c, :, col:col + 512])
            x1v = work.tile([C1A, 512], F32, name="x1v", tag="x1v")
            nc.scalar.activation(out=x1v[:], in_=x1c[:], func=AF.Copy)
            h1 = work.tile([C1A, 512], F32, name="h1", tag="h1")
            nc.scalar.activation(out=h1[:], in_=x1v[:], func=AF.Relu,
                                 scale=sc1[:], bias=bi1[:])
            ps = psum.tile([C1B, 512], F32, name="c2ps", tag="mm")
            nc.tensor.matmul(ps[:], wsb['w2_T'][:], h1[:])
            sl = l1pT[:, S1 * c:S1 * (c + 1)]
            if k == 0:
                nc.vector.tensor_copy(out=sl, in_=ps[:])
            else:
                nc.vector.tensor_tensor(out=sl, in0=sl, in1=ps[:], op=OP.max)
    nc.vector.tensor_scalar(out=l1pT[:], in0=l1pT[:], scalar1=wsb['b1c2'][:],
                            scalar2=None, op0=OP.add)
    if dbg:
        nc.gpsimd.dma_start(dbg['l1p'][:], l1pT[:])
    if stop_after == 4:
        bail()
        return

    # ---- SA2 prep ----
    zpad = sing.tile([128, 60], F32, name="zpad")
    nc.vector.memset(zpad[:], 0.0)
    for c in range(BC):
        nc.gpsimd.dma_start(F2d[c][:, 0:3],
                            nxsd[64 * c].rearrange("dd q -> q dd"))
        for j in range(4):
            nc.gpsimd.dma_start(F2d[c][128 * j:128 * (j + 1), 3], zpad[:, 0:1])
            nc.gpsimd.dma_start(F2d[c][128 * j:128 * (j + 1), 132:192], zpad[:])
        for j in range(4):
            pst = psumT.tile([P, P], F32, name="ftps", tag="T")
            nc.tensor.transpose(pst[:], l1pT[:, S1 * c + 128 * j:S1 * c + 128 * (j + 1)],
                                wsb['ident'][:])
            stg = work.tile([P, P], F32, name="fstg", tag="fstg")
            nc.vector.tensor_copy(out=stg[:], in_=pst[:])
            nc.gpsimd.dma_start(F2d[c][128 * j:128 * (j + 1), 4:132], stg[:])

    nxs2 = sing.tile([P, 3, S2], F32, name="nxs2")
    fpool2 = ctx.enter_context(tc.tile_pool(name="fps2", bufs=1))
    fps_loop_split(ctx, tc, fpool2,
                   lambda c, dd: nxsd[64 * c, dd, :].rearrange(
                       "(p f) -> p f", p=128),
                   S2, 4, "f2", nxs2)
    nc.gpsimd.dma_start(nxsd2[:], nxs2[:])
    if dbg:
        for c in range(BC):
            nc.gpsimd.dma_start(dbg['nx2'][c],
                                nxsd2[64 * c].rearrange("dd q -> q dd"))

    nxT2f = sing.tile([4, BC * S2], F32, name="nxT2f")
    for c in range(BC):
        nc.gpsimd.dma_start(nxT2f[0:3, S2 * c:S2 * (c + 1)], nxsd2[64 * c])
    nc.gpsimd.dma_start(nxT2f[3:4, :], d['constrow'][1:2, 0:BC * S2])
    nxT2 = nxT2f
    q4T2 = sing.tile([3, BC * S2], F32, name="q4T2")
    nc.vector.tensor_scalar_mul(out=q4T2[:], in0=nxT2[0:3, :], scalar1=2.0)
    monerow = sing.tile([1, 128], F32, name="monerow")
    nc.gpsimd.dma_start(monerow[:], d['constrow'][0:1, 0:128])
    xyzT2 = [sing.tile([3, S1], F32, name=f"xyzT2_{c}") for c in range(BC)]
    rn2ts = [sing.tile([1, S1], F32, name=f"rn2t_{c}") for c in range(BC)]
    ones3 = sing.tile([3, 1], F32, name="ones3")
    nc.vector.memset(ones3[:], 1.0)
    for c in range(BC):
        nc.gpsimd.dma_start(xyzT2[c][:], nxsd[64 * c])
        sq2t = work.tile([3, S1], F32, name="sq2t", tag="sq2t")
        nc.scalar.activation(out=sq2t[:], in_=xyzT2[c][:], func=AF.Square)
        psr = psum.tile([1, S1], F32, name="rnps", tag="mm")
        nc.tensor.matmul(psr[:], ones3[:], sq2t[:])
        nc.vector.tensor_copy(out=rn2ts[c][:], in_=psr[:])

    if stop_after == 5:
        bail()
        return

    # ---- KNN2 + gather + MLP2 ----
    sum2 = sing.tile([C2A, 8], F32, name="sum2")
    sq2 = sing.tile([C2A, 8], F32, name="sq2")
    l2paT = big.tile([128, BC * S2], F32, name="l2paT")
    x2sb = big.tile([C2A, BC * S2 * K], F32, name="x2sb")
    l2pbT = big.tile([128, BC * S2], F32, name="l2pbT")

    for c in range(BC):
        ps = psum.tile([P, S1], F32, name="kn2ps", tag="mm")
        nc.tensor.matmul(ps[:], q4T2[:, S2 * c:S2 * (c + 1)], xyzT2[c][:],
                         start=True, stop=False)
        nc.tensor.matmul(ps[:], monerow[:], rn2ts[c][:], start=False, stop=True)
        sc2t = work.tile([P, S1], F32, name="sc2t", tag="sc2t")
        nc.scalar.activation(out=sc2t[:], in_=ps[:], func=AF.Copy)
        m8a = work.tile([P, 8], F32, name="m8a2", tag="m8a2")
        m8b = work.tile([P, 8], F32, name="m8b2", tag="m8b2")
        ia = work.tile([P, 16], U32, name="iab2", tag="iab2")
        nc.vector.max(m8a[:], sc2t[:])
        nc.vector.max_index(ia[:, 0:8], m8a[:], sc2t[:])
        nc.vector.match_replace(sc2t[:], m8a[:], sc2t[:], -1e30)
        nc.vector.max(m8b[:], sc2t[:])
        nc.vector.max_index(ia[:, 8:16], m8b[:], sc2t[:])
        if dbg:
            iaf2 = work.tile([P, 16], F32, name="iaf2", tag="iaf2")
            nc.vector.tensor_copy(out=iaf2[:], in_=ia[:])
            pst2 = psumT.tile([16, P], F32, name="idx2ps", tag="T")
            nc.tensor.transpose(pst2[:], iaf2[:], wsb['ident'][:])
            dcp2 = work.tile([16, P], U32, name="dcp2", tag="dcp2")
            nc.vector.tensor_copy(out=dcp2[:], in_=pst2[:])
            nc.gpsimd.dma_start(dbg['idx2'][c], dcp2[:])
        rhx = big.tile([4, S2 * K], F32, name="rhx", tag="rhx")
        rhp = big.tile([C2A, S2 * K], F32, name="rhp", tag="rhp")
        for k in range(K):
            gblk2 = work.tile([P, 192], F32, name="gblk2", tag="gblk2")
            nc.gpsimd.indirect_dma_start(
                out=gblk2[:], out_offset=None, in_=F2d[c][:],
                in_offset=bass.IndirectOffsetOnAxis(ap=ia[:, k:k + 1], axis=0))
            psx = psumT.tile([4, P], F32, name="psx", tag="T")
            nc.tensor.transpose(psx[:], gblk2[:, 0:4], wsb['ident'][:])
            nc.vector.tensor_copy(out=rhx[:, 128 * k:128 * (k + 1)], in_=psx[:])
            psp = psumT.tile([C2A, P], F32, name="psp", tag="T")
            nc.tensor.transpose(psp[:], gblk2[:, 4:132], wsb['ident'][:])
            nc.vector.tensor_copy(out=rhp[:, 128 * k:128 * (k + 1)], in_=psp[:])
        for chk in range(4):
            col = 512 * chk
            ps2 = psum.tile([C2A, 512], F32, name="c1ps2", tag="mm")
            nc.tensor.matmul(ps2[:], wsb['A2x_T'][:], rhx[:, col:col + 512],
                             start=True, stop=False)
            nc.tensor.matmul(ps2[:], wsb['A2p_T'][:], rhp[:, col:col + 512],
                             start=False, stop=False)
            nc.tensor.matmul(
                ps2[:], wsb['A2xn_T'][:],
                nxT2[:, S2 * c:S2 * (c + 1)][:, None, :].broadcast_to((4, 4, S2)),
                start=False, stop=True)
            x2col = S2 * K * c + col
            nc.scalar.activation(out=x2sb[:, x2col:x2col + 512], in_=ps2[:],
                                 func=AF.Copy,
                                 accum_out=sum2[:, 4 * c + chk:4 * c + chk + 1])
            sqt2 = work.tile([C2A, 512], F32, name="sqt2", tag="sqt2")
            nc.scalar.activation(out=sqt2[:], in_=x2sb[:, x2col:x2col + 512],
                                 func=AF.Square,
                                 accum_out=sq2[:, 4 * c + chk:4 * c + chk + 1])

    red2 = sing.tile([C2A, 2], F32, name="red2")
    nc.vector.tensor_reduce(out=red2[:, 0:1], in_=sum2[:, None, :], axis=AX.X, op=OP.add)
    nc.vector.tensor_reduce(out=red2[:, 1:2], in_=sq2[:, None, :], axis=AX.X, op=OP.add)
    nc.gpsimd.dma_start(cc2[0][:], red2[:])
    if not no_cc:
        nc.gpsimd.collective_compute("AllReduce", OP.add, replica_groups=RG,
                                     ins=[cc2[0][:]], outs=[cc2[1][:]])
    stat2 = sing.tile([C2A, 2], F32, name="stat2")
    nc.gpsimd.dma_start(stat2[:], cc2[0 if no_cc else 1][:])
    sc2, bi2 = bn_affine(tc, bpool, stat2[:, 0:1], stat2[:, 1:2],
                         wsb['bn2_g'][:], wsb['bn2_be'][:], B * S2 * K, C2A, "bn2")

    for c in range(BC):
        for chk in range(4):
            col = S2 * K * c + 512 * chk
            h2 = work.tile([C2A, 512], F32, name="h2", tag="h2")
            nc.scalar.activation(out=h2[:], in_=x2sb[:, col:col + 512],
                                 func=AF.Relu, scale=sc2[:], bias=bi2[:])
            psa = psum.tile([128, 512], F32, name="c2psa", tag="mm")
            nc.tensor.matmul(psa[:], wsb['B2a_T'][:], h2[:])
            psb = psum.tile([128, 512], F32, name="c2psb", tag="mm")
            nc.tensor.matmul(psb[:], wsb['B2b_T'][:], h2[:])
            for half, (pp, ll) in enumerate(((psa, l2paT), (psb, l2pbT))):
                sl = ll[:, S2 * c:S2 * (c + 1)]
                for kk in range(4):
                    yk = pp[:, 128 * kk:128 * (kk + 1)]
                    if chk == 0 and kk == 0:
                        nc.vector.tensor_copy(out=sl, in_=yk)
                    else:
                        nc.vector.tensor_tensor(out=sl, in0=sl, in1=yk, op=OP.max)
    nc.vector.tensor_scalar(out=l2paT[:], in0=l2paT[:], scalar1=wsb['b2c2_0'][:],
                            scalar2=None, op0=OP.add)
    nc.vector.tensor_scalar(out=l2pbT[:], in0=l2pbT[:], scalar1=wsb['b2c2_1'][:],
                            scalar2=None, op0=OP.add)
    if dbg:
        nc.gpsimd.dma_start(dbg['l2pa'][:], l2paT[:])
        nc.gpsimd.dma_start(dbg['l2pb'][:], l2pbT[:])
    if stop_after == 6:
        bail()
        return

    # ---- SA3 ----
    NR3 = BC * S2
    x3a = big.tile([128, NR3], F32, name="x3a")
    x3b = big.tile([128, NR3], F32, name="x3b")
    s3 = sing.tile([128, 4], F32, name="s3")
    for half, (x3, xw, paw, pbw) in enumerate(
            ((x3a, 'A3x_Ta', 'A3pa_Ta', 'A3pb_Ta'),
             (x3b, 'A3x_Tb', 'A3pa_Tb', 'A3pb_Tb'))):
        ps3 = psum.tile([128, NR3], F32, name="ps3", tag="mm")
        nc.tensor.matmul(ps3[:], wsb[xw][:], nxT2[:], start=True, stop=False)
        nc.tensor.matmul(ps3[:], wsb[paw][:], l2paT[:], start=False, stop=False)
        nc.tensor.matmul(ps3[:], wsb[pbw][:], l2pbT[:], start=False, stop=True)
        nc.scalar.activation(out=x3[:], in_=ps3[:], func=AF.Copy,
                             accum_out=s3[:, 2 * half:2 * half + 1])
        sqt3 = work.tile([128, NR3], F32, name="sqt3", tag="sqt3")
        nc.scalar.activation(out=sqt3[:], in_=x3[:], func=AF.Square,
                             accum_out=s3[:, 2 * half + 1:2 * half + 2])
    nc.gpsimd.dma_start(cc3[0][:], s3[:])
    if not no_cc:
        nc.gpsimd.collective_compute("AllReduce", OP.add, replica_groups=RG,
                                     ins=[cc3[0][:]], outs=[cc3[1][:]])
    stat3 = sing.tile([128, 4], F32, name="stat3")
    nc.gpsimd.dma_start(stat3[:], cc3[0 if no_cc else 1][:])
    n3 = B * S2
    sc3a, bi3a = bn_affine(tc, bpool, stat3[:, 0:1], stat3[:, 1:2],
                           wsb['bn3_g_0'][:], wsb['bn3_be_0'][:], n3, 128, "bn3a")
    sc3b, bi3b = bn_affine(tc, bpool, stat3[:, 2:3], stat3[:, 3:4],
                           wsb['bn3_g_1'][:], wsb['bn3_be_1'][:], n3, 128, "bn3b")
    h3a = work.tile([128, NR3], F32, name="h3a")
    h3b = work.tile([128, NR3], F32, name="h3b")
    nc.scalar.activation(out=h3a[:], in_=x3a[:], func=AF.Relu, scale=sc3a[:], bias=bi3a[:])
    nc.scalar.activation(out=h3b[:], in_=x3b[:], func=AF.Relu, scale=sc3b[:], bias=bi3b[:])
    ga = sing.tile([128, BC], F32, name="ga")
    gb = sing.tile([128, BC], F32, name="gb")
    for half, g in ((0, ga), (1, gb)):
        psg3 = psum.tile([128, NR3], F32, name="psg3", tag="mm")
        nc.tensor.matmul(psg3[:], wsb[f'C3_{half}0'][:], h3a[:], start=True, stop=False)
        nc.tensor.matmul(psg3[:], wsb[f'C3_{half}1'][:], h3b[:], start=False, stop=True)
        nc.vector.tensor_reduce(out=g[:], in_=psg3[:].rearrange("p (c q) -> p c q", c=BC),
                                axis=AX.X, op=OP.max)
        nc.vector.tensor_scalar(out=g[:], in0=g[:],
                                scalar1=wsb[f'b3c2_{half}'][:],
                                scalar2=None, op0=OP.add)
    if dbg:
        nc.gpsimd.dma_start(dbg['ga'][:], ga[:])
        nc.gpsimd.dma_start(dbg['gb'][:], gb[:])
    if stop_after == 7:
        bail()
        return

    # ---- AllGather + FC head ----
    nc.gpsimd.dma_start(gg[0][0].rearrange("c p -> p c"), ga[:])
    nc.gpsimd.dma_start(gg[0][1].rearrange("c p -> p c"), gb[:])
    if not no_cc:
        nc.gpsimd.collective_compute("AllGather", OP.bypass, replica_groups=RG,
                                     ins=[gg[0][:]], outs=[gg[1][:]])
    for n in range(NCORES):
        ggsrc = gg[0] if no_cc else gg[1][n]
        nc.gpsimd.dma_start(gre[0, :, BC * n:BC * (n + 1)],
                            ggsrc[0].rearrange("c p -> p c"))
        nc.gpsimd.dma_start(gre[1, :, BC * n:BC * (n + 1)],
                            ggsrc[1].rearrange("c p -> p c"))
    gaal = sing.tile([128, B], F32, name="gaal")
    gbal = sing.tile([128, B], F32, name="gbal")
    nc.gpsimd.dma_start(gaal[:], gre[0])
    nc.gpsimd.dma_start(gbal[:], gre[1])

    def fc_layer(xins, wnames, gslice, beslice, name, alpha=0.2):
        ps = psum.tile([128, B], F32, name=f"{name}ps", tag="mm")
        for i, (xt, wn) in enumerate(zip(xins, wnames)):
            nc.tensor.matmul(ps[:], wsb[wn][:], xt[:], start=(i == 0),
                             stop=(i == len(xins) - 1))
        xsb = work.tile([128, B], F32, name=f"{name}x", tag=f"{name}x")
        ssq = sing.tile([128, 2], F32, name=f"{name}ssq")
        nc.scalar.activation(out=xsb[:], in_=ps[:], func=AF.Copy,
                             accum_out=ssq[:, 0:1])
        sqf = work.tile([128, B], F32, name=f"{name}sq", tag=f"{name}sq")
        nc.scalar.activation(out=sqf[:], in_=xsb[:], func=AF.Square,
                             accum_out=ssq[:, 1:2])
        sc, bi = bn_affine(tc, bpool, ssq[:, 0:1], ssq[:, 1:2], gslice, beslice,
                           B, 128, name)
        act = work.tile([128, B], F32, name=f"{name}act", tag=f"{name}act")
        vv = work.tile([128, B], F32, name=f"{name}vv", tag=f"{name}vv")
        nc.scalar.activation(out=vv[:], in_=xsb[:], func=AF.Identity,
                             scale=sc[:], bias=bi[:])
        av = work.tile([128, B], F32, name=f"{name}av", tag=f"{name}av")
        nc.vector.tensor_scalar_mul(out=av[:], in0=vv[:], scalar1=alpha)
        nc.vector.tensor_tensor(out=act[:], in0=vv[:], in1=av[:], op=OP.max)
        return act

    h1a = fc_layer([gaal, gbal], ['FC1_00', 'FC1_01'],
                   wsb['fbn1_g_0'][:], wsb['fbn1_be_0'][:], "fc1a")
    h1b = fc_layer([gaal, gbal], ['FC1_10', 'FC1_11'],
                   wsb['fbn1_g_1'][:], wsb['fbn1_be_1'][:], "fc1b")
    h2f = fc_layer([h1a, h1b], ['FC2_0', 'FC2_1'],
                   wsb['fbn2_g'][:], wsb['fbn2_be'][:], "fc2")
    ps_o = psum.tile([1, B], F32, name="ps_o", tag="mm")
    nc.tensor.matmul(ps_o[:], wsb['FC3_T'][:], h2f[:])
    o_sb = sing.tile([1, B], F32, name="o_sb")
    nc.vector.tensor_scalar(out=o_sb[:], in0=ps_o[:], scalar1=wsb['fc3_b'][:],
                            scalar2=None, op0=OP.add)
    nc.gpsimd.dma_start(out_d[:, 0][None, :], o_sb[:])


# ===================== host-side entry point =====================
_NC_CACHE = {}


def _get_nc():
    if 'nc' not in _NC_CACHE:
        _NC_CACHE['nc'] = build_nc(debug=False)
    return _NC_CACHE['nc']


def _kernel_numpy(inputs):
    """Exact numpy fallback of the reference model (host-side)."""
    f = np.float32
    pts = np.asarray(inputs['points'], f)
    Bn, Nn = pts.shape[0], pts.shape[1]

    def fps(x, npoint):
        n = x.shape[0]
        xs_, ys_, zs_ = x[:, 0], x[:, 1], x[:, 2]
        dist = np.full(n, 1e10, f)
        idxs = np.zeros(npoint, np.int64)
        far = 0
        for i in range(npoint):
            idxs[i] = far
            c = x[far]
            e = ((xs_ - c[0]) ** 2).astype(f) + ((ys_ - c[1]) ** 2).astype(f)
            dist = np.minimum(dist, (e + ((zs_ - c[2]) ** 2).astype(f)).astype(f))
            far = int(np.argmax(dist))
        return idxs

    def knn(q, r, k):
        d = (np.sum(q ** 2, -1)[:, None] - 2.0 * (q @ r.T) + np.sum(r ** 2, -1)[None, :])
        return np.argsort(d, axis=1, kind='stable')[:, :k]

    def bn(x, g, b, axes):
        m = x.mean(axes, keepdims=True, dtype=np.float64).astype(f)
        v = x.var(axes, keepdims=True).astype(f)
        return (x - m) / np.sqrt(v + 1e-5) * g + b

    def mlp2(x, w1, b1, g1, be1, w2, b2, axes):
        h = x @ np.asarray(w1, f).T + b1
        h = np.maximum(bn(h, g1, be1, axes), 0)
        return h @ np.asarray(w2, f).T + b2

    def sa_knn(xyz, ptsf, npoint, k, w1, b1, g1, be1, w2, b2):
        nx_l, np_l, gx_l, gp_l = [], [], [], []
        for b_ in range(xyz.shape[0]):
            fi = fps(xyz[b_], npoint)
            nxb = xyz[b_][fi]
            idx = knn(nxb, xyz[b_], k)
            gx_l.append(xyz[b_][idx] - nxb[:, None, :])
            gp_l.append(ptsf[b_][idx])
            nx_l.append(nxb)
        nxa = np.stack(nx_l); gx = np.stack(gx_l); gp = np.stack(gp_l)
        grouped = np.concatenate([gx, gp], -1)
        out = mlp2(grouped, w1, b1, g1, be1, w2, b2, (0, 1, 2))
        return nxa, out.max(2)

    i = {k: np.asarray(v, f) for k, v in inputs.items()}
    l1x, l1p = sa_knn(pts, pts, 512, 16, i['sa1_c1_w'], i['sa1_c1_b'],
                      i['sa1_bn_g'], i['sa1_bn_be'], i['sa1_c2_w'], i['sa1_c2_b'])
    l2x, l2p = sa_knn(l1x, l1p, 128, 16, i['sa2_c1_w'], i['sa2_c1_b'],
                      i['sa2_bn_g'], i['sa2_bn_be'], i['sa2_c2_w'], i['sa2_c2_b'])
    grouped = np.concatenate([l2x, l2p], -1)[:, None]
    g = mlp2(grouped, i['sa3_c1_w'], i['sa3_c1_b'], i['sa3_bn_g'], i['sa3_bn_be'],
             i['sa3_c2_w'], i['sa3_c2_b'], (0, 1, 2)).max(2)[:, 0]

    def lrelu(x):
        return np.where(x > 0, x, 0.2 * x)
    h = g @ i['fc1_w'].T + i['fc1_b']
    h = lrelu(bn(h, i['fc1_bn_g'], i['fc1_bn_be'], (0,)))
    h = h @ i['fc2_w'].T + i['fc2_b']
    h = lrelu(bn(h, i['fc2_bn_g'], i['fc2_bn_be'], (0,)))
    return (h @ i['fc3_w'].T + i['fc3_b']).astype(f)


def kernel(**inputs):
    """Full-input entry: shard over 8 NeuronCores, run, return (16,1) logits."""
    try:
        from concourse.bass_utils import run_bass_kernel_spmd
        w = prep_common_weights(inputs)
        pts = np.asarray(inputs['points'], np.float32)
        in_maps = []
        for t in range(NCORES):
            m = {'points': np.ascontiguousarray(pts[BC * t:BC * (t + 1)])}
            for name, shp in WEIGHT_SHAPES.items():
                m[name] = np.ascontiguousarray(w[name].reshape(shp))
            in_maps.append(m)
        nc = _get_nc()
        res = run_bass_kernel_spmd(nc, in_maps, list(range(NCORES)))
        out = np.asarray(res.results[0]['out'], np.float32)
        return out
    except Exception:
        import traceback
        traceback.print_exc()
        return _kernel_numpy(inputs)



# revision 54
# speedup vs baseline: 1.0304x; 1.0126x over previous
"""PointCloudDiscriminator Trainium2 Bass kernel (SPMD 8 cores, 2 clouds/core)."""
import numpy as np
from contextlib import ExitStack

import concourse.bass as bass
import concourse.bacc as bacc_mod
import concourse.tile as tile
import concourse.mybir as mybir
from concourse import bass_isa

F32 = mybir.dt.float32
U32 = mybir.dt.uint32
I16 = mybir.dt.int16
AF = mybir.ActivationFunctionType
OP = mybir.AluOpType
AX = mybir.AxisListType
EPS = 1e-5

B, N, S1, S2, K = 16, 8192, 512, 128, 16
NCORES = 8
BC = B // NCORES
C1A, C1B = 64, 128
C2A, C2B = 128, 256


def prep_common_weights(inp):
    f = np.float32
    w = {}
    A = lambda x: np.ascontiguousarray(np.asarray(x, f))
    pad4 = lambda a: np.concatenate([a, np.zeros((1, a.shape[1]), f)], 0)
    w1 = A(inp['sa1_c1_w'])
    w['w1s_T'] = A((w1[:, :3] + w1[:, 3:]).T)
    w['w1x_T'] = A(w1[:, :3].T)
    w['w2_T'] = A(np.asarray(inp['sa1_c2_w'], f).T)
    w['bn1_g'] = A(inp['sa1_bn_g']); w['bn1_be'] = A(inp['sa1_bn_be'])
    w['b1c2'] = A(inp['sa1_c2_b'])
    w2c1 = A(inp['sa2_c1_w'])
    w['A2x_T'] = pad4(A(w2c1[:, :3].T)); w['A2xn_T'] = A(-w['A2x_T'])
    w['A2p_T'] = A(w2c1[:, 3:].T)
    w['bn2_g'] = A(inp['sa2_bn_g']); w['bn2_be'] = A(inp['sa2_bn_be'])
    w2c2 = A(inp['sa2_c2_w'])
    w['B2a_T'] = A(w2c2[:128].T); w['B2b_T'] = A(w2c2[128:].T)
    w['b2c2'] = A(inp['sa2_c2_b'])
    w3c1 = A(inp['sa3_c1_w'])
    w['A3x_Ta'] = pad4(A(w3c1[:128, :3].T)); w['A3x_Tb'] = pad4(A(w3c1[128:, :3].T))
    w['A3pa_Ta'] = A(w3c1[:128, 3:131].T); w['A3pa_Tb'] = A(w3c1[128:, 3:131].T)
    w['A3pb_Ta'] = A(w3c1[:128, 131:259].T); w['A3pb_Tb'] = A(w3c1[128:, 131:259].T)
    w['bn3_g'] = A(inp['sa3_bn_g']); w['bn3_be'] = A(inp['sa3_bn_be'])
    w3c2 = A(inp['sa3_c2_w'])
    for r in range(2):
        for c in range(2):
            w[f'C3_{r}{c}'] = A(w3c2[128 * r:128 * (r + 1), 128 * c:128 * (c + 1)].T)
    w['b3c2'] = A(inp['sa3_c2_b'])
    f1 = A(inp['fc1_w'])
    for r in range(2):
        for c in range(2):
            w[f'FC1_{r}{c}'] = A(f1[128 * r:128 * (r + 1), 128 * c:128 * (c + 1)].T)
    w['fbn1_g'] = A(inp['fc1_bn_g']); w['fbn1_be'] = A(inp['fc1_bn_be'])
    f2 = A(inp['fc2_w'])
    w['FC2_0'] = A(f2[:, :128].T); w['FC2_1'] = A(f2[:, 128:].T)
    w['fbn2_g'] = A(inp['fc2_bn_g']); w['fbn2_be'] = A(inp['fc2_bn_be'])
    w['FC3_T'] = A(np.asarray(inp['fc3_w'], f).T)
    w['fc3_b'] = A(inp['fc3_b'])
    w['ident'] = np.eye(128, dtype=f)
    w['constrow'] = np.stack([np.full(1024, -1.0, f), np.zeros(1024, f)])
    return w


WEIGHT_SHAPES = {
    'w1s_T': (3, 64), 'w1x_T': (3, 64), 'w2_T': (64, 128),
    'bn1_g': (64,), 'bn1_be': (64,), 'b1c2': (128,),
    'A2x_T': (4, 128), 'A2xn_T': (4, 128), 'A2p_T': (128, 128),
    'bn2_g': (128,), 'bn2_be': (128,),
    'B2a_T': (128, 128), 'B2b_T': (128, 128), 'b2c2': (256,),
    'A3x_Ta': (4, 128), 'A3x_Tb': (4, 128),
    'A3pa_Ta': (128, 128), 'A3pa_Tb': (128, 128),
    'A3pb_Ta': (128, 128), 'A3pb_Tb': (128, 128),
    'bn3_g': (256,), 'bn3_be': (256,),
    'C3_00': (128, 128), 'C3_01': (128, 128), 'C3_10': (128, 128), 'C3_11': (128, 128),
    'b3c2': (256,),
    'FC1_00': (128, 128), 'FC1_01': (128, 128), 'FC1_10': (128, 128), 'FC1_11': (128, 128),
    'fbn1_g': (256,), 'fbn1_be': (256,),
    'FC2_0': (128, 128), 'FC2_1': (128, 128),
    'fbn2_g': (128,), 'fbn2_be': (128,),
    'FC3_T': (128, 1), 'fc3_b': (1,),
    'ident': (128, 128),
    'constrow': (2, 1024),
}


def build_nc(debug=False, no_cc=False, stop_after=None):
    nc = bacc_mod.Bacc()
    d = {'points': nc.dram_tensor("points", (BC, N, 3), F32, kind="ExternalInput")}
    for name, shp in WEIGHT_SHAPES.items():
        d[name] = nc.dram_tensor(name, shp, F32, kind="ExternalInput")
    out_d = nc.dram_tensor("out", (B, 1), F32, kind="ExternalOutput")
    F1d = [nc.dram_tensor(f"F1d{c}", (N, C1A), F32) for c in range(BC)]
    pre = nc.dram_tensor("pre", (3, 128, 128), F32)
    xyzTre = [nc.dram_tensor(f"xyzTre{c}", (4, N), F32) for c in range(BC)]
    nxTre = nc.dram_tensor("nxTre", (3, BC * S1), F32)
    pre2 = nc.dram_tensor("pre2", (3, 128, 8), F32)
    nxT2re = nc.dram_tensor("nxT2re", (3, BC * S2), F32)
    xyzT2re = [nc.dram_tensor(f"xyzT2re{c}", (4, S1), F32) for c in range(BC)]
    gre = nc.dram_tensor("gre", (2, 128, B), F32)
    nxd = nc.dram_tensor("nxd", (BC, S1, 3), F32)
    nxsd = nc.dram_tensor("nxsd", (128, 3, S1), F32)
    nxsd2 = nc.dram_tensor("nxsd2", (128, 3, S2), F32)
    F2d = [nc.dram_tensor(f"F2d{c}", (S1, 192), F32) for c in range(BC)]
    x1d = nc.dram_tensor("x1d", (BC, C1A, N), F32)
    x2d = nc.dram_tensor("x2d", (BC, C2A, S2 * K), F32)
    nx2d = nc.dram_tensor("nx2d", (BC, S2, 3), F32)
    cc1i = nc.dram_tensor("cc1i", (C1A, 2), F32)
    cc1o = nc.dram_tensor("cc1o", (C1A, 2), F32, addr_space="Shared")
    cc2i = nc.dram_tensor("cc2i", (C2A, 2), F32)
    cc2o = nc.dram_tensor("cc2o", (C2A, 2), F32, addr_space="Shared")
    cc3i = nc.dram_tensor("cc3i", (128, 4), F32)
    cc3o = nc.dram_tensor("cc3o", (128, 4), F32, addr_space="Shared")
    ggi = nc.dram_tensor("ggi", (2, BC, 128), F32)
    ggo = nc.dram_tensor("ggo", (NCORES, 2, BC, 128), F32, addr_space="Shared")
    dbg = {}
    if debug:
        dbg['nx'] = nc.dram_tensor("dbg_nx", (BC, S1, 3), F32, kind="ExternalOutput")
        dbg['idx1'] = nc.dram_tensor("dbg_idx1", (BC, 16, S1), U32, kind="ExternalOutput")
        dbg['l1p'] = nc.dram_tensor("dbg_l1p", (C1B, BC * S1), F32, kind="ExternalOutput")
        dbg['nx2'] = nc.dram_tensor("dbg_nx2", (BC, S2, 3), F32, kind="ExternalOutput")
        dbg['idx2'] = nc.dram_tensor("dbg_idx2", (BC, 16, S2), U32, kind="ExternalOutput")
        dbg['l2pa'] = nc.dram_tensor("dbg_l2pa", (128, BC * S2), F32, kind="ExternalOutput")
        dbg['l2pb'] = nc.dram_tensor("dbg_l2pb", (128, BC * S2), F32, kind="ExternalOutput")
        dbg['ga'] = nc.dram_tensor("dbg_ga", (128, BC), F32, kind="ExternalOutput")
        dbg['gb'] = nc.dram_tensor("dbg_gb", (128, BC), F32, kind="ExternalOutput")
    with tile.TileContext(nc) as tc:
        with nc.allow_non_contiguous_dma(reason="small strided restaging DMAs"), ExitStack() as ctx:
            emit(ctx, tc, d, out_d, F1d, nxd, F2d, nx2d, x1d, x2d,
                 (pre, xyzTre, nxTre, pre2, nxT2re, xyzT2re, gre, nxsd, nxsd2),
                 (cc1i, cc1o), (cc2i, cc2o), (cc3i, cc3o), (ggi, ggo), dbg, no_cc,
                 stop_after)
    nc.compile()
    return nc


def fps_loop(ctx, tc, pool, xs, ys, zs, xyzneg, nx, nsteps, free, name, ones1, psum,
             nxs=None):
    nc = tc.nc
    dist = pool.tile([128, free], F32, name=f"{name}_dist")
    nc.vector.memset(dist[:], 1e10)
    cneg = pool.tile([128, 3], F32, name=f"{name}_cneg")
    m8 = pool.tile([128, 8], F32, name=f"{name}_m8")
    gm = pool.tile([128, 1], F32, name=f"{name}_gm")
    r = pool.tile([128, 3], F32, name=f"{name}_r")
    junk = pool.tile([128, free], F32, name=f"{name}_junk")
    e1 = pool.tile([128, free], F32, name=f"{name}_e1")
    e2 = pool.tile([128, free], F32, name=f"{name}_e2")
    e3 = pool.tile([128, free], F32, name=f"{name}_e3")
    aa = pool.tile([128, free], F32, name=f"{name}_aa")
    ind0 = pool.tile([128, free], F32, name=f"{name}_ind0")
    nc.vector.memset(ind0[:], 0.0)
    nc.vector.memset(ind0[0:1, 0:1], 1.0)
    nc.vector.memset(ind0[64:65, 0:1], 1.0)
    # hi-half (partition 64:128) slices of partition_all_reduce return zeros on
    # HW, so route per-cloud reductions through disjoint COLUMNS of full-128
    # reduces: m2 packs per-cloud maxima, r6 per-cloud coordinate sums.
    m2 = pool.tile([128, 2], F32, name=f"{name}_m2")
    nc.vector.memset(m2[:], -1e30)
    gm2 = pool.tile([128, 2], F32, name=f"{name}_gm2")
    r6 = pool.tile([128, 6], F32, name=f"{name}_r6")
    nc.vector.memset(r6[:], 0.0)
    c6 = pool.tile([128, 6], F32, name=f"{name}_c6")

    def extract_c(mask_src, scal):
        # r[p,d] = sum_f (mask==scal ? -coord); full-128 add -> cneg everywhere
        for dd in range(3):
            nc.vector.scalar_tensor_tensor(
                out=junk[:], in0=mask_src, scalar=scal, in1=xyzneg[:, dd, :],
                op0=OP.is_equal, op1=OP.mult, accum_out=r[:, dd:dd + 1])
        nc.vector.tensor_copy(out=r6[0:64, 0:3], in_=r[0:64, :])
        nc.vector.tensor_copy(out=r6[64:128, 3:6], in_=r[64:128, :])
        nc.gpsimd.partition_all_reduce(c6[:], r6[:], 128, bass_isa.ReduceOp.add)
        nc.vector.tensor_copy(out=cneg[0:64, :], in_=c6[0:64, 0:3])
        nc.vector.tensor_copy(out=cneg[64:128, :], in_=c6[64:128, 3:6])

    def record(i):
        if nxs is not None:
            # all partitions hold their cloud-half's reduced value; negate into
            # the transposed [128, 3, nsteps] buffer
            nc.vector.tensor_scalar_mul(out=nxs[:, :, i], in0=cneg[:],
                                        scalar1=-1.0)
        else:
            nc.scalar.activation(out=nx[0:1, i, :], in_=cneg[0:1, :],
                                 func=AF.Copy, scale=-1.0)
            nc.scalar.activation(out=nx[64:65, i, :], in_=cneg[64:65, :],
                                 func=AF.Copy, scale=-1.0)

    extract_c(ind0[:], 1.0)
    record(0)
    for i in range(1, nsteps):
        nc.scalar.activation(out=e1[:], in_=xyzneg[:, 0, :], func=AF.Square,
                             scale=-1.0, bias=cneg[:, 0:1])
        nc.scalar.activation(out=e2[:], in_=xyzneg[:, 1, :], func=AF.Square,
                             scale=-1.0, bias=cneg[:, 1:2])
        nc.scalar.activation(out=e3[:], in_=xyzneg[:, 2, :], func=AF.Square,
                             scale=-1.0, bias=cneg[:, 2:3])
        nc.vector.tensor_tensor(out=aa[:], in0=e1[:], in1=e2[:], op=OP.add)
        nc.vector.tensor_tensor(out=e1[:], in0=aa[:], in1=e3[:], op=OP.add)
        nc.vector.tensor_tensor(out=dist[:], in0=dist[:], in1=e1[:], op=OP.min)
        nc.vector.max(m8[:], dist[:])
        nc.vector.tensor_copy(out=m2[0:64, 0:1], in_=m8[0:64, 0:1])
        nc.vector.tensor_copy(out=m2[64:128, 1:2], in_=m8[64:128, 0:1])
        nc.gpsimd.partition_all_reduce(gm2[:], m2[:], 128, bass_isa.ReduceOp.max)
        nc.vector.tensor_copy(out=gm[0:64, :], in_=gm2[0:64, 0:1])
        nc.vector.tensor_copy(out=gm[64:128, :], in_=gm2[64:128, 1:2])
        extract_c(dist[:], gm[:, 0:1])
        record(i)


def fps_loop_split(ctx, tc, pool, xyzsrc, nsteps, vw, name, nxs, on_chunk=None,
                   chunk=None, on_step=None):
    """Per-cloud FPS chains: cloud c uses its own [128, vw] tiles spanning all
    128 partitions (point idx = p*vw + f), so reductions are full-128 (the only
    partition_all_reduce form that works on HW). The two chains interleave on
    the engines. xyzsrc(c, dd) -> DRAM AP of cloud c's coord row, (128, vw).
    Records into nxs[128, 3, nsteps] partition-halves (downstream layout
    unchanged: cloud c at partition 64*c)."""
    nc = tc.nc
    mf = max(vw, 8)
    T = {}
    for c in range(2):
        xyzneg = pool.tile([128, 3, vw], F32, name=f"{name}_xyzn{c}")
        for dd in range(3):
            xt = pool.tile([128, vw], F32, name=f"{name}_x{c}{dd}")
            nc.gpsimd.dma_start(xt[:], xyzsrc(c, dd))
            nc.vector.tensor_scalar_mul(out=xyzneg[:, dd, :], in0=xt[:],
                                        scalar1=-1.0)
        dist = pool.tile([128, mf], F32, name=f"{name}_dist{c}")
        nc.vector.memset(dist[:], 1e10)
        if mf > vw:
            nc.vector.memset(dist[:, vw:mf], -1e30)
        # per-step extract history: the add-reduce writes straight into
        # hist[:, :, i]; step i+1's bias reads hist[:, d, i]; one bulk negate
        # after the loop replaces per-step record ops
        hist = pool.tile([128, 3, nsteps], F32, name=f"{name}_hist{c}")
        m8 = pool.tile([128, 8], F32, name=f"{name}_m8{c}")
        gm = pool.tile([128, 1], F32, name=f"{name}_gm{c}")
        r = pool.tile([128, 3], F32, name=f"{name}_r{c}")
        junk = pool.tile([128, vw], F32, name=f"{name}_junk{c}")
        e1 = pool.tile([128, vw], F32, name=f"{name}_e1{c}")
        e2 = pool.tile([128, vw], F32, name=f"{name}_e2{c}")
        e3 = pool.tile([128, vw], F32, name=f"{name}_e3{c}")
        aa = pool.tile([128, vw], F32, name=f"{name}_aa{c}")
        ind0 = pool.tile([128, vw], F32, name=f"{name}_ind0{c}")
        nc.vector.memset(ind0[:], 0.0)
        nc.vector.memset(ind0[0:1, 0:1], 1.0)
        T[c] = (xyzneg, dist, hist, m8, gm, r, junk, e1, e2, e3, aa, ind0)

    def extract_c(c, mask_src, scal, i):
        xyzneg, dist, hist, m8, gm, r, junk = T[c][:7]
        for dd in range(3):
            nc.vector.scalar_tensor_tensor(
                out=junk[:], in0=mask_src, scalar=scal, in1=xyzneg[:, dd, :],
                op0=OP.is_equal, op1=OP.mult, accum_out=r[:, dd:dd + 1])
        nc.gpsimd.partition_all_reduce(hist[:, :, i], r[:], 128,
                                       bass_isa.ReduceOp.add)

    for c in range(2):
        extract_c(c, T[c][11][:], 1.0, 0)
    # phase-interleaved emission: both clouds' reduces are in flight before
    # either cloud's dependent phase queues, so the in-order engine queues
    # overlap the two serial chains.
    for i in range(1, nsteps):
        for c in range(2):
            xyzneg, dist, hist, m8, gm, r, junk, e1, e2, e3, aa, ind0 = T[c]
            nc.scalar.activation(out=e1[:], in_=xyzneg[:, 0, :], func=AF.Square,
                                 scale=-1.0, bias=hist[:, 0:1, i - 1])
            nc.scalar.activation(out=e2[:], in_=xyzneg[:, 1, :], func=AF.Square,
                                 scale=-1.0, bias=hist[:, 1:2, i - 1])
            nc.scalar.activation(out=e3[:], in_=xyzneg[:, 2, :], func=AF.Square,
                                 scale=-1.0, bias=hist[:, 2:3, i - 1])
            nc.vector.tensor_tensor(out=aa[:], in0=e1[:], in1=e2[:], op=OP.add)
            nc.vector.tensor_tensor(out=e1[:], in0=aa[:], in1=e3[:], op=OP.add)
            nc.vector.tensor_tensor(out=dist[:, 0:vw], in0=dist[:, 0:vw],
                                    in1=e1[:], op=OP.min)
            nc.vector.max(m8[:], dist[:])
            nc.gpsimd.partition_all_reduce(gm[:], m8[:, 0:1], 128,
                                           bass_isa.ReduceOp.max)
        if on_step is not None:
            # emitted between the max-reduce issue and the dependent extract,
            # so the drained KNN piece runs during the gpsimd round-trip
            on_step()
        for c in range(2):
            extract_c(c, T[c][1][:, 0:vw], T[c][4][:, 0:1], i)
        if chunk is not None and (i + 1) % chunk == 0:
            # chunk of samples complete: negate its history slice into nxs and
            # hand off (e.g. to emit the KNN tiles that only need these queries)
            j = (i + 1) // chunk - 1
            for c in range(2):
                nc.vector.tensor_scalar_mul(
                    out=nxs[64 * c:64 * (c + 1), :, chunk * j:chunk * (j + 1)],
                    in0=T[c][2][64 * c:64 * (c + 1), :, chunk * j:chunk * (j + 1)],
                    scalar1=-1.0)
            if on_chunk is not None:
                on_chunk(j)
    if chunk is None:
        # bulk negate the per-step history into the shared nxs record buffer
        for c in range(2):
            nc.vector.tensor_scalar_mul(
                out=nxs[64 * c:64 * (c + 1), :, :],
                in0=T[c][2][64 * c:64 * (c + 1), :, :], scalar1=-1.0)


def bn_affine(tc, pool, sums, sqs, g_sb, be_sb, count, cpart, name):
    nc = tc.nc
    mean = pool.tile([cpart, 1], F32, name=f"{name}_mean")
    var = pool.tile([cpart, 1], F32, name=f"{name}_var")
    scale = pool.tile([cpart, 1], F32, name=f"{name}_scale")
    bias = pool.tile([cpart, 1], F32, name=f"{name}_bias")
    tmp = pool.tile([cpart, 1], F32, name=f"{name}_tmp")
    inv_n = 1.0 / float(count)
    nc.scalar.mul(mean[:], sums, inv_n)
    nc.scalar.mul(var[:], sqs, inv_n)
    nc.vector.tensor_tensor(out=tmp[:], in0=mean[:], in1=mean[:], op=OP.mult)
    nc.vector.tensor_tensor(out=var[:], in0=var[:], in1=tmp[:], op=OP.subtract)
    nc.vector.tensor_scalar_add(out=var[:], in0=var[:], scalar1=EPS)
    nc.vector.reciprocal(tmp[:], var[:])
    nc.scalar.activation(out=tmp[:], in_=tmp[:], func=AF.Sqrt)
    nc.vector.tensor_tensor(out=scale[:], in0=tmp[:], in1=g_sb, op=OP.mult)
    nc.vector.tensor_tensor(out=tmp[:], in0=mean[:], in1=scale[:], op=OP.mult)
    nc.vector.tensor_tensor(out=bias[:], in0=be_sb, in1=tmp[:], op=OP.subtract)
    scale_a = pool.tile([cpart, 1], F32, name=f"{name}_scale_a")
    bias_a = pool.tile([cpart, 1], F32, name=f"{name}_bias_a")
    nc.scalar.activation(out=scale_a[:], in_=scale[:], func=AF.Copy)
    nc.scalar.activation(out=bias_a[:], in_=bias[:], func=AF.Copy)
    return scale_a, bias_a


def emit(ctx, tc, d, out_d, F1d, nxd, F2d, nx2d, x1d, x2d, stg, cc1, cc2, cc3, gg, dbg,
         no_cc=False, stop_after=None):
    pre, xyzTre, nxTre, pre2, nxT2re, xyzT2re, gre, nxsd, nxsd2 = stg
    nc = tc.nc

    def bail():
        zout = sing.tile([16, 1], F32, name="zout")
        nc.vector.memset(zout[:], 0.0)
        nc.gpsimd.dma_start(out_d[:], zout[:])
    P = 128
    RG = [list(range(NCORES))]
    sing = ctx.enter_context(tc.tile_pool(name="sing", bufs=1))
    big = ctx.enter_context(tc.tile_pool(name="big", bufs=1))
    work = ctx.enter_context(tc.tile_pool(name="work", bufs=1))
    psum = ctx.enter_context(tc.tile_pool(name="psum", bufs=3, space="PSUM"))
    psumT = ctx.enter_context(tc.tile_pool(name="psumT", bufs=3, space="PSUM"))
    bpool = ctx.enter_context(tc.tile_pool(name="bnp", bufs=1))


    ones1 = sing.tile([1, 128], F32, name="ones1")
    nc.vector.memset(ones1[:], 1.0)
    wsb = {}
    for name, shp in WEIGHT_SHAPES.items():
        if len(shp) == 1:
            if shp[0] > 128:
                for hh in range(shp[0] // 128):
                    t = sing.tile([128, 1], F32, name=f"w_{name}_{hh}")
                    nc.gpsimd.dma_start(t[:], d[name][128 * hh:128 * (hh + 1), None])
                    wsb[f"{name}_{hh}"] = t
                continue
            t = sing.tile([shp[0], 1], F32, name=f"w_{name}")
            nc.gpsimd.dma_start(t[:], d[name][:, None])
        else:
            t = sing.tile(list(shp), F32, name=f"w_{name}")
            nc.gpsimd.dma_start(t[:], d[name][:])
        wsb[name] = t

    # ---- points load (restage so each SBUF tile = ONE DMA) ----
    for dd in range(3):
        for c in range(BC):
            nc.gpsimd.dma_start(
                pre[dd, 64 * c:64 * (c + 1), :],
                d['points'][c, :, dd].rearrange("(p f) -> p f", p=64))
    xs = sing.tile([P, 128], F32, name="xs")
    ys = sing.tile([P, 128], F32, name="ys")
    zs = sing.tile([P, 128], F32, name="zs")
    for dd, t in enumerate((xs, ys, zs)):
        nc.gpsimd.dma_start(t[:], pre[dd])
    xyzneg = sing.tile([P, 3, 128], F32, name="xyzneg")
    for dd, t in enumerate((xs, ys, zs)):
        nc.vector.tensor_scalar_mul(out=xyzneg[:, dd, :], in0=t[:], scalar1=-1.0)
    sqt0 = work.tile([P, 128], F32, name="sqt0", tag="sqt0")
    rnf = sing.tile([P, 128], F32, name="rnf")
    nc.scalar.activation(out=rnf[:], in_=xyzneg[:, 0, :], func=AF.Square)
    nc.scalar.activation(out=sqt0[:], in_=xyzneg[:, 1, :], func=AF.Square)
    nc.vector.tensor_tensor(out=rnf[:], in0=rnf[:], in1=sqt0[:], op=OP.add)
    nc.scalar.activation(out=sqt0[:], in_=xyzneg[:, 2, :], func=AF.Square)
    nc.vector.tensor_tensor(out=rnf[:], in0=rnf[:], in1=sqt0[:], op=OP.add)
    # xyzT staging: rows xyz from points, row3 = rn (per cloud), all in DRAM
    for c in range(BC):
        for dd, t in enumerate((xs, ys, zs)):
            nc.gpsimd.dma_start(xyzTre[c][dd:dd + 1, :], t[64 * c:64 * (c + 1), :])
        nc.gpsimd.dma_start(xyzTre[c][3:4, :], rnf[64 * c:64 * (c + 1), :])
    xyzTt = sing.tile([4, N], F32, name="xyzTt")

    def fill_xyzT(c):
        nc.gpsimd.dma_start(xyzTt[:], xyzTre[c][:])

    # ---- F1 rows-major -> F1d ----
    for c in range(BC):
        fill_xyzT(c)
        for j in range(8):
            ps = psum.tile([P, 512], F32, name="f1ps", tag="mm")
            st = work.tile([P, 512], F32, name="f1st", tag="f1st")
            for jj in range(8):
                ch = 8 * j + jj
                nc.tensor.matmul(ps[:, 64 * jj:64 * (jj + 1)],
                                 xyzTt[0:3, 128 * ch:128 * (ch + 1)],
                                 wsb['w1s_T'][:])
            nc.scalar.activation(out=st[:], in_=ps[:], func=AF.Copy)
            nc.gpsimd.dma_start(
                F1d[c][:].rearrange("(j p) q -> p j q", p=128)[:, 8 * j:8 * (j + 1), :],
                st[:].rearrange("p (j q) -> p j q", j=8))

    if stop_after == 1:
        bail()
        return

    # ---- FPS1 with KNN1 tiles emitted per 128-sample chunk so the KNN
    # matmul/gather/scan work fills FPS1's idle engine time ----
    nxs = sing.tile([P, 3, S1], F32, name="nxs")
    nxT = sing.tile([3, BC * S1], F32, name="nxT")
    q4T = sing.tile([4, BC * S1], F32, name="q4T")
    nc.gpsimd.dma_start(q4T[3:4, :], d['constrow'][0:1, :])
    Gc = sing.tile([C1A, BC * S1], F32, name="Gc")
    scores = big.tile([P, N], F32, name="scores")
    sum1 = sing.tile([C1A, 128], F32, name="sum1")
    sq1 = sing.tile([C1A, 128], F32, name="sq1")
    l1pT = big.tile([C1B, BC * S1], F32, name="l1pT")
    fpool = ctx.enter_context(tc.tile_pool(name="fps1", bufs=1))

    def stage_chunk1(j):
        # queries 128j..128(j+1) of each cloud are final: stage nxsd/nxT/q4T/Gc
        # for them, then emit their two KNN tiles (t=j cloud 0, t=4+j cloud 1)
        nc.gpsimd.dma_start(nxsd[:, :, 128 * j:128 * (j + 1)],
                            nxs[:, :, 128 * j:128 * (j + 1)])
        for c in range(BC):
            q0 = S1 * c + 128 * j
            nc.gpsimd.dma_start(nxT[:, q0:q0 + 128],
                                nxsd[64 * c][:, 128 * j:128 * (j + 1)])
            nc.vector.tensor_scalar_mul(out=q4T[0:3, q0:q0 + 128],
                                        in0=nxT[:, q0:q0 + 128], scalar1=2.0)
            psg = psum.tile([C1A, 128], F32, name="gcps", tag="mm")
            nc.tensor.matmul(psg[:], wsb['w1x_T'][:], nxT[:, q0:q0 + 128])
            nc.scalar.activation(out=Gc[:, q0:q0 + 128], in_=psg[:], func=AF.Copy)
        knn_q.append(emit_knn1_tile(j))
        knn_q.append(emit_knn1_tile(4 + j))

    knn_q = []

    def drain_knn(n=1):
        # advance the pending KNN generators by n pieces (called once per FPS
        # step so KNN work lands in FPS1's per-step latency bubbles)
        for _ in range(n):
            while knn_q:
                try:
                    next(knn_q[0])
                    return
                except StopIteration:
                    knn_q.pop(0)
            return

    def emit_knn1_tile(t):
        c = t // 4
        fill_xyzT(c)
        for jj in range(16):
            ps = psum.tile([P, 512], F32, name="knnps", tag="mm")
            nc.tensor.matmul(ps[:], q4T[:, 128 * t:128 * (t + 1)],
                             xyzTt[:, 512 * jj:512 * (jj + 1)])
            nc.scalar.activation(out=scores[:, 512 * jj:512 * (jj + 1)], in_=ps[:],
                                  func=AF.Copy)
            yield
        m8a = work.tile([P, 8], F32, name="m8a", tag="m8a")
        m8b = work.tile([P, 8], F32, name="m8b", tag="m8b")
        ia = work.tile([P, 16], U32, name="iab", tag="iab")
        nc.vector.max(m8a[:], scores[:])
        yield
        nc.vector.max_index(ia[:, 0:8], m8a[:], scores[:])
        yield
        nc.vector.match_replace(scores[:], m8a[:], scores[:], -1e30)
        yield
        nc.vector.max(m8b[:], scores[:])
        yield
        nc.vector.max_index(ia[:, 8:16], m8b[:], scores[:])
        yield
        if dbg:
            iaf = work.tile([P, 16], F32, name="iaf", tag="iaf")
            nc.vector.tensor_copy(out=iaf[:], in_=ia[:])
            pst = psumT.tile([16, P], F32, name="idxps", tag="T")
            nc.tensor.transpose(pst[:], iaf[:], wsb['ident'][:])
            dcp = work.tile([16, P], U32, name="dcp", tag="dcp")
            nc.vector.tensor_copy(out=dcp[:], in_=pst[:])
            nc.gpsimd.dma_start(dbg['idx1'][c, :, 128 * (t % 4):128 * (t % 4 + 1)],
                              dcp[:])
        # gather + conv1-space blocks, k-major columns: col = 512*k + 128*(t%4) + q
        for k in range(K):
            gblk = work.tile([P, C1A], F32, name="gblk", tag="gblk")
            nc.gpsimd.indirect_dma_start(
                out=gblk[:], out_offset=None, in_=F1d[c][:],
                in_offset=bass.IndirectOffsetOnAxis(ap=ia[:, k:k + 1], axis=0))
            psx1 = psumT.tile([C1A, P], F32, name="psx1", tag="T")
            nc.tensor.transpose(psx1[:], gblk[:], wsb['ident'][:])
            q0 = S1 * c + 128 * (t % 4)
            xblk = work.tile([C1A, P], F32, name="xblk", tag="xblk")
            nc.vector.scalar_tensor_tensor(
                out=xblk[:], in0=psx1[:], scalar=0.0,
                in1=Gc[:, q0:q0 + 128],
                op0=OP.bypass, op1=OP.subtract,
                accum_out=sum1[:, 64 * c + 16 * (t % 4) + k:64 * c + 16 * (t % 4) + k + 1])
            sqt = work.tile([C1A, P], F32, name="sqt", tag="sqt")
            nc.scalar.activation(
                out=sqt[:], in_=xblk[:], func=AF.Square,
                accum_out=sq1[:, 64 * c + 16 * (t % 4) + k:64 * c + 16 * (t % 4) + k + 1])
            nc.gpsimd.dma_start(
                x1d[c, :, 512 * k + 128 * (t % 4):512 * k + 128 * (t % 4) + 128],
                xblk[:])
            yield

    if stop_after == 22:
        nc.vector.memset(nxs[:], 0.25)
        bail()
        return
    fps_loop_split(ctx, tc, fpool,
                   lambda c, dd: xyzTre[c][dd, :].rearrange("(p f) -> p f", p=128),
                   S1, 64, "f1", nxs, on_chunk=stage_chunk1, chunk=128,
                   on_step=drain_knn)
    while knn_q:
        drain_knn()
    if dbg:
        for c in range(BC):
            nc.gpsimd.dma_start(dbg['nx'][c],
                                nxsd[64 * c].rearrange("dd q -> q dd"))
    if stop_after in (2, 21):
        bail()
        return

    # FPS2 emitted here (only needs nxsd): the last KNN1 chunk and the
    # BN1-collective round-trip hide inside its step latency bubbles
    nxs2 = sing.tile([P, 3, S2], F32, name="nxs2")
    fpool2 = ctx.enter_context(tc.tile_pool(name="fps2", bufs=1))
    fps_loop_split(ctx, tc, fpool2,
                   lambda c, dd: nxsd[64 * c, dd, :].rearrange(
                       "(p f) -> p f", p=128),
                   S2, 4, "f2", nxs2, on_step=drain_knn, chunk=S2)
    nc.gpsimd.dma_start(nxsd2[:], nxs2[:])
    if dbg:
        for c in range(BC):
            nc.gpsimd.dma_start(dbg['nx2'][c],
                                nxsd2[64 * c].rearrange("dd q -> q dd"))

    nxT2f = sing.tile([4, BC * S2], F32, name="nxT2f")
    for c in range(BC):
        nc.gpsimd.dma_start(nxT2f[0:3, S2 * c:S2 * (c + 1)], nxsd2[64 * c])
    nc.gpsimd.dma_start(nxT2f[3:4, :], d['constrow'][1:2, 0:BC * S2])
    nxT2 = nxT2f
    q4T2 = sing.tile([3, BC * S2], F32, name="q4T2")
    nc.vector.tensor_scalar_mul(out=q4T2[:], in0=nxT2[0:3, :], scalar1=2.0)
    monerow = sing.tile([1, 128], F32, name="monerow")
    nc.gpsimd.dma_start(monerow[:], d['constrow'][0:1, 0:128])
    xyzT2 = [sing.tile([3, S1], F32, name=f"xyzT2_{c}") for c in range(BC)]
    rn2ts = [sing.tile([1, S1], F32, name=f"rn2t_{c}") for c in range(BC)]
    ones3 = sing.tile([3, 1], F32, name="ones3")
    nc.vector.memset(ones3[:], 1.0)
    for c in range(BC):
        nc.gpsimd.dma_start(xyzT2[c][:], nxsd[64 * c])
        sq2t = work.tile([3, S1], F32, name="sq2t", tag="sq2t")
        nc.scalar.activation(out=sq2t[:], in_=xyzT2[c][:], func=AF.Square)
        psr = psum.tile([1, S1], F32, name="rnps", tag="mm")
        nc.tensor.matmul(psr[:], ones3[:], sq2t[:])
        nc.vector.tensor_copy(out=rn2ts[c][:], in_=psr[:])


    red1 = sing.tile([C1A, 2], F32, name="red1")
    nc.vector.tensor_reduce(out=red1[:, 0:1], in_=sum1[:, None, :], axis=AX.X, op=OP.add)
    nc.vector.tensor_reduce(out=red1[:, 1:2], in_=sq1[:, None, :], axis=AX.X, op=OP.add)
    nc.gpsimd.dma_start(cc1[0][:], red1[:])
    if stop_after == 3:
        bail()
        return
    if not no_cc:
        nc.gpsimd.collective_compute("AllReduce", OP.add, replica_groups=RG,
                                     ins=[cc1[0][:]], outs=[cc1[1][:]])
    stat1 = sing.tile([C1A, 2], F32, name="stat1")
    nc.gpsimd.dma_start(stat1[:], cc1[0 if no_cc else 1][:])
    sc1, bi1 = bn_affine(tc, bpool, stat1[:, 0:1], stat1[:, 1:2],
                         wsb['bn1_g'][:], wsb['bn1_be'][:], B * S1 * K, C1A, "bn1")

    for c in range(BC):
        for k in range(K):
            col = 512 * k
            x1c = work.tile([C1A, 512], F32, name="x1c", tag="x1c")
            nc.gpsimd.dma_start(x1c[:], x1d[c, :, col:col + 512])
            h1 = work.tile([C1A, 512], F32, name="h1", tag="h1")
            nc.scalar.activation(out=h1[:], in_=x1c[:], func=AF.Relu,
                                 scale=sc1[:], bias=bi1[:])
            ps = psum.tile([C1B, 512], F32, name="c2ps", tag="mm")
            nc.tensor.matmul(ps[:], wsb['w2_T'][:], h1[:])
            sl = l1pT[:, S1 * c:S1 * (c + 1)]
            if k == 0:
                nc.vector.tensor_copy(out=sl, in_=ps[:])
            else:
                nc.vector.tensor_tensor(out=sl, in0=sl, in1=ps[:], op=OP.max)
    nc.vector.tensor_scalar(out=l1pT[:], in0=l1pT[:], scalar1=wsb['b1c2'][:],
                            scalar2=None, op0=OP.add)
    if dbg:
        nc.gpsimd.dma_start(dbg['l1p'][:], l1pT[:])
    if stop_after == 4:
        bail()
        return

    # ---- SA2 prep ----
    zpad = sing.tile([128, 60], F32, name="zpad")
    nc.vector.memset(zpad[:], 0.0)
    for c in range(BC):
        nc.gpsimd.dma_start(F2d[c][:, 0:3],
                            nxsd[64 * c].rearrange("dd q -> q dd"))
        for j in range(4):
            nc.gpsimd.dma_start(F2d[c][128 * j:128 * (j + 1), 3], zpad[:, 0:1])
            nc.gpsimd.dma_start(F2d[c][128 * j:128 * (j + 1), 132:192], zpad[:])
        for j in range(4):
            pst = psumT.tile([P, P], F32, name="ftps", tag="T")
            nc.tensor.transpose(pst[:], l1pT[:, S1 * c + 128 * j:S1 * c + 128 * (j + 1)],
                                wsb['ident'][:])
            stg = work.tile([P, P], F32, name="fstg", tag="fstg")
            nc.vector.tensor_copy(out=stg[:], in_=pst[:])
            nc.gpsimd.dma_start(F2d[c][128 * j:128 * (j + 1), 4:132], stg[:])

    if stop_after == 5:
        bail()
        return

    # ---- KNN2 + gather + MLP2 ----
    sum2 = sing.tile([C2A, 8], F32, name="sum2")
    sq2 = sing.tile([C2A, 8], F32, name="sq2")
    l2paT = big.tile([128, BC * S2], F32, name="l2paT")
    x2sb = big.tile([C2A, BC * S2 * K], F32, name="x2sb")
    l2pbT = big.tile([128, BC * S2], F32, name="l2pbT")

    for c in range(BC):
        ps = psum.tile([P, S1], F32, name="kn2ps", tag="mm")
        nc.tensor.matmul(ps[:], q4T2[:, S2 * c:S2 * (c + 1)], xyzT2[c][:],
                         start=True, stop=False)
        nc.tensor.matmul(ps[:], monerow[:], rn2ts[c][:], start=False, stop=True)
        sc2t = work.tile([P, S1], F32, name="sc2t", tag="sc2t")
        nc.scalar.activation(out=sc2t[:], in_=ps[:], func=AF.Copy)
        m8a = work.tile([P, 8], F32, name="m8a2", tag="m8a2")
        m8b = work.tile([P, 8], F32, name="m8b2", tag="m8b2")
        ia = work.tile([P, 16], U32, name="iab2", tag="iab2")
        nc.vector.max(m8a[:], sc2t[:])
        nc.vector.max_index(ia[:, 0:8], m8a[:], sc2t[:])
        nc.vector.match_replace(sc2t[:], m8a[:], sc2t[:], -1e30)
        nc.vector.max(m8b[:], sc2t[:])
        nc.vector.max_index(ia[:, 8:16], m8b[:], sc2t[:])
        if dbg:
            iaf2 = work.tile([P, 16], F32, name="iaf2", tag="iaf2")
            nc.vector.tensor_copy(out=iaf2[:], in_=ia[:])
            pst2 = psumT.tile([16, P], F32, name="idx2ps", tag="T")
            nc.tensor.transpose(pst2[:], iaf2[:], wsb['ident'][:])
            dcp2 = work.tile([16, P], U32, name="dcp2", tag="dcp2")
            nc.vector.tensor_copy(out=dcp2[:], in_=pst2[:])
            nc.gpsimd.dma_start(dbg['idx2'][c], dcp2[:])
        rhx = big.tile([4, S2 * K], F32, name="rhx", tag="rhx")
        rhp = big.tile([C2A, S2 * K], F32, name="rhp", tag="rhp")
        for k in range(K):
            gblk2 = work.tile([P, 192], F32, name="gblk2", tag="gblk2")
            nc.gpsimd.indirect_dma_start(
                out=gblk2[:], out_offset=None, in_=F2d[c][:],
                in_offset=bass.IndirectOffsetOnAxis(ap=ia[:, k:k + 1], axis=0))
            psx = psumT.tile([4, P], F32, name="psx", tag="T")
            nc.tensor.transpose(psx[:], gblk2[:, 0:4], wsb['ident'][:])
            nc.vector.tensor_copy(out=rhx[:, 128 * k:128 * (k + 1)], in_=psx[:])
            psp = psumT.tile([C2A, P], F32, name="psp", tag="T")
            nc.tensor.transpose(psp[:], gblk2[:, 4:132], wsb['ident'][:])
            nc.vector.tensor_copy(out=rhp[:, 128 * k:128 * (k + 1)], in_=psp[:])
        for chk in range(4):
            col = 512 * chk
            ps2 = psum.tile([C2A, 512], F32, name="c1ps2", tag="mm")
            nc.tensor.matmul(ps2[:], wsb['A2x_T'][:], rhx[:, col:col + 512],
                             start=True, stop=False)
            nc.tensor.matmul(ps2[:], wsb['A2p_T'][:], rhp[:, col:col + 512],
                             start=False, stop=False)
            nc.tensor.matmul(
                ps2[:], wsb['A2xn_T'][:],
                nxT2[:, S2 * c:S2 * (c + 1)][:, None, :].broadcast_to((4, 4, S2)),
                start=False, stop=True)
            x2col = S2 * K * c + col
            nc.scalar.activation(out=x2sb[:, x2col:x2col + 512], in_=ps2[:],
                                 func=AF.Copy,
                                 accum_out=sum2[:, 4 * c + chk:4 * c + chk + 1])
            sqt2 = work.tile([C2A, 512], F32, name="sqt2", tag="sqt2")
            nc.scalar.activation(out=sqt2[:], in_=x2sb[:, x2col:x2col + 512],
                                 func=AF.Square,
                                 accum_out=sq2[:, 4 * c + chk:4 * c + chk + 1])

    red2 = sing.tile([C2A, 2], F32, name="red2")
    nc.vector.tensor_reduce(out=red2[:, 0:1], in_=sum2[:, None, :], axis=AX.X, op=OP.add)
    nc.vector.tensor_reduce(out=red2[:, 1:2], in_=sq2[:, None, :], axis=AX.X, op=OP.add)
    nc.gpsimd.dma_start(cc2[0][:], red2[:])
    if not no_cc:
        nc.gpsimd.collective_compute("AllReduce", OP.add, replica_groups=RG,
                                     ins=[cc2[0][:]], outs=[cc2[1][:]])
    stat2 = sing.tile([C2A, 2], F32, name="stat2")
    nc.gpsimd.dma_start(stat2[:], cc2[0 if no_cc else 1][:])
    sc2, bi2 = bn_affine(tc, bpool, stat2[:, 0:1], stat2[:, 1:2],
                         wsb['bn2_g'][:], wsb['bn2_be'][:], B * S2 * K, C2A, "bn2")

    for c in range(BC):
        for chk in range(4):
            col = S2 * K * c + 512 * chk
            h2 = work.tile([C2A, 512], F32, name="h2", tag="h2")
            nc.scalar.activation(out=h2[:], in_=x2sb[:, col:col + 512],
                                 func=AF.Relu, scale=sc2[:], bias=bi2[:])
            psa = psum.tile([128, 512], F32, name="c2psa", tag="mm")
            nc.tensor.matmul(psa[:], wsb['B2a_T'][:], h2[:])
            psb = psum.tile([128, 512], F32, name="c2psb", tag="mm")
            nc.tensor.matmul(psb[:], wsb['B2b_T'][:], h2[:])
            for half, (pp, ll) in enumerate(((psa, l2paT), (psb, l2pbT))):
                sl = ll[:, S2 * c:S2 * (c + 1)]
                for kk in range(4):
                    yk = pp[:, 128 * kk:128 * (kk + 1)]
                    if chk == 0 and kk == 0:
                        nc.vector.tensor_copy(out=sl, in_=yk)
                    else:
                        nc.vector.tensor_tensor(out=sl, in0=sl, in1=yk, op=OP.max)
    nc.vector.tensor_scalar(out=l2paT[:], in0=l2paT[:], scalar1=wsb['b2c2_0'][:],
                            scalar2=None, op0=OP.add)
    nc.vector.tensor_scalar(out=l2pbT[:], in0=l2pbT[:], scalar1=wsb['b2c2_1'][:],
                            scalar2=None, op0=OP.add)
    if dbg:
        nc.gpsimd.dma_start(dbg['l2pa'][:], l2paT[:])
        nc.gpsimd.dma_start(dbg['l2pb'][:], l2pbT[:])
    if stop_after == 6:
        bail()
        return

    # ---- SA3 ----
    NR3 = BC * S2
    x3a = big.tile([128, NR3], F32, name="x3a")
    x3b = big.tile([128, NR3], F32, name="x3b")
    s3 = sing.tile([128, 4], F32, name="s3")
    for half, (x3, xw, paw, pbw) in enumerate(
            ((x3a, 'A3x_Ta', 'A3pa_Ta', 'A3pb_Ta'),
             (x3b, 'A3x_Tb', 'A3pa_Tb', 'A3pb_Tb'))):
        ps3 = psum.tile([128, NR3], F32, name="ps3", tag="mm")
        nc.tensor.matmul(ps3[:], wsb[xw][:], nxT2[:], start=True, stop=False)
        nc.tensor.matmul(ps3[:], wsb[paw][:], l2paT[:], start=False, stop=False)
        nc.tensor.matmul(ps3[:], wsb[pbw][:], l2pbT[:], start=False, stop=True)
        nc.scalar.activation(out=x3[:], in_=ps3[:], func=AF.Copy,
                             accum_out=s3[:, 2 * half:2 * half + 1])
        sqt3 = work.tile([128, NR3], F32, name="sqt3", tag="sqt3")
        nc.scalar.activation(out=sqt3[:], in_=x3[:], func=AF.Square,
                             accum_out=s3[:, 2 * half + 1:2 * half + 2])
    nc.gpsimd.dma_start(cc3[0][:], s3[:])
    if not no_cc:
        nc.gpsimd.collective_compute("AllReduce", OP.add, replica_groups=RG,
                                     ins=[cc3[0][:]], outs=[cc3[1][:]])
    stat3 = sing.tile([128, 4], F32, name="stat3")
    nc.gpsimd.dma_start(stat3[:], cc3[0 if no_cc else 1][:])
    n3 = B * S2
    sc3a, bi3a = bn_affine(tc, bpool, stat3[:, 0:1], stat3[:, 1:2],
                           wsb['bn3_g_0'][:], wsb['bn3_be_0'][:], n3, 128, "bn3a")
    sc3b, bi3b = bn_affine(tc, bpool, stat3[:, 2:3], stat3[:, 3:4],
                           wsb['bn3_g_1'][:], wsb['bn3_be_1'][:], n3, 128, "bn3b")
    h3a = work.tile([128, NR3], F32, name="h3a")
    h3b = work.tile([128, NR3], F32, name="h3b")
    nc.scalar.activation(out=h3a[:], in_=x3a[:], func=AF.Relu, scale=sc3a[:], bias=bi3a[:])
    nc.scalar.activation(out=h3b[:], in_=x3b[:], func=AF.Relu, scale=sc3b[:], bias=bi3b[:])
    ga = sing.tile([128, BC], F32, name="ga")
    gb = sing.tile([128, BC], F32, name="gb")
    for half, g in ((0, ga), (1, gb)):
        psg3 = psum.tile([128, NR3], F32, name="psg3", tag="mm")
        nc.tensor.matmul(psg3[:], wsb[f'C3_{half}0'][:], h3a[:], start=True, stop=False)
        nc.tensor.matmul(psg3[:], wsb[f'C3_{half}1'][:], h3b[:], start=False, stop=True)
        nc.vector.tensor_reduce(out=g[:], in_=psg3[:].rearrange("p (c q) -> p c q", c=BC),
                                axis=AX.X, op=OP.max)
        nc.vector.tensor_scalar(out=g[:], in0=g[:],
                                scalar1=wsb[f'b3c2_{half}'][:],
                                scalar2=None, op0=OP.add)
    if dbg:
        nc.gpsimd.dma_start(dbg['ga'][:], ga[:])
        nc.gpsimd.dma_start(dbg['gb'][:], gb[:])
    if stop_after == 7:
        bail()
        return

    # ---- AllGather + FC head ----
    nc.gpsimd.dma_start(gg[0][0].rearrange("c p -> p c"), ga[:])
    nc.gpsimd.dma_start(gg[0][1].rearrange("c p -> p c"), gb[:])
    if not no_cc:
        nc.gpsimd.collective_compute("AllGather", OP.bypass, replica_groups=RG,
                                     ins=[gg[0][:]], outs=[gg[1][:]])
    for n in range(NCORES):
        ggsrc = gg[0] if no_cc else gg[1][n]
        nc.gpsimd.dma_start(gre[0, :, BC * n:BC * (n + 1)],
                            ggsrc[0].rearrange("c p -> p c"))
        nc.gpsimd.dma_start(gre[1, :, BC * n:BC * (n + 1)],
                            ggsrc[1].rearrange("c p -> p c"))
    gaal = sing.tile([128, B], F32, name="gaal")
    gbal = sing.tile([128, B], F32, name="gbal")
    nc.gpsimd.dma_start(gaal[:], gre[0])
    nc.gpsimd.dma_start(gbal[:], gre[1])

    def fc_layer(xins, wnames, gslice, beslice, name, alpha=0.2):
        ps = psum.tile([128, B], F32, name=f"{name}ps", tag="mm")
        for i, (xt, wn) in enumerate(zip(xins, wnames)):
            nc.tensor.matmul(ps[:], wsb[wn][:], xt[:], start=(i == 0),
                             stop=(i == len(xins) - 1))
        xsb = work.tile([128, B], F32, name=f"{name}x", tag=f"{name}x")
        ssq = sing.tile([128, 2], F32, name=f"{name}ssq")
        nc.scalar.activation(out=xsb[:], in_=ps[:], func=AF.Copy,
                             accum_out=ssq[:, 0:1])
        sqf = work.tile([128, B], F32, name=f"{name}sq", tag=f"{name}sq")
        nc.scalar.activation(out=sqf[:], in_=xsb[:], func=AF.Square,
                             accum_out=ssq[:, 1:2])
        sc, bi = bn_affine(tc, bpool, ssq[:, 0:1], ssq[:, 1:2], gslice, beslice,
                           B, 128, name)
        act = work.tile([128, B], F32, name=f"{name}act", tag=f"{name}act")
        vv = work.tile([128, B], F32, name=f"{name}vv", tag=f"{name}vv")
        nc.scalar.activation(out=vv[:], in_=xsb[:], func=AF.Identity,
                             scale=sc[:], bias=bi[:])
        av = work.tile([128, B], F32, name=f"{name}av", tag=f"{name}av")
        nc.vector.tensor_scalar_mul(out=av[:], in0=vv[:], scalar1=alpha)
        nc.vector.tensor_tensor(out=act[:], in0=vv[:], in1=av[:], op=OP.max)
        return act

    h1a = fc_layer([gaal, gbal], ['FC1_00', 'FC1_01'],
                   wsb['fbn1_g_0'][:], wsb['fbn1_be_0'][:], "fc1a")
    h1b = fc_layer([gaal, gbal], ['FC1_10', 'FC1_11'],
                   wsb['fbn1_g_1'][:], wsb['fbn1_be_1'][:], "fc1b")
    h2f = fc_layer([h1a, h1b], ['FC2_0', 'FC2_1'],
                   wsb['fbn2_g'][:], wsb['fbn2_be'][:], "fc2")
    ps_o = psum.tile([1, B], F32, name="ps_o", tag="mm")
    nc.tensor.matmul(ps_o[:], wsb['FC3_T'][:], h2f[:])
    o_sb = sing.tile([1, B], F32, name="o_sb")
    nc.vector.tensor_scalar(out=o_sb[:], in0=ps_o[:], scalar1=wsb['fc3_b'][:],
                            scalar2=None, op0=OP.add)
    nc.gpsimd.dma_start(out_d[:, 0][None, :], o_sb[:])


# ===================== host-side entry point =====================
_NC_CACHE = {}


def _get_nc():
    if 'nc' not in _NC_CACHE:
        _NC_CACHE['nc'] = build_nc(debug=False)
    return _NC_CACHE['nc']


def _kernel_numpy(inputs):
    """Exact numpy fallback of the reference model (host-side)."""
    f = np.float32
    pts = np.asarray(inputs['points'], f)
    Bn, Nn = pts.shape[0], pts.shape[1]

    def fps(x, npoint):
        n = x.shape[0]
        xs_, ys_, zs_ = x[:, 0], x[:, 1], x[:, 2]
        dist = np.full(n, 1e10, f)
        idxs = np.zeros(npoint, np.int64)
        far = 0
        for i in range(npoint):
            idxs[i] = far
            c = x[far]
            e = ((xs_ - c[0]) ** 2).astype(f) + ((ys_ - c[1]) ** 2).astype(f)
            dist = np.minimum(dist, (e + ((zs_ - c[2]) ** 2).astype(f)).astype(f))
            far = int(np.argmax(dist))
        return idxs

    def knn(q, r, k):
        d = (np.sum(q ** 2, -1)[:, None] - 2.0 * (q @ r.T) + np.sum(r ** 2, -1)[None, :])
        return np.argsort(d, axis=1, kind='stable')[:, :k]

    def bn(x, g, b, axes):
        m = x.mean(axes, keepdims=True, dtype=np.float64).astype(f)
        v = x.var(axes, keepdims=True).astype(f)
        return (x - m) / np.sqrt(v + 1e-5) * g + b

    def mlp2(x, w1, b1, g1, be1, w2, b2, axes):
        h = x @ np.asarray(w1, f).T + b1
        h = np.maximum(bn(h, g1, be1, axes), 0)
        return h @ np.asarray(w2, f).T + b2

    def sa_knn(xyz, ptsf, npoint, k, w1, b1, g1, be1, w2, b2):
        nx_l, np_l, gx_l, gp_l = [], [], [], []
        for b_ in range(xyz.shape[0]):
            fi = fps(xyz[b_], npoint)
            nxb = xyz[b_][fi]
            idx = knn(nxb, xyz[b_], k)
            gx_l.append(xyz[b_][idx] - nxb[:, None, :])
            gp_l.append(ptsf[b_][idx])
            nx_l.append(nxb)
        nxa = np.stack(nx_l); gx = np.stack(gx_l); gp = np.stack(gp_l)
        grouped = np.concatenate([gx, gp], -1)
        out = mlp2(grouped, w1, b1, g1, be1, w2, b2, (0, 1, 2))
        return nxa, out.max(2)

    i = {k: np.asarray(v, f) for k, v in inputs.items()}
    l1x, l1p = sa_knn(pts, pts, 512, 16, i['sa1_c1_w'], i['sa1_c1_b'],
                      i['sa1_bn_g'], i['sa1_bn_be'], i['sa1_c2_w'], i['sa1_c2_b'])
    l2x, l2p = sa_knn(l1x, l1p, 128, 16, i['sa2_c1_w'], i['sa2_c1_b'],
                      i['sa2_bn_g'], i['sa2_bn_be'], i['sa2_c2_w'], i['sa2_c2_b'])
    grouped = np.concatenate([l2x, l2p], -1)[:, None]
    g = mlp2(grouped, i['sa3_c1_w'], i['sa3_c1_b'], i['sa3_bn_g'], i['sa3_bn_be'],
             i['sa3_c2_w'], i['sa3_c2_b'], (0, 1, 2)).max(2)[:, 0]

    def lrelu(x):
        return np.where(x > 0, x, 0.2 * x)
    h = g @ i['fc1_w'].T + i['fc1_b']
    h = lrelu(bn(h, i['fc1_bn_g'], i['fc1_bn_be'], (0,)))
    h = h @ i['fc2_w'].T + i['fc2_b']
    h = lrelu(bn(h, i['fc2_bn_g'], i['fc2_bn_be'], (0,)))
    return (h @ i['fc3_w'].T + i['fc3_b']).astype(f)


def kernel(**inputs):
    """Full-input entry: shard over 8 NeuronCores, run, return (16,1) logits."""
    try:
        from concourse.bass_utils import run_bass_kernel_spmd
        w = prep_common_weights(inputs)
        pts = np.asarray(inputs['points'], np.float32)
        in_maps = []
        for t in range(NCORES):
            m = {'points': np.ascontiguousarray(pts[BC * t:BC * (t + 1)])}
            for name, shp in WEIGHT_SHAPES.items():
                m[name] = np.ascontiguousarray(w[name].reshape(shp))
            in_maps.append(m)
        nc = _get_nc()
        res = run_bass_kernel_spmd(nc, in_maps, list(range(NCORES)))
        out = np.asarray(res.results[0]['out'], np.float32)
        return out
    except Exception:
        import traceback
        traceback.print_exc()
        return _kernel_numpy(inputs)



# revision 62
# speedup vs baseline: 1.0515x; 1.0205x over previous
"""PointCloudDiscriminator Trainium2 Bass kernel (SPMD 8 cores, 2 clouds/core)."""
import numpy as np
from contextlib import ExitStack

import concourse.bass as bass
import concourse.bacc as bacc_mod
import concourse.tile as tile
import concourse.mybir as mybir
from concourse import bass_isa

F32 = mybir.dt.float32
U32 = mybir.dt.uint32
I16 = mybir.dt.int16
AF = mybir.ActivationFunctionType
OP = mybir.AluOpType
AX = mybir.AxisListType
EPS = 1e-5

B, N, S1, S2, K = 16, 8192, 512, 128, 16
NCORES = 8
BC = B // NCORES
C1A, C1B = 64, 128
C2A, C2B = 128, 256


def prep_common_weights(inp):
    f = np.float32
    w = {}
    A = lambda x: np.ascontiguousarray(np.asarray(x, f))
    pad4 = lambda a: np.concatenate([a, np.zeros((1, a.shape[1]), f)], 0)
    w1 = A(inp['sa1_c1_w'])
    w['w1s_T'] = A((w1[:, :3] + w1[:, 3:]).T)
    w['w1x_T'] = A(w1[:, :3].T)
    w['w2_T'] = A(np.asarray(inp['sa1_c2_w'], f).T)
    w['bn1_g'] = A(inp['sa1_bn_g']); w['bn1_be'] = A(inp['sa1_bn_be'])
    w['b1c2'] = A(inp['sa1_c2_b'])
    w2c1 = A(inp['sa2_c1_w'])
    w['A2x_T'] = pad4(A(w2c1[:, :3].T)); w['A2xn_T'] = A(-w['A2x_T'])
    w['A2p_T'] = A(w2c1[:, 3:].T)
    w['bn2_g'] = A(inp['sa2_bn_g']); w['bn2_be'] = A(inp['sa2_bn_be'])
    w2c2 = A(inp['sa2_c2_w'])
    w['B2a_T'] = A(w2c2[:128].T); w['B2b_T'] = A(w2c2[128:].T)
    w['b2c2'] = A(inp['sa2_c2_b'])
    w3c1 = A(inp['sa3_c1_w'])
    w['A3x_Ta'] = pad4(A(w3c1[:128, :3].T)); w['A3x_Tb'] = pad4(A(w3c1[128:, :3].T))
    w['A3pa_Ta'] = A(w3c1[:128, 3:131].T); w['A3pa_Tb'] = A(w3c1[128:, 3:131].T)
    w['A3pb_Ta'] = A(w3c1[:128, 131:259].T); w['A3pb_Tb'] = A(w3c1[128:, 131:259].T)
    w['bn3_g'] = A(inp['sa3_bn_g']); w['bn3_be'] = A(inp['sa3_bn_be'])
    w3c2 = A(inp['sa3_c2_w'])
    for r in range(2):
        for c in range(2):
            w[f'C3_{r}{c}'] = A(w3c2[128 * r:128 * (r + 1), 128 * c:128 * (c + 1)].T)
    w['b3c2'] = A(inp['sa3_c2_b'])
    f1 = A(inp['fc1_w'])
    for r in range(2):
        for c in range(2):
            w[f'FC1_{r}{c}'] = A(f1[128 * r:128 * (r + 1), 128 * c:128 * (c + 1)].T)
    w['fbn1_g'] = A(inp['fc1_bn_g']); w['fbn1_be'] = A(inp['fc1_bn_be'])
    f2 = A(inp['fc2_w'])
    w['FC2_0'] = A(f2[:, :128].T); w['FC2_1'] = A(f2[:, 128:].T)
    w['fbn2_g'] = A(inp['fc2_bn_g']); w['fbn2_be'] = A(inp['fc2_bn_be'])
    w['FC3_T'] = A(np.asarray(inp['fc3_w'], f).T)
    w['fc3_b'] = A(inp['fc3_b'])
    w['ident'] = np.eye(128, dtype=f)
    w['constrow'] = np.stack([np.full(1024, -1.0, f), np.zeros(1024, f)])
    return w


WEIGHT_SHAPES = {
    'w1s_T': (3, 64), 'w1x_T': (3, 64), 'w2_T': (64, 128),
    'bn1_g': (64,), 'bn1_be': (64,), 'b1c2': (128,),
    'A2x_T': (4, 128), 'A2xn_T': (4, 128), 'A2p_T': (128, 128),
    'bn2_g': (128,), 'bn2_be': (128,),
    'B2a_T': (128, 128), 'B2b_T': (128, 128), 'b2c2': (256,),
    'A3x_Ta': (4, 128), 'A3x_Tb': (4, 128),
    'A3pa_Ta': (128, 128), 'A3pa_Tb': (128, 128),
    'A3pb_Ta': (128, 128), 'A3pb_Tb': (128, 128),
    'bn3_g': (256,), 'bn3_be': (256,),
    'C3_00': (128, 128), 'C3_01': (128, 128), 'C3_10': (128, 128), 'C3_11': (128, 128),
    'b3c2': (256,),
    'FC1_00': (128, 128), 'FC1_01': (128, 128), 'FC1_10': (128, 128), 'FC1_11': (128, 128),
    'fbn1_g': (256,), 'fbn1_be': (256,),
    'FC2_0': (128, 128), 'FC2_1': (128, 128),
    'fbn2_g': (128,), 'fbn2_be': (128,),
    'FC3_T': (128, 1), 'fc3_b': (1,),
    'ident': (128, 128),
    'constrow': (2, 1024),
}


def build_nc(debug=False, no_cc=False, stop_after=None):
    nc = bacc_mod.Bacc()
    d = {'points': nc.dram_tensor("points", (BC, N, 3), F32, kind="ExternalInput")}
    for name, shp in WEIGHT_SHAPES.items():
        d[name] = nc.dram_tensor(name, shp, F32, kind="ExternalInput")
    out_d = nc.dram_tensor("out", (B, 1), F32, kind="ExternalOutput")
    F1d = [nc.dram_tensor(f"F1d{c}", (N, C1A), F32) for c in range(BC)]
    pre = nc.dram_tensor("pre", (3, 128, 128), F32)
    xyzTre = [nc.dram_tensor(f"xyzTre{c}", (4, N), F32) for c in range(BC)]
    nxTre = nc.dram_tensor("nxTre", (3, BC * S1), F32)
    pre2 = nc.dram_tensor("pre2", (3, 128, 8), F32)
    nxT2re = nc.dram_tensor("nxT2re", (3, BC * S2), F32)
    xyzT2re = [nc.dram_tensor(f"xyzT2re{c}", (4, S1), F32) for c in range(BC)]
    gre = nc.dram_tensor("gre", (2, 128, B), F32)
    nxd = nc.dram_tensor("nxd", (BC, S1, 3), F32)
    nxsd = nc.dram_tensor("nxsd", (128, 3, S1), F32)
    nxsd2 = nc.dram_tensor("nxsd2", (128, 3, S2), F32)
    F2d = [nc.dram_tensor(f"F2d{c}", (S1, 192), F32) for c in range(BC)]
    x1d = nc.dram_tensor("x1d", (BC, C1A, N), F32)
    x2d = nc.dram_tensor("x2d", (BC, C2A, S2 * K), F32)
    nx2d = nc.dram_tensor("nx2d", (BC, S2, 3), F32)
    cc1i = nc.dram_tensor("cc1i", (C1A, 2), F32)
    cc1o = nc.dram_tensor("cc1o", (C1A, 2), F32, addr_space="Shared")
    cc2i = nc.dram_tensor("cc2i", (C2A, 2), F32)
    cc2o = nc.dram_tensor("cc2o", (C2A, 2), F32, addr_space="Shared")
    cc3i = nc.dram_tensor("cc3i", (128, 4), F32)
    cc3o = nc.dram_tensor("cc3o", (128, 4), F32, addr_space="Shared")
    ggi = nc.dram_tensor("ggi", (2, BC, 128), F32)
    ggo = nc.dram_tensor("ggo", (NCORES, 2, BC, 128), F32, addr_space="Shared")
    dbg = {}
    if debug:
        dbg['nx'] = nc.dram_tensor("dbg_nx", (BC, S1, 3), F32, kind="ExternalOutput")
        dbg['idx1'] = nc.dram_tensor("dbg_idx1", (BC, 16, S1), U32, kind="ExternalOutput")
        dbg['l1p'] = nc.dram_tensor("dbg_l1p", (C1B, BC * S1), F32, kind="ExternalOutput")
        dbg['nx2'] = nc.dram_tensor("dbg_nx2", (BC, S2, 3), F32, kind="ExternalOutput")
        dbg['idx2'] = nc.dram_tensor("dbg_idx2", (BC, 16, S2), U32, kind="ExternalOutput")
        dbg['l2pa'] = nc.dram_tensor("dbg_l2pa", (128, BC * S2), F32, kind="ExternalOutput")
        dbg['l2pb'] = nc.dram_tensor("dbg_l2pb", (128, BC * S2), F32, kind="ExternalOutput")
        dbg['ga'] = nc.dram_tensor("dbg_ga", (128, BC), F32, kind="ExternalOutput")
        dbg['gb'] = nc.dram_tensor("dbg_gb", (128, BC), F32, kind="ExternalOutput")
    with tile.TileContext(nc) as tc:
        with nc.allow_non_contiguous_dma(reason="small strided restaging DMAs"), ExitStack() as ctx:
            emit(ctx, tc, d, out_d, F1d, nxd, F2d, nx2d, x1d, x2d,
                 (pre, xyzTre, nxTre, pre2, nxT2re, xyzT2re, gre, nxsd, nxsd2),
                 (cc1i, cc1o), (cc2i, cc2o), (cc3i, cc3o), (ggi, ggo), dbg, no_cc,
                 stop_after)
    nc.compile()
    return nc


def fps_loop(ctx, tc, pool, xs, ys, zs, xyzneg, nx, nsteps, free, name, ones1, psum,
             nxs=None):
    nc = tc.nc
    dist = pool.tile([128, free], F32, name=f"{name}_dist")
    nc.vector.memset(dist[:], 1e10)
    cneg = pool.tile([128, 3], F32, name=f"{name}_cneg")
    m8 = pool.tile([128, 8], F32, name=f"{name}_m8")
    gm = pool.tile([128, 1], F32, name=f"{name}_gm")
    r = pool.tile([128, 3], F32, name=f"{name}_r")
    junk = pool.tile([128, free], F32, name=f"{name}_junk")
    e1 = pool.tile([128, free], F32, name=f"{name}_e1")
    e2 = pool.tile([128, free], F32, name=f"{name}_e2")
    e3 = pool.tile([128, free], F32, name=f"{name}_e3")
    aa = pool.tile([128, free], F32, name=f"{name}_aa")
    ind0 = pool.tile([128, free], F32, name=f"{name}_ind0")
    nc.vector.memset(ind0[:], 0.0)
    nc.vector.memset(ind0[0:1, 0:1], 1.0)
    nc.vector.memset(ind0[64:65, 0:1], 1.0)
    # hi-half (partition 64:128) slices of partition_all_reduce return zeros on
    # HW, so route per-cloud reductions through disjoint COLUMNS of full-128
    # reduces: m2 packs per-cloud maxima, r6 per-cloud coordinate sums.
    m2 = pool.tile([128, 2], F32, name=f"{name}_m2")
    nc.vector.memset(m2[:], -1e30)
    gm2 = pool.tile([128, 2], F32, name=f"{name}_gm2")
    r6 = pool.tile([128, 6], F32, name=f"{name}_r6")
    nc.vector.memset(r6[:], 0.0)
    c6 = pool.tile([128, 6], F32, name=f"{name}_c6")

    def extract_c(mask_src, scal):
        # r[p,d] = sum_f (mask==scal ? -coord); full-128 add -> cneg everywhere
        for dd in range(3):
            nc.vector.scalar_tensor_tensor(
                out=junk[:], in0=mask_src, scalar=scal, in1=xyzneg[:, dd, :],
                op0=OP.is_equal, op1=OP.mult, accum_out=r[:, dd:dd + 1])
        nc.vector.tensor_copy(out=r6[0:64, 0:3], in_=r[0:64, :])
        nc.vector.tensor_copy(out=r6[64:128, 3:6], in_=r[64:128, :])
        nc.gpsimd.partition_all_reduce(c6[:], r6[:], 128, bass_isa.ReduceOp.add)
        nc.vector.tensor_copy(out=cneg[0:64, :], in_=c6[0:64, 0:3])
        nc.vector.tensor_copy(out=cneg[64:128, :], in_=c6[64:128, 3:6])

    def record(i):
        if nxs is not None:
            # all partitions hold their cloud-half's reduced value; negate into
            # the transposed [128, 3, nsteps] buffer
            nc.vector.tensor_scalar_mul(out=nxs[:, :, i], in0=cneg[:],
                                        scalar1=-1.0)
        else:
            nc.scalar.activation(out=nx[0:1, i, :], in_=cneg[0:1, :],
                                 func=AF.Copy, scale=-1.0)
            nc.scalar.activation(out=nx[64:65, i, :], in_=cneg[64:65, :],
                                 func=AF.Copy, scale=-1.0)

    extract_c(ind0[:], 1.0)
    record(0)
    for i in range(1, nsteps):
        nc.scalar.activation(out=e1[:], in_=xyzneg[:, 0, :], func=AF.Square,
                             scale=-1.0, bias=cneg[:, 0:1])
        nc.scalar.activation(out=e2[:], in_=xyzneg[:, 1, :], func=AF.Square,
                             scale=-1.0, bias=cneg[:, 1:2])
        nc.scalar.activation(out=e3[:], in_=xyzneg[:, 2, :], func=AF.Square,
                             scale=-1.0, bias=cneg[:, 2:3])
        nc.vector.tensor_tensor(out=aa[:], in0=e1[:], in1=e2[:], op=OP.add)
        nc.vector.tensor_tensor(out=e1[:], in0=aa[:], in1=e3[:], op=OP.add)
        nc.vector.tensor_tensor(out=dist[:], in0=dist[:], in1=e1[:], op=OP.min)
        nc.vector.max(m8[:], dist[:])
        nc.vector.tensor_copy(out=m2[0:64, 0:1], in_=m8[0:64, 0:1])
        nc.vector.tensor_copy(out=m2[64:128, 1:2], in_=m8[64:128, 0:1])
        nc.gpsimd.partition_all_reduce(gm2[:], m2[:], 128, bass_isa.ReduceOp.max)
        nc.vector.tensor_copy(out=gm[0:64, :], in_=gm2[0:64, 0:1])
        nc.vector.tensor_copy(out=gm[64:128, :], in_=gm2[64:128, 1:2])
        extract_c(dist[:], gm[:, 0:1])
        record(i)


def fps_loop_split(ctx, tc, pool, xyzsrc, nsteps, vw, name, nxs, on_chunk=None,
                   chunk=None, on_step=None):
    """Per-cloud FPS chains: cloud c uses its own [128, vw] tiles spanning all
    128 partitions (point idx = p*vw + f), so reductions are full-128 (the only
    partition_all_reduce form that works on HW). The two chains interleave on
    the engines. xyzsrc(c, dd) -> DRAM AP of cloud c's coord row, (128, vw).
    Records into nxs[128, 3, nsteps] partition-halves (downstream layout
    unchanged: cloud c at partition 64*c)."""
    nc = tc.nc
    mf = max(vw, 8)
    T = {}
    for c in range(2):
        xyzneg = pool.tile([128, 3, vw], F32, name=f"{name}_xyzn{c}")
        for dd in range(3):
            xt = pool.tile([128, vw], F32, name=f"{name}_x{c}{dd}")
            nc.gpsimd.dma_start(xt[:], xyzsrc(c, dd))
            nc.vector.tensor_scalar_mul(out=xyzneg[:, dd, :], in0=xt[:],
                                        scalar1=-1.0)
        dist = pool.tile([128, mf], F32, name=f"{name}_dist{c}")
        nc.vector.memset(dist[:], 1e10)
        if mf > vw:
            nc.vector.memset(dist[:, vw:mf], -1e30)
        # per-step extract history: the add-reduce writes straight into
        # hist[:, :, i]; step i+1's bias reads hist[:, d, i]; one bulk negate
        # after the loop replaces per-step record ops
        hist = pool.tile([128, 3, nsteps], F32, name=f"{name}_hist{c}")
        m8 = pool.tile([128, 8], F32, name=f"{name}_m8{c}")
        gm = pool.tile([128, 1], F32, name=f"{name}_gm{c}")
        r = pool.tile([128, 3], F32, name=f"{name}_r{c}")
        junk = pool.tile([128, vw], F32, name=f"{name}_junk{c}")
        e1 = pool.tile([128, vw], F32, name=f"{name}_e1{c}")
        e2 = pool.tile([128, vw], F32, name=f"{name}_e2{c}")
        e3 = pool.tile([128, vw], F32, name=f"{name}_e3{c}")
        aa = pool.tile([128, vw], F32, name=f"{name}_aa{c}")
        ind0 = pool.tile([128, vw], F32, name=f"{name}_ind0{c}")
        nc.vector.memset(ind0[:], 0.0)
        nc.vector.memset(ind0[0:1, 0:1], 1.0)
        T[c] = (xyzneg, dist, hist, m8, gm, r, junk, e1, e2, e3, aa, ind0)

    def extract_c(c, mask_src, scal, i):
        xyzneg, dist, hist, m8, gm, r, junk = T[c][:7]
        for dd in range(3):
            nc.vector.scalar_tensor_tensor(
                out=junk[:], in0=mask_src, scalar=scal, in1=xyzneg[:, dd, :],
                op0=OP.is_equal, op1=OP.mult, accum_out=r[:, dd:dd + 1])
        nc.gpsimd.partition_all_reduce(hist[:, :, i], r[:], 128,
                                       bass_isa.ReduceOp.add)

    for c in range(2):
        extract_c(c, T[c][11][:], 1.0, 0)
    # phase-interleaved emission: both clouds' reduces are in flight before
    # either cloud's dependent phase queues, so the in-order engine queues
    # overlap the two serial chains.
    for i in range(1, nsteps):
        for c in range(2):
            xyzneg, dist, hist, m8, gm, r, junk, e1, e2, e3, aa, ind0 = T[c]
            nc.scalar.activation(out=e1[:], in_=xyzneg[:, 0, :], func=AF.Square,
                                 scale=-1.0, bias=hist[:, 0:1, i - 1])
            nc.scalar.activation(out=e2[:], in_=xyzneg[:, 1, :], func=AF.Square,
                                 scale=-1.0, bias=hist[:, 1:2, i - 1])
            nc.scalar.activation(out=e3[:], in_=xyzneg[:, 2, :], func=AF.Square,
                                 scale=-1.0, bias=hist[:, 2:3, i - 1])
            nc.vector.tensor_tensor(out=aa[:], in0=e1[:], in1=e2[:], op=OP.add)
            nc.vector.tensor_tensor(out=e1[:], in0=aa[:], in1=e3[:], op=OP.add)
            nc.vector.tensor_tensor(out=dist[:, 0:vw], in0=dist[:, 0:vw],
                                    in1=e1[:], op=OP.min)
            nc.vector.max(m8[:], dist[:])
            nc.gpsimd.partition_all_reduce(gm[:], m8[:, 0:1], 128,
                                           bass_isa.ReduceOp.max)
        if on_step is not None:
            # emitted between the max-reduce issue and the dependent extract,
            # so the drained KNN piece runs during the gpsimd round-trip
            on_step()
        for c in range(2):
            extract_c(c, T[c][1][:, 0:vw], T[c][4][:, 0:1], i)
        if chunk is not None and (i + 1) % chunk == 0:
            # chunk of samples complete: negate its history slice into nxs and
            # hand off (e.g. to emit the KNN tiles that only need these queries)
            j = (i + 1) // chunk - 1
            for c in range(2):
                nc.vector.tensor_scalar_mul(
                    out=nxs[64 * c:64 * (c + 1), :, chunk * j:chunk * (j + 1)],
                    in0=T[c][2][64 * c:64 * (c + 1), :, chunk * j:chunk * (j + 1)],
                    scalar1=-1.0)
            if on_chunk is not None:
                on_chunk(j)
    if chunk is None:
        # bulk negate the per-step history into the shared nxs record buffer
        for c in range(2):
            nc.vector.tensor_scalar_mul(
                out=nxs[64 * c:64 * (c + 1), :, :],
                in0=T[c][2][64 * c:64 * (c + 1), :, :], scalar1=-1.0)


def bn_affine(tc, pool, sums, sqs, g_sb, be_sb, count, cpart, name):
    nc = tc.nc
    mean = pool.tile([cpart, 1], F32, name=f"{name}_mean")
    var = pool.tile([cpart, 1], F32, name=f"{name}_var")
    scale = pool.tile([cpart, 1], F32, name=f"{name}_scale")
    bias = pool.tile([cpart, 1], F32, name=f"{name}_bias")
    tmp = pool.tile([cpart, 1], F32, name=f"{name}_tmp")
    inv_n = 1.0 / float(count)
    nc.scalar.mul(mean[:], sums, inv_n)
    nc.scalar.mul(var[:], sqs, inv_n)
    nc.vector.tensor_tensor(out=tmp[:], in0=mean[:], in1=mean[:], op=OP.mult)
    nc.vector.tensor_tensor(out=var[:], in0=var[:], in1=tmp[:], op=OP.subtract)
    nc.vector.tensor_scalar_add(out=var[:], in0=var[:], scalar1=EPS)
    nc.vector.reciprocal(tmp[:], var[:])
    nc.scalar.activation(out=tmp[:], in_=tmp[:], func=AF.Sqrt)
    nc.vector.tensor_tensor(out=scale[:], in0=tmp[:], in1=g_sb, op=OP.mult)
    nc.vector.tensor_tensor(out=tmp[:], in0=mean[:], in1=scale[:], op=OP.mult)
    nc.vector.tensor_tensor(out=bias[:], in0=be_sb, in1=tmp[:], op=OP.subtract)
    scale_a = pool.tile([cpart, 1], F32, name=f"{name}_scale_a")
    bias_a = pool.tile([cpart, 1], F32, name=f"{name}_bias_a")
    nc.scalar.activation(out=scale_a[:], in_=scale[:], func=AF.Copy)
    nc.scalar.activation(out=bias_a[:], in_=bias[:], func=AF.Copy)
    return scale_a, bias_a


def emit(ctx, tc, d, out_d, F1d, nxd, F2d, nx2d, x1d, x2d, stg, cc1, cc2, cc3, gg, dbg,
         no_cc=False, stop_after=None):
    pre, xyzTre, nxTre, pre2, nxT2re, xyzT2re, gre, nxsd, nxsd2 = stg
    nc = tc.nc

    def bail():
        zout = sing.tile([16, 1], F32, name="zout")
        nc.vector.memset(zout[:], 0.0)
        nc.gpsimd.dma_start(out_d[:], zout[:])
    P = 128
    RG = [list(range(NCORES))]
    sing = ctx.enter_context(tc.tile_pool(name="sing", bufs=1))
    big = ctx.enter_context(tc.tile_pool(name="big", bufs=1))
    work = ctx.enter_context(tc.tile_pool(name="work", bufs=1))
    psum = ctx.enter_context(tc.tile_pool(name="psum", bufs=3, space="PSUM"))
    psumT = ctx.enter_context(tc.tile_pool(name="psumT", bufs=3, space="PSUM"))
    bpool = ctx.enter_context(tc.tile_pool(name="bnp", bufs=1))


    ones1 = sing.tile([1, 128], F32, name="ones1")
    nc.vector.memset(ones1[:], 1.0)
    wsb = {}
    for name, shp in WEIGHT_SHAPES.items():
        if len(shp) == 1:
            if shp[0] > 128:
                for hh in range(shp[0] // 128):
                    t = sing.tile([128, 1], F32, name=f"w_{name}_{hh}")
                    nc.gpsimd.dma_start(t[:], d[name][128 * hh:128 * (hh + 1), None])
                    wsb[f"{name}_{hh}"] = t
                continue
            t = sing.tile([shp[0], 1], F32, name=f"w_{name}")
            nc.gpsimd.dma_start(t[:], d[name][:, None])
        else:
            t = sing.tile(list(shp), F32, name=f"w_{name}")
            nc.gpsimd.dma_start(t[:], d[name][:])
        wsb[name] = t

    # ---- points load (restage so each SBUF tile = ONE DMA) ----
    for dd in range(3):
        for c in range(BC):
            nc.gpsimd.dma_start(
                pre[dd, 64 * c:64 * (c + 1), :],
                d['points'][c, :, dd].rearrange("(p f) -> p f", p=64))
    xs = sing.tile([P, 128], F32, name="xs")
    ys = sing.tile([P, 128], F32, name="ys")
    zs = sing.tile([P, 128], F32, name="zs")
    for dd, t in enumerate((xs, ys, zs)):
        nc.gpsimd.dma_start(t[:], pre[dd])
    xyzneg = sing.tile([P, 3, 128], F32, name="xyzneg")
    for dd, t in enumerate((xs, ys, zs)):
        nc.vector.tensor_scalar_mul(out=xyzneg[:, dd, :], in0=t[:], scalar1=-1.0)
    sqt0 = work.tile([P, 128], F32, name="sqt0", tag="sqt0")
    rnf = sing.tile([P, 128], F32, name="rnf")
    nc.scalar.activation(out=rnf[:], in_=xyzneg[:, 0, :], func=AF.Square)
    nc.scalar.activation(out=sqt0[:], in_=xyzneg[:, 1, :], func=AF.Square)
    nc.vector.tensor_tensor(out=rnf[:], in0=rnf[:], in1=sqt0[:], op=OP.add)
    nc.scalar.activation(out=sqt0[:], in_=xyzneg[:, 2, :], func=AF.Square)
    nc.vector.tensor_tensor(out=rnf[:], in0=rnf[:], in1=sqt0[:], op=OP.add)
    # xyzT staging: rows xyz from points, row3 = rn (per cloud), all in DRAM
    for c in range(BC):
        for dd, t in enumerate((xs, ys, zs)):
            nc.gpsimd.dma_start(xyzTre[c][dd:dd + 1, :], t[64 * c:64 * (c + 1), :])
        nc.gpsimd.dma_start(xyzTre[c][3:4, :], rnf[64 * c:64 * (c + 1), :])
    xyzTt = sing.tile([4, N], F32, name="xyzTt")

    def fill_xyzT(c):
        nc.gpsimd.dma_start(xyzTt[:], xyzTre[c][:])

    # ---- F1 rows-major -> F1d ----
    for c in range(BC):
        fill_xyzT(c)
        for j in range(8):
            ps = psum.tile([P, 512], F32, name="f1ps", tag="mm")
            st = work.tile([P, 512], F32, name="f1st", tag="f1st")
            for jj in range(8):
                ch = 8 * j + jj
                nc.tensor.matmul(ps[:, 64 * jj:64 * (jj + 1)],
                                 xyzTt[0:3, 128 * ch:128 * (ch + 1)],
                                 wsb['w1s_T'][:])
            nc.scalar.activation(out=st[:], in_=ps[:], func=AF.Copy)
            nc.gpsimd.dma_start(
                F1d[c][:].rearrange("(j p) q -> p j q", p=128)[:, 8 * j:8 * (j + 1), :],
                st[:].rearrange("p (j q) -> p j q", j=8))

    if stop_after == 1:
        bail()
        return

    # ---- FPS1 with KNN1 tiles emitted per 128-sample chunk so the KNN
    # matmul/gather/scan work fills FPS1's idle engine time ----
    nxs = sing.tile([P, 3, S1], F32, name="nxs")
    nxT = sing.tile([3, BC * S1], F32, name="nxT")
    q4T = sing.tile([4, BC * S1], F32, name="q4T")
    nc.gpsimd.dma_start(q4T[3:4, :], d['constrow'][0:1, :])
    Gc = sing.tile([C1A, BC * S1], F32, name="Gc")
    scores = big.tile([P, N], F32, name="scores")
    sum1 = sing.tile([C1A, 128], F32, name="sum1")
    sq1 = sing.tile([C1A, 128], F32, name="sq1")
    l1pT = big.tile([C1B, BC * S1], F32, name="l1pT")
    fpool = ctx.enter_context(tc.tile_pool(name="fps1", bufs=1))

    def stage_chunk1(j):
        # queries 128j..128(j+1) of each cloud are final: stage nxsd/nxT/q4T/Gc
        # for them, then emit their two KNN tiles (t=j cloud 0, t=4+j cloud 1)
        nc.gpsimd.dma_start(nxsd[:, :, 128 * j:128 * (j + 1)],
                            nxs[:, :, 128 * j:128 * (j + 1)])
        for c in range(BC):
            q0 = S1 * c + 128 * j
            nc.gpsimd.dma_start(nxT[:, q0:q0 + 128],
                                nxsd[64 * c][:, 128 * j:128 * (j + 1)])
            nc.vector.tensor_scalar_mul(out=q4T[0:3, q0:q0 + 128],
                                        in0=nxT[:, q0:q0 + 128], scalar1=2.0)
            psg = psum.tile([C1A, 128], F32, name="gcps", tag="mm")
            nc.tensor.matmul(psg[:], wsb['w1x_T'][:], nxT[:, q0:q0 + 128])
            nc.scalar.activation(out=Gc[:, q0:q0 + 128], in_=psg[:], func=AF.Copy)
        knn_q.append(emit_knn1_tile(j))
        knn_q.append(emit_knn1_tile(4 + j))

    knn_q = []

    def drain_knn(n=1):
        # advance the pending KNN generators by n pieces (called once per FPS
        # step so KNN work lands in FPS1's per-step latency bubbles)
        for _ in range(n):
            while knn_q:
                try:
                    next(knn_q[0])
                    return
                except StopIteration:
                    knn_q.pop(0)
            return

    def emit_knn1_tile(t):
        c = t // 4
        fill_xyzT(c)
        for jj in range(16):
            ps = psum.tile([P, 512], F32, name="knnps", tag="mm")
            nc.tensor.matmul(ps[:], q4T[:, 128 * t:128 * (t + 1)],
                             xyzTt[:, 512 * jj:512 * (jj + 1)])
            nc.scalar.activation(out=scores[:, 512 * jj:512 * (jj + 1)], in_=ps[:],
                                  func=AF.Copy)
            yield
        m8a = work.tile([P, 8], F32, name="m8a", tag="m8a")
        m8b = work.tile([P, 8], F32, name="m8b", tag="m8b")
        ia = work.tile([P, 16], U32, name="iab", tag="iab")
        nc.vector.max(m8a[:], scores[:])
        yield
        nc.vector.max_index(ia[:, 0:8], m8a[:], scores[:])
        yield
        nc.vector.match_replace(scores[:], m8a[:], scores[:], -1e30)
        yield
        nc.vector.max(m8b[:], scores[:])
        yield
        nc.vector.max_index(ia[:, 8:16], m8b[:], scores[:])
        yield
        if dbg:
            iaf = work.tile([P, 16], F32, name="iaf", tag="iaf")
            nc.vector.tensor_copy(out=iaf[:], in_=ia[:])
            pst = psumT.tile([16, P], F32, name="idxps", tag="T")
            nc.tensor.transpose(pst[:], iaf[:], wsb['ident'][:])
            dcp = work.tile([16, P], U32, name="dcp", tag="dcp")
            nc.vector.tensor_copy(out=dcp[:], in_=pst[:])
            nc.gpsimd.dma_start(dbg['idx1'][c, :, 128 * (t % 4):128 * (t % 4 + 1)],
                              dcp[:])
        # gather + conv1-space blocks, k-major columns: col = 512*k + 128*(t%4) + q
        for k in range(K):
            gblk = work.tile([P, C1A], F32, name="gblk", tag=f"gblk_{k % 2}")
            nc.gpsimd.indirect_dma_start(
                out=gblk[:], out_offset=None, in_=F1d[c][:],
                in_offset=bass.IndirectOffsetOnAxis(ap=ia[:, k:k + 1], axis=0))
            psx1 = psumT.tile([C1A, P], F32, name="psx1", tag="T")
            nc.tensor.transpose(psx1[:], gblk[:], wsb['ident'][:])
            q0 = S1 * c + 128 * (t % 4)
            xblk = work.tile([C1A, P], F32, name="xblk", tag="xblk")
            nc.vector.scalar_tensor_tensor(
                out=xblk[:], in0=psx1[:], scalar=0.0,
                in1=Gc[:, q0:q0 + 128],
                op0=OP.bypass, op1=OP.subtract,
                accum_out=sum1[:, 64 * c + 16 * (t % 4) + k:64 * c + 16 * (t % 4) + k + 1])
            sqt = work.tile([C1A, P], F32, name="sqt", tag="sqt")
            nc.scalar.activation(
                out=sqt[:], in_=xblk[:], func=AF.Square,
                accum_out=sq1[:, 64 * c + 16 * (t % 4) + k:64 * c + 16 * (t % 4) + k + 1])
            nc.gpsimd.dma_start(
                x1d[c, :, 512 * k + 128 * (t % 4):512 * k + 128 * (t % 4) + 128],
                xblk[:])
            yield

    if stop_after == 22:
        nc.vector.memset(nxs[:], 0.25)
        bail()
        return
    fps_loop_split(ctx, tc, fpool,
                   lambda c, dd: xyzTre[c][dd, :].rearrange("(p f) -> p f", p=128),
                   S1, 64, "f1", nxs, on_chunk=stage_chunk1, chunk=128,
                   on_step=drain_knn)
    while knn_q:
        drain_knn()
    if dbg:
        for c in range(BC):
            nc.gpsimd.dma_start(dbg['nx'][c],
                                nxsd[64 * c].rearrange("dd q -> q dd"))
    if stop_after in (2, 21):
        bail()
        return

    # FPS2 emitted here (only needs nxsd): the last KNN1 chunk and the
    # BN1-collective round-trip hide inside its step latency bubbles
    nxs2 = sing.tile([P, 3, S2], F32, name="nxs2")
    fpool2 = ctx.enter_context(tc.tile_pool(name="fps2", bufs=1))
    fps_loop_split(ctx, tc, fpool2,
                   lambda c, dd: nxsd[64 * c, dd, :].rearrange(
                       "(p f) -> p f", p=128),
                   S2, 4, "f2", nxs2, on_step=drain_knn, chunk=S2)
    nc.gpsimd.dma_start(nxsd2[:], nxs2[:])
    if dbg:
        for c in range(BC):
            nc.gpsimd.dma_start(dbg['nx2'][c],
                                nxsd2[64 * c].rearrange("dd q -> q dd"))

    nxT2f = sing.tile([4, BC * S2], F32, name="nxT2f")
    for c in range(BC):
        nc.gpsimd.dma_start(nxT2f[0:3, S2 * c:S2 * (c + 1)], nxsd2[64 * c])
    nc.gpsimd.dma_start(nxT2f[3:4, :], d['constrow'][1:2, 0:BC * S2])
    nxT2 = nxT2f
    q4T2 = sing.tile([3, BC * S2], F32, name="q4T2")
    nc.vector.tensor_scalar_mul(out=q4T2[:], in0=nxT2[0:3, :], scalar1=2.0)
    monerow = sing.tile([1, 128], F32, name="monerow")
    nc.gpsimd.dma_start(monerow[:], d['constrow'][0:1, 0:128])
    xyzT2 = [sing.tile([3, S1], F32, name=f"xyzT2_{c}") for c in range(BC)]
    rn2ts = [sing.tile([1, S1], F32, name=f"rn2t_{c}") for c in range(BC)]
    ones3 = sing.tile([3, 1], F32, name="ones3")
    nc.vector.memset(ones3[:], 1.0)
    for c in range(BC):
        nc.gpsimd.dma_start(xyzT2[c][:], nxsd[64 * c])
        sq2t = work.tile([3, S1], F32, name="sq2t", tag="sq2t")
        nc.scalar.activation(out=sq2t[:], in_=xyzT2[c][:], func=AF.Square)
        psr = psum.tile([1, S1], F32, name="rnps", tag="mm")
        nc.tensor.matmul(psr[:], ones3[:], sq2t[:])
        nc.vector.tensor_copy(out=rn2ts[c][:], in_=psr[:])


    red1 = sing.tile([C1A, 2], F32, name="red1")
    nc.vector.tensor_reduce(out=red1[:, 0:1], in_=sum1[:, None, :], axis=AX.X, op=OP.add)
    nc.vector.tensor_reduce(out=red1[:, 1:2], in_=sq1[:, None, :], axis=AX.X, op=OP.add)
    nc.gpsimd.dma_start(cc1[0][:], red1[:])
    if stop_after == 3:
        bail()
        return
    if not no_cc:
        nc.gpsimd.collective_compute("AllReduce", OP.add, replica_groups=RG,
                                     ins=[cc1[0][:]], outs=[cc1[1][:]])
    stat1 = sing.tile([C1A, 2], F32, name="stat1")
    nc.gpsimd.dma_start(stat1[:], cc1[0 if no_cc else 1][:])
    sc1, bi1 = bn_affine(tc, bpool, stat1[:, 0:1], stat1[:, 1:2],
                         wsb['bn1_g'][:], wsb['bn1_be'][:], B * S1 * K, C1A, "bn1")

    for c in range(BC):
        for k in range(K):
            col = 512 * k
            x1c = work.tile([C1A, 512], F32, name="x1c", tag="x1c")
            nc.gpsimd.dma_start(x1c[:], x1d[c, :, col:col + 512])
            h1 = work.tile([C1A, 512], F32, name="h1", tag="h1")
            nc.scalar.activation(out=h1[:], in_=x1c[:], func=AF.Relu,
                                 scale=sc1[:], bias=bi1[:])
            ps = psum.tile([C1B, 512], F32, name="c2ps", tag="mm")
            nc.tensor.matmul(ps[:], wsb['w2_T'][:], h1[:])
            sl = l1pT[:, S1 * c:S1 * (c + 1)]
            if k == 0:
                nc.vector.tensor_copy(out=sl, in_=ps[:])
            else:
                nc.vector.tensor_tensor(out=sl, in0=sl, in1=ps[:], op=OP.max)
    nc.vector.tensor_scalar(out=l1pT[:], in0=l1pT[:], scalar1=wsb['b1c2'][:],
                            scalar2=None, op0=OP.add)
    if dbg:
        nc.gpsimd.dma_start(dbg['l1p'][:], l1pT[:])
    if stop_after == 4:
        bail()
        return

    # ---- SA2 prep ----
    zpad = sing.tile([128, 60], F32, name="zpad")
    nc.vector.memset(zpad[:], 0.0)
    for c in range(BC):
        nc.gpsimd.dma_start(F2d[c][:, 0:3],
                            nxsd[64 * c].rearrange("dd q -> q dd"))
        for j in range(4):
            nc.gpsimd.dma_start(F2d[c][128 * j:128 * (j + 1), 3], zpad[:, 0:1])
            nc.gpsimd.dma_start(F2d[c][128 * j:128 * (j + 1), 132:192], zpad[:])
        for j in range(4):
            pst = psumT.tile([P, P], F32, name="ftps", tag="T")
            nc.tensor.transpose(pst[:], l1pT[:, S1 * c + 128 * j:S1 * c + 128 * (j + 1)],
                                wsb['ident'][:])
            stg = work.tile([P, P], F32, name="fstg", tag="fstg")
            nc.vector.tensor_copy(out=stg[:], in_=pst[:])
            nc.gpsimd.dma_start(F2d[c][128 * j:128 * (j + 1), 4:132], stg[:])

    if stop_after == 5:
        bail()
        return

    # ---- KNN2 + gather + MLP2 ----
    sum2 = sing.tile([C2A, 8], F32, name="sum2")
    sq2 = sing.tile([C2A, 8], F32, name="sq2")
    l2paT = big.tile([128, BC * S2], F32, name="l2paT")
    x2sb = big.tile([C2A, BC * S2 * K], F32, name="x2sb")
    l2pbT = big.tile([128, BC * S2], F32, name="l2pbT")

    for c in range(BC):
        ps = psum.tile([P, S1], F32, name="kn2ps", tag="mm")
        nc.tensor.matmul(ps[:], q4T2[:, S2 * c:S2 * (c + 1)], xyzT2[c][:],
                         start=True, stop=False)
        nc.tensor.matmul(ps[:], monerow[:], rn2ts[c][:], start=False, stop=True)
        sc2t = work.tile([P, S1], F32, name="sc2t", tag="sc2t")
        nc.scalar.activation(out=sc2t[:], in_=ps[:], func=AF.Copy)
        m8a = work.tile([P, 8], F32, name="m8a2", tag="m8a2")
        m8b = work.tile([P, 8], F32, name="m8b2", tag="m8b2")
        ia = work.tile([P, 16], U32, name="iab2", tag="iab2")
        nc.vector.max(m8a[:], sc2t[:])
        nc.vector.max_index(ia[:, 0:8], m8a[:], sc2t[:])
        nc.vector.match_replace(sc2t[:], m8a[:], sc2t[:], -1e30)
        nc.vector.max(m8b[:], sc2t[:])
        nc.vector.max_index(ia[:, 8:16], m8b[:], sc2t[:])
        if dbg:
            iaf2 = work.tile([P, 16], F32, name="iaf2", tag="iaf2")
            nc.vector.tensor_copy(out=iaf2[:], in_=ia[:])
            pst2 = psumT.tile([16, P], F32, name="idx2ps", tag="T")
            nc.tensor.transpose(pst2[:], iaf2[:], wsb['ident'][:])
            dcp2 = work.tile([16, P], U32, name="dcp2", tag="dcp2")
            nc.vector.tensor_copy(out=dcp2[:], in_=pst2[:])
            nc.gpsimd.dma_start(dbg['idx2'][c], dcp2[:])
        rhx = big.tile([4, S2 * K], F32, name="rhx", tag="rhx")
        rhp = big.tile([C2A, S2 * K], F32, name="rhp", tag="rhp")
        for k in range(K):
            gblk2 = work.tile([P, 192], F32, name="gblk2", tag=f"gblk2_{k % 2}")
            nc.gpsimd.indirect_dma_start(
                out=gblk2[:], out_offset=None, in_=F2d[c][:],
                in_offset=bass.IndirectOffsetOnAxis(ap=ia[:, k:k + 1], axis=0))
            psx = psumT.tile([4, P], F32, name="psx", tag="T")
            nc.tensor.transpose(psx[:], gblk2[:, 0:4], wsb['ident'][:])
            nc.vector.tensor_copy(out=rhx[:, 128 * k:128 * (k + 1)], in_=psx[:])
            psp = psumT.tile([C2A, P], F32, name="psp", tag="T")
            nc.tensor.transpose(psp[:], gblk2[:, 4:132], wsb['ident'][:])
            nc.vector.tensor_copy(out=rhp[:, 128 * k:128 * (k + 1)], in_=psp[:])
        for chk in range(4):
            col = 512 * chk
            ps2 = psum.tile([C2A, 512], F32, name="c1ps2", tag="mm")
            nc.tensor.matmul(ps2[:], wsb['A2x_T'][:], rhx[:, col:col + 512],
                             start=True, stop=False)
            nc.tensor.matmul(ps2[:], wsb['A2p_T'][:], rhp[:, col:col + 512],
                             start=False, stop=False)
            nc.tensor.matmul(
                ps2[:], wsb['A2xn_T'][:],
                nxT2[:, S2 * c:S2 * (c + 1)][:, None, :].broadcast_to((4, 4, S2)),
                start=False, stop=True)
            x2col = S2 * K * c + col
            nc.scalar.activation(out=x2sb[:, x2col:x2col + 512], in_=ps2[:],
                                 func=AF.Copy,
                                 accum_out=sum2[:, 4 * c + chk:4 * c + chk + 1])
            sqt2 = work.tile([C2A, 512], F32, name="sqt2", tag="sqt2")
            nc.scalar.activation(out=sqt2[:], in_=x2sb[:, x2col:x2col + 512],
                                 func=AF.Square,
                                 accum_out=sq2[:, 4 * c + chk:4 * c + chk + 1])

    red2 = sing.tile([C2A, 2], F32, name="red2")
    nc.vector.tensor_reduce(out=red2[:, 0:1], in_=sum2[:, None, :], axis=AX.X, op=OP.add)
    nc.vector.tensor_reduce(out=red2[:, 1:2], in_=sq2[:, None, :], axis=AX.X, op=OP.add)
    nc.gpsimd.dma_start(cc2[0][:], red2[:])
    if not no_cc:
        nc.gpsimd.collective_compute("AllReduce", OP.add, replica_groups=RG,
                                     ins=[cc2[0][:]], outs=[cc2[1][:]])
    stat2 = sing.tile([C2A, 2], F32, name="stat2")
    nc.gpsimd.dma_start(stat2[:], cc2[0 if no_cc else 1][:])
    sc2, bi2 = bn_affine(tc, bpool, stat2[:, 0:1], stat2[:, 1:2],
                         wsb['bn2_g'][:], wsb['bn2_be'][:], B * S2 * K, C2A, "bn2")

    for c in range(BC):
        for chk in range(4):
            col = S2 * K * c + 512 * chk
            h2 = work.tile([C2A, 512], F32, name="h2", tag="h2")
            nc.scalar.activation(out=h2[:], in_=x2sb[:, col:col + 512],
                                 func=AF.Relu, scale=sc2[:], bias=bi2[:])
            psa = psum.tile([128, 512], F32, name="c2psa", tag="mm")
            nc.tensor.matmul(psa[:], wsb['B2a_T'][:], h2[:])
            psb = psum.tile([128, 512], F32, name="c2psb", tag="mm")
            nc.tensor.matmul(psb[:], wsb['B2b_T'][:], h2[:])
            for half, (pp, ll) in enumerate(((psa, l2paT), (psb, l2pbT))):
                sl = ll[:, S2 * c:S2 * (c + 1)]
                for kk in range(4):
                    yk = pp[:, 128 * kk:128 * (kk + 1)]
                    if chk == 0 and kk == 0:
                        nc.vector.tensor_copy(out=sl, in_=yk)
                    else:
                        nc.vector.tensor_tensor(out=sl, in0=sl, in1=yk, op=OP.max)
    nc.vector.tensor_scalar(out=l2paT[:], in0=l2paT[:], scalar1=wsb['b2c2_0'][:],
                            scalar2=None, op0=OP.add)
    nc.vector.tensor_scalar(out=l2pbT[:], in0=l2pbT[:], scalar1=wsb['b2c2_1'][:],
                            scalar2=None, op0=OP.add)
    if dbg:
        nc.gpsimd.dma_start(dbg['l2pa'][:], l2paT[:])
        nc.gpsimd.dma_start(dbg['l2pb'][:], l2pbT[:])
    if stop_after == 6:
        bail()
        return

    # ---- SA3 ----
    NR3 = BC * S2
    x3a = big.tile([128, NR3], F32, name="x3a")
    x3b = big.tile([128, NR3], F32, name="x3b")
    s3 = sing.tile([128, 4], F32, name="s3")
    for half, (x3, xw, paw, pbw) in enumerate(
            ((x3a, 'A3x_Ta', 'A3pa_Ta', 'A3pb_Ta'),
             (x3b, 'A3x_Tb', 'A3pa_Tb', 'A3pb_Tb'))):
        ps3 = psum.tile([128, NR3], F32, name="ps3", tag="mm")
        nc.tensor.matmul(ps3[:], wsb[xw][:], nxT2[:], start=True, stop=False)
        nc.tensor.matmul(ps3[:], wsb[paw][:], l2paT[:], start=False, stop=False)
        nc.tensor.matmul(ps3[:], wsb[pbw][:], l2pbT[:], start=False, stop=True)
        nc.scalar.activation(out=x3[:], in_=ps3[:], func=AF.Copy,
                             accum_out=s3[:, 2 * half:2 * half + 1])
        sqt3 = work.tile([128, NR3], F32, name="sqt3", tag="sqt3")
        nc.scalar.activation(out=sqt3[:], in_=x3[:], func=AF.Square,
                             accum_out=s3[:, 2 * half + 1:2 * half + 2])
    nc.gpsimd.dma_start(cc3[0][:], s3[:])
    if not no_cc:
        nc.gpsimd.collective_compute("AllReduce", OP.add, replica_groups=RG,
                                     ins=[cc3[0][:]], outs=[cc3[1][:]])
    stat3 = sing.tile([128, 4], F32, name="stat3")
    nc.gpsimd.dma_start(stat3[:], cc3[0 if no_cc else 1][:])
    n3 = B * S2
    sc3a, bi3a = bn_affine(tc, bpool, stat3[:, 0:1], stat3[:, 1:2],
                           wsb['bn3_g_0'][:], wsb['bn3_be_0'][:], n3, 128, "bn3a")
    sc3b, bi3b = bn_affine(tc, bpool, stat3[:, 2:3], stat3[:, 3:4],
                           wsb['bn3_g_1'][:], wsb['bn3_be_1'][:], n3, 128, "bn3b")
    h3a = work.tile([128, NR3], F32, name="h3a")
    h3b = work.tile([128, NR3], F32, name="h3b")
    nc.scalar.activation(out=h3a[:], in_=x3a[:], func=AF.Relu, scale=sc3a[:], bias=bi3a[:])
    nc.scalar.activation(out=h3b[:], in_=x3b[:], func=AF.Relu, scale=sc3b[:], bias=bi3b[:])
    ga = sing.tile([128, BC], F32, name="ga")
    gb = sing.tile([128, BC], F32, name="gb")
    for half, g in ((0, ga), (1, gb)):
        psg3 = psum.tile([128, NR3], F32, name="psg3", tag="mm")
        nc.tensor.matmul(psg3[:], wsb[f'C3_{half}0'][:], h3a[:], start=True, stop=False)
        nc.tensor.matmul(psg3[:], wsb[f'C3_{half}1'][:], h3b[:], start=False, stop=True)
        nc.vector.tensor_reduce(out=g[:], in_=psg3[:].rearrange("p (c q) -> p c q", c=BC),
                                axis=AX.X, op=OP.max)
        nc.vector.tensor_scalar(out=g[:], in0=g[:],
                                scalar1=wsb[f'b3c2_{half}'][:],
                                scalar2=None, op0=OP.add)
    if dbg:
        nc.gpsimd.dma_start(dbg['ga'][:], ga[:])
        nc.gpsimd.dma_start(dbg['gb'][:], gb[:])
    if stop_after == 7:
        bail()
        return

    # ---- AllGather + FC head ----
    nc.gpsimd.dma_start(gg[0][0].rearrange("c p -> p c"), ga[:])
    nc.gpsimd.dma_start(gg[0][1].rearrange("c p -> p c"), gb[:])
    if not no_cc:
        nc.gpsimd.collective_compute("AllGather", OP.bypass, replica_groups=RG,
                                     ins=[gg[0][:]], outs=[gg[1][:]])
    for n in range(NCORES):
        ggsrc = gg[0] if no_cc else gg[1][n]
        nc.gpsimd.dma_start(gre[0, :, BC * n:BC * (n + 1)],
                            ggsrc[0].rearrange("c p -> p c"))
        nc.gpsimd.dma_start(gre[1, :, BC * n:BC * (n + 1)],
                            ggsrc[1].rearrange("c p -> p c"))
    gaal = sing.tile([128, B], F32, name="gaal")
    gbal = sing.tile([128, B], F32, name="gbal")
    nc.gpsimd.dma_start(gaal[:], gre[0])
    nc.gpsimd.dma_start(gbal[:], gre[1])

    def fc_layer(xins, wnames, gslice, beslice, name, alpha=0.2):
        ps = psum.tile([128, B], F32, name=f"{name}ps", tag="mm")
        for i, (xt, wn) in enumerate(zip(xins, wnames)):
            nc.tensor.matmul(ps[:], wsb[wn][:], xt[:], start=(i == 0),
                             stop=(i == len(xins) - 1))
        xsb = work.tile([128, B], F32, name=f"{name}x", tag=f"{name}x")
        ssq = sing.tile([128, 2], F32, name=f"{name}ssq")
        nc.scalar.activation(out=xsb[:], in_=ps[:], func=AF.Copy,
                             accum_out=ssq[:, 0:1])
        sqf = work.tile([128, B], F32, name=f"{name}sq", tag=f"{name}sq")
        nc.scalar.activation(out=sqf[:], in_=xsb[:], func=AF.Square,
                             accum_out=ssq[:, 1:2])
        sc, bi = bn_affine(tc, bpool, ssq[:, 0:1], ssq[:, 1:2], gslice, beslice,
                           B, 128, name)
        act = work.tile([128, B], F32, name=f"{name}act", tag=f"{name}act")
        vv = work.tile([128, B], F32, name=f"{name}vv", tag=f"{name}vv")
        nc.scalar.activation(out=vv[:], in_=xsb[:], func=AF.Identity,
                             scale=sc[:], bias=bi[:])
        av = work.tile([128, B], F32, name=f"{name}av", tag=f"{name}av")
        nc.vector.tensor_scalar_mul(out=av[:], in0=vv[:], scalar1=alpha)
        nc.vector.tensor_tensor(out=act[:], in0=vv[:], in1=av[:], op=OP.max)
        return act

    h1a = fc_layer([gaal, gbal], ['FC1_00', 'FC1_01'],
                   wsb['fbn1_g_0'][:], wsb['fbn1_be_0'][:], "fc1a")
    h1b = fc_layer([gaal, gbal], ['FC1_10', 'FC1_11'],
                   wsb['fbn1_g_1'][:], wsb['fbn1_be_1'][:], "fc1b")
    h2f = fc_layer([h1a, h1b], ['FC2_0', 'FC2_1'],
                   wsb['fbn2_g'][:], wsb['fbn2_be'][:], "fc2")
    ps_o = psum.tile([1, B], F32, name="ps_o", tag="mm")
    nc.tensor.matmul(ps_o[:], wsb['FC3_T'][:], h2f[:])
    o_sb = sing.tile([1, B], F32, name="o_sb")
    nc.vector.tensor_scalar(out=o_sb[:], in0=ps_o[:], scalar1=wsb['fc3_b'][:],
                            scalar2=None, op0=OP.add)
    nc.gpsimd.dma_start(out_d[:, 0][None, :], o_sb[:])


# ===================== host-side entry point =====================
_NC_CACHE = {}


def _get_nc():
    if 'nc' not in _NC_CACHE:
        _NC_CACHE['nc'] = build_nc(debug=False)
    return _NC_CACHE['nc']


def _kernel_numpy(inputs):
    """Exact numpy fallback of the reference model (host-side)."""
    f = np.float32
    pts = np.asarray(inputs['points'], f)
    Bn, Nn = pts.shape[0], pts.shape[1]

    def fps(x, npoint):
        n = x.shape[0]
        xs_, ys_, zs_ = x[:, 0], x[:, 1], x[:, 2]
        dist = np.full(n, 1e10, f)
        idxs = np.zeros(npoint, np.int64)
        far = 0
        for i in range(npoint):
            idxs[i] = far
            c = x[far]
            e = ((xs_ - c[0]) ** 2).astype(f) + ((ys_ - c[1]) ** 2).astype(f)
            dist = np.minimum(dist, (e + ((zs_ - c[2]) ** 2).astype(f)).astype(f))
            far = int(np.argmax(dist))
        return idxs

    def knn(q, r, k):
        d = (np.sum(q ** 2, -1)[:, None] - 2.0 * (q @ r.T) + np.sum(r ** 2, -1)[None, :])
        return np.argsort(d, axis=1, kind='stable')[:, :k]

    def bn(x, g, b, axes):
        m = x.mean(axes, keepdims=True, dtype=np.float64).astype(f)
        v = x.var(axes, keepdims=True).astype(f)
        return (x - m) / np.sqrt(v + 1e-5) * g + b

    def mlp2(x, w1, b1, g1, be1, w2, b2, axes):
        h = x @ np.asarray(w1, f).T + b1
        h = np.maximum(bn(h, g1, be1, axes), 0)
        return h @ np.asarray(w2, f).T + b2

    def sa_knn(xyz, ptsf, npoint, k, w1, b1, g1, be1, w2, b2):
        nx_l, np_l, gx_l, gp_l = [], [], [], []
        for b_ in range(xyz.shape[0]):
            fi = fps(xyz[b_], npoint)
            nxb = xyz[b_][fi]
            idx = knn(nxb, xyz[b_], k)
            gx_l.append(xyz[b_][idx] - nxb[:, None, :])
            gp_l.append(ptsf[b_][idx])
            nx_l.append(nxb)
        nxa = np.stack(nx_l); gx = np.stack(gx_l); gp = np.stack(gp_l)
        grouped = np.concatenate([gx, gp], -1)
        out = mlp2(grouped, w1, b1, g1, be1, w2, b2, (0, 1, 2))
        return nxa, out.max(2)

    i = {k: np.asarray(v, f) for k, v in inputs.items()}
    l1x, l1p = sa_knn(pts, pts, 512, 16, i['sa1_c1_w'], i['sa1_c1_b'],
                      i['sa1_bn_g'], i['sa1_bn_be'], i['sa1_c2_w'], i['sa1_c2_b'])
    l2x, l2p = sa_knn(l1x, l1p, 128, 16, i['sa2_c1_w'], i['sa2_c1_b'],
                      i['sa2_bn_g'], i['sa2_bn_be'], i['sa2_c2_w'], i['sa2_c2_b'])
    grouped = np.concatenate([l2x, l2p], -1)[:, None]
    g = mlp2(grouped, i['sa3_c1_w'], i['sa3_c1_b'], i['sa3_bn_g'], i['sa3_bn_be'],
             i['sa3_c2_w'], i['sa3_c2_b'], (0, 1, 2)).max(2)[:, 0]

    def lrelu(x):
        return np.where(x > 0, x, 0.2 * x)
    h = g @ i['fc1_w'].T + i['fc1_b']
    h = lrelu(bn(h, i['fc1_bn_g'], i['fc1_bn_be'], (0,)))
    h = h @ i['fc2_w'].T + i['fc2_b']
    h = lrelu(bn(h, i['fc2_bn_g'], i['fc2_bn_be'], (0,)))
    return (h @ i['fc3_w'].T + i['fc3_b']).astype(f)


def kernel(**inputs):
    """Full-input entry: shard over 8 NeuronCores, run, return (16,1) logits."""
    try:
        from concourse.bass_utils import run_bass_kernel_spmd
        w = prep_common_weights(inputs)
        pts = np.asarray(inputs['points'], np.float32)
        in_maps = []
        for t in range(NCORES):
            m = {'points': np.ascontiguousarray(pts[BC * t:BC * (t + 1)])}
            for name, shp in WEIGHT_SHAPES.items():
                m[name] = np.ascontiguousarray(w[name].reshape(shp))
            in_maps.append(m)
        nc = _get_nc()
        res = run_bass_kernel_spmd(nc, in_maps, list(range(NCORES)))
        out = np.asarray(res.results[0]['out'], np.float32)
        return out
    except Exception:
        import traceback
        traceback.print_exc()
        return _kernel_numpy(inputs)

